# revision 54
# baseline (speedup 1.0000x reference)
"""PointsFusion Trainium2 kernel (optimized, v2).

Pipeline per batch b (B=4, N=4096, k=32):
  knn1 = 32-NN of p1 in p1, knn2 = 32-NN of p1 in p2 (exact, via DVE 8-max rounds)
  gather neighbor coords, features (resi, dist) -> conv(4->64)->BN->relu
  -> conv(64->64)->BN->relu -> conv(64->128)->BN->relu -> channel-max scores
  -> softmax over 64 neighbors -> weighted sum of neighbor coords.

Sharding: 8 cores = (batch b, half h of the 4096 query points). BatchNorm uses
global batch stats -> 3 tiny AllReduces of per-channel sum/sumsq.

v2 changes vs v1 (3.03ms):
  - phase 1: each (tile, kn) gets its OWN msb distance buffer, distances for
    both knns emitted eagerly, and the two top-k chains of a tile are
    round-interleaved so the DVE never stalls on its own serial chain
  - activation spills y1/y2/y3 + conv2/conv3 weights in fp16 (halves HBM
    traffic; fp16 keeps 0.05% precision so top-k stays exact in f32)
  - BN stats: per-chunk sums ride the PSUM->SBUF copies via accum_out;
    sumsq via one GpSimd scalar_tensor_tensor pass per tile (GpSimd is idle
    in phases 2/3) -- frees ~11us/tile of Scalar time
  - phase 4: channel-max as 2 partition_all_reduce of [128, 4096] instead of
    4 of [128, 2048] (amortizes the ~5us GpSimd handshake)

Layouts (per 128-query tile):
  pixel space: 16 chunks of 512; chunk c = kn*8+g, pixel j = c*512 + s*16 + p
  (g = query group, p = query-in-group, s = neighbor slot, kn = which knn).
  64-channel activations are packed [128, 4096]: chunk c lives at partitions
  64*(c%2)..+64, free 512*(c//2)..+512 (keeps matmul rhs bases in {0, 64}).

Self-contained: hardcodes shapes; no sibling imports.
"""

import sys

import numpy as np

for _p in ("/opt/trn_rl_repo", "/opt/pypackages"):
    if _p not in sys.path:
        sys.path.append(_p)

import concourse.bass as bass  # noqa: E402  (imported for side effects/typing)
import concourse.mybir as mybir  # noqa: E402
import concourse.tile as tile  # noqa: E402
from concourse import bacc, bass_isa  # noqa: E402
from concourse.bass_utils import run_bass_kernel_spmd  # noqa: E402
from concourse.masks import make_identity  # noqa: E402

F32 = mybir.dt.float32
F32R = mybir.dt.float32r
F16 = mybir.dt.float16
U16 = mybir.dt.uint16
I16 = mybir.dt.int16
AF = mybir.ActivationFunctionType
OP = mybir.AluOpType

NCORES = 8
B = 4
N = 4096          # candidate points per batch
KNN = 32          # neighbors per knn
QPC = 2048        # query points per core
NT = 16           # query tiles of 128 per core
C1, C2, C3 = 64, 64, 128
NTOT = float(B * N * 2 * KNN)   # BN stat count (global)
BN_EPS = 1e-3
NEG = -1.0e30

# HW-bisect flags (CoreSim passes all combos; some features hang real HW).
# partition_all_reduce crashes the device for free sizes > 2048 (ucode
# buffer limit) -- only the 2048-wide quarter variants are safe.
USE_TTR_SUMSQ = False    # tensor_tensor_reduce sumsq: CRASHES HW, keep False
USE_STT_SUMSQ = True     # sumsq via vector scalar_tensor_tensor (ph 2/3)
# "mixed" (gpsimd quarters + DVE shift-DMA max-tree) is numerically correct
# in CoreSim but produces wrong results on real HW -- do not use.
PAR_MODE = "f16q"        # f32q | f16q | mixed (gpsimd 3 quarters + DVE tree)
HYBRID_CMAX = True       # odd tiles: PE-transpose + DVE reduce channel-max


def _pk(cc):
    """packed [128, 4096] slice coords for chunk cc."""
    return 64 * (cc % 2), 512 * (cc // 2)


def _build_program(single=False):
    nc = bacc.Bacc(
        "TRN2", target_bir_lowering=False, debug=False,
        num_devices=1 if single else NCORES,
    )
    nc._single_core_nocoll = single

    ap = {}
    def din(name, shape, dt=F32):
        ap[name] = nc.dram_tensor(name, shape, dt, kind="ExternalInput").ap()
    din("qf", [4, QPC])
    din("t1", [4, N])
    din("t2", [4, N])
    din("gt", [128, N])
    din("gt2", [128, N])
    din("nqsq", [128, NT])
    din("w1t", [4, C1], F32R)
    din("w2t", [128, C2], F16)    # duplicated at partition 64
    din("w3t", [128, C3], F16)    # duplicated at partition 64
    din("gb1", [C1, 2])
    din("gb2", [C2, 2])
    din("gb3", [C3, 2])
    din("selw", [8, 128])
    din("termt", [16, NT * 8 * C1], F32R)
    din("sel16", [16, 512], F32R)
    din("eout", [128, 32])

    ap["out"] = nc.dram_tensor("out", [3, QPC], F32, kind="ExternalOutput").ap()

    ap["y1d"] = nc.dram_tensor("y1d", [NT, 128, 4096], F16).ap()
    ap["y2d"] = nc.dram_tensor("y2d", [NT, 128, 4096], F16).ap()
    ap["y3d"] = nc.dram_tensor("y3d", [NT, C3, 8192], F16).ap()
    ap["g1d"] = nc.dram_tensor("g1d", [NT, 128, 512], F32).ap()
    ap["g2d"] = nc.dram_tensor("g2d", [NT, 128, 512], F32).ap()
    for i, c in ((0, C1), (1, C2), (2, C3)):
        ap[f"arin{i}"] = nc.dram_tensor(f"arin{i}", [c * 2], F32).ap()
        ap[f"arout{i}"] = nc.dram_tensor(f"arout{i}", [c * 2], F32).ap()

    with tile.TileContext(nc) as tc:
        _kernel_body(tc, ap)
    nc.compile()
    return nc


def _kernel_body(tc, d):
    nc = tc.nc
    from contextlib import ExitStack

    ctx = ExitStack()
    with ctx:
        cpool = ctx.enter_context(tc.tile_pool(name="consts", bufs=1))
        w2 = cpool.tile([128, C2], F16)
        w3 = cpool.tile([128, C3], F16)
        gb1 = cpool.tile([C1, 2], F32)
        gb2 = cpool.tile([C2, 2], F32)
        gb3 = cpool.tile([C3, 2], F32)
        selw = cpool.tile([8, 128], F32)
        eout = cpool.tile([128, 32], F32)
        ident = cpool.tile([128, 128], F32)
        make_identity(nc, ident[:])
        ident16 = cpool.tile([128, 128], F16)
        nc.vector.tensor_copy(out=ident16[:], in_=ident[:])
        for nm, sb in [("w2t", w2), ("w3t", w3),
                       ("gb1", gb1), ("gb2", gb2), ("gb3", gb3),
                       ("selw", selw), ("eout", eout)]:
            nc.sync.dma_start(out=sb[:], in_=d[nm][:])

        spool = ctx.enter_context(tc.tile_pool(name="stats", bufs=1))
        sm1 = spool.tile([C1, NT * 16], F32)
        sq1 = spool.tile([128, NT], F32)
        sm2 = spool.tile([C2, NT * 16], F32)
        sq2 = spool.tile([128, NT], F32)
        sm3 = spool.tile([C3, NT * 16], F32)
        sq3 = spool.tile([C3, NT * 2], F32)
        ab1 = spool.tile([128, 2], F32)   # col0 = scale a, col1 = bias b (dup at 64)
        ab2 = spool.tile([128, 2], F32)
        ab3 = spool.tile([C3, 2], F32)

        # ---------------- Phase 1: knn + gather + feat + conv1 ----------------
        with tc.tile_pool(name="p1c", bufs=1) as p1c, \
             tc.tile_pool(name="p1m", bufs=2) as mpool, \
             tc.tile_pool(name="p1psum", bufs=3, space="PSUM") as pp, \
             tc.tile_pool(name="p1tp", bufs=1, space="PSUM") as tpp, \
             tc.tile_pool(name="p1cpsum", bufs=3, space="PSUM") as cp, \
             tc.tile_pool(name="p1feat", bufs=1) as fpool, \
             tc.tile_pool(name="p1work", bufs=3) as wp, \
             tc.tile_pool(name="p1tt", bufs=2) as ttp, \
             tc.tile_pool(name="p1y", bufs=2) as yp:
            # phase-1-only constants (pool closes after phase 1, freeing
            # SBUF for the later phases' double buffers)
            tt = p1c.tile([36, N], F32)     # t1 rows 0-3, t2 rows 32-35
            t1 = tt[0:4, :]
            t2 = tt[32:36, :]
            gt = p1c.tile([128, N], F32)
            gt2 = p1c.tile([128, N], F32)
            qfc = p1c.tile([36, QPC], F32)  # qf dup'd at rows 0-3 and 32-35
            nqsq = p1c.tile([128, NT], F32)
            w1 = p1c.tile([4, C1], F32R)
            sel16 = p1c.tile([16, 512], F32R)
            nc.sync.dma_start(out=tt[0:4, :], in_=d["t1"][:])
            nc.sync.dma_start(out=tt[32:36, :], in_=d["t2"][:])
            nc.sync.dma_start(out=qfc[0:4, :], in_=d["qf"][:])
            nc.sync.dma_start(out=qfc[32:36, :], in_=d["qf"][:])
            for nm, sb in [("gt", gt), ("gt2", gt2), ("nqsq", nqsq),
                           ("w1t", w1), ("sel16", sel16)]:
                nc.sync.dma_start(out=sb[:], in_=d[nm][:])
            msbs = {}

            def emit_dist(t, kn, msb):
                # distance matmuls + msb copies for (t, kn)
                tab = (t1, t2)[kn]
                qfk = qfc[32 * kn:32 * kn + 4, :]
                for ch in range(8):
                    pm = pp.tile([128, 512], F32, tag="pm")
                    nc.tensor.matmul(
                        out=pm[:],
                        lhsT=qfk[:, t * 128:(t + 1) * 128],
                        rhs=tab[:, ch * 512:(ch + 1) * 512],
                        start=True, stop=True,
                    )
                    nc.scalar.activation(
                        out=msb[:, ch * 512:(ch + 1) * 512], in_=pm[:],
                        func=AF.Identity, bias=nqsq[:, t:t + 1])

            def start_tile(t):
                for kn in (0, 1):
                    m = mpool.tile([128, N], F32, tag=f"msb{kn}")
                    msbs[(t, kn)] = m
                    emit_dist(t, kn, m)

            start_tile(0)
            for t in range(NT):
                # software pipeline: issue tile t+1's distance stages (both
                # knns) ahead of tile t's topk/conv1 chain
                if t + 1 < NT:
                    start_tile(t + 1)
                mA = msbs.pop((t, 0))
                mB = msbs.pop((t, 1))
                termt = ttp.tile([16, 8 * C1], F32R, tag="termt")
                nc.sync.dma_start(
                    out=termt[:],
                    in_=d["termt"][:, t * 8 * C1:(t + 1) * 8 * C1])
                vals = wp.tile([128, 64], F32, tag="vals")
                idxu = wp.tile([128, 64], U16, tag="idxu")
                idxi = wp.tile([128, 64], I16, tag="idxi")
                # two-level top-32 (exact except when one 128-candidate chunk
                # holds >8 of a query's true top-32: P ~ 3e-5 per query):
                #   L1: top-8 of each of 32 chunks of 128 -> 256 candidates
                #   L2: top-32 of the candidates via max8+match_replace rounds
                #   FIND: global indices via find_index8 on the full row
                # 32 chunks of 128: P(a query's true top-32 has >8 members in
                # one chunk) ~ 3e-5; 16 chunks of 256 pushes rel err over the
                # 2e-2 budget (measured 3.1e-2) -- keep 32.
                NCH = 32
                CW = N // NCH
                l1a = wp.tile([128, NCH * 8], F32, tag="l1v0")
                l1b = wp.tile([128, NCH * 8], F32, tag="l1v1")
                l1 = {0: l1a, 1: l1b}
                for c in range(NCH):
                    for kn, m in ((0, mA), (1, mB)):
                        nc.vector.max(
                            out=l1[kn][:, c * 8:(c + 1) * 8],
                            in_=m[:, c * CW:(c + 1) * CW])
                for r in range(4):
                    for kn in (0, 1):
                        v8 = vals[:, kn * 32 + r * 8: kn * 32 + r * 8 + 8]
                        nc.vector.max(out=v8, in_=l1[kn][:])
                    if r < 3:
                        for kn in (0, 1):
                            v8 = vals[:, kn * 32 + r * 8: kn * 32 + r * 8 + 8]
                            nc.vector.match_replace(
                                out=l1[kn][:], in_to_replace=v8,
                                in_values=l1[kn][:], imm_value=NEG)
                for r in range(4):
                    for kn, m in ((0, mA), (1, mB)):
                        v8 = vals[:, kn * 32 + r * 8: kn * 32 + r * 8 + 8]
                        i8 = idxu[:, kn * 32 + r * 8: kn * 32 + r * 8 + 8]
                        nc.vector.max_index(out=i8, in_max=v8, in_values=m[:])
                nc.vector.tensor_copy(out=idxi[:], in_=idxu[:])

                # gather neighbor coords; both tables carry xyz on band rows
                # 16g+{0..2} (gt = p1 for knn1, gt2 = p2 for knn2); spill raw
                # for the fusion phase
                g1 = wp.tile([128, 512], F32, tag="g1")
                g2 = wp.tile([128, 512], F32, tag="g2")
                nc.gpsimd.ap_gather(
                    out_ap=g1[:], in_ap=gt[:], idxs_ap=idxi[:, 0:32],
                    channels=128, num_elems=N, d=1, num_idxs=512)
                nc.gpsimd.ap_gather(
                    out_ap=g2[:], in_ap=gt2[:], idxs_ap=idxi[:, 32:64],
                    channels=128, num_elems=N, d=1, num_idxs=512)
                nc.sync.dma_start(out=d["g1d"][t], in_=g1[:])
                nc.sync.dma_start(out=d["g2d"][t], in_=g2[:])

                # conv1 rhs must start at partition 0: DMA bands into a flat
                # [4, 8192] tile (raw nn coords; the -q term is folded into
                # the conv1 matmul).  Band copies split across ACT / GpSimd
                # descriptor queues to keep them off the SP sequencer.
                feat = fpool.tile([4, 8192], F32R, tag="feat")
                for g in range(8):
                    nc.scalar.dma_start(
                        out=feat[0:3, g * 512:(g + 1) * 512],
                        in_=g1[16 * g: 16 * g + 3, :].bitcast(F32R))
                    nc.gpsimd.dma_start(
                        out=feat[0:3, (8 + g) * 512:(9 + g) * 512],
                        in_=g2[16 * g: 16 * g + 3, :].bitcast(F32R))

                # dist = sqrt(max(-val, 0)) into feat row 3
                d2 = wp.tile([128, 64], F32, tag="d2")
                nc.vector.tensor_scalar(
                    out=d2[:], in0=vals[:], scalar1=-1.0,
                    scalar2=0.0, op0=OP.mult, op1=OP.max)
                nc.scalar.activation(out=d2[:], in_=d2[:], func=AF.Sqrt)
                # shuffle dist to pixel layout: PE-transpose to [nbr, query],
                # then ONE batched DMA per knn half (dst iterates (s, g, p))
                dtp = tpp.tile([64, 128], F32, tag="dtp")
                nc.tensor.transpose(out=dtp[:], in_=d2[:], identity=ident[:])
                d2t = wp.tile([64, 128], F32, tag="d2t")
                nc.scalar.activation(out=d2t[:], in_=dtp[:], func=AF.Identity)
                for kn in (0, 1):
                    for g in range(8):
                        c = kn * 8 + g
                        eng = (nc.sync, nc.scalar, nc.gpsimd)[c % 3]
                        eng.dma_start(
                            out=feat[3:4, c * 512:(c + 1) * 512]
                                .rearrange("c (s p) -> c s p", s=32),
                            in_=d2t[kn * 32:(kn + 1) * 32,
                                    16 * g:16 * g + 16].bitcast(F32R))

                # conv1: 16 chunks -> y1 packed [128, 4096] fp16; second
                # matmul accumulates the host-precomputed -W1[:, :3] @ q term
                y1 = yp.tile([128, 4096], F16, tag="y1")
                for c in range(16):
                    g = c % 8
                    bp_, fo = _pk(c)
                    pc = cp.tile([C1, 512], F32, tag="pc1")
                    nc.tensor.matmul(
                        out=pc[:],
                        lhsT=w1[:],
                        rhs=feat[:, c * 512:(c + 1) * 512],
                        start=True, stop=False)
                    nc.tensor.matmul(
                        out=pc[:],
                        lhsT=termt[:, g * C1:(g + 1) * C1],
                        rhs=sel16[:],
                        start=False, stop=True)
                    nc.scalar.activation(
                        out=y1[bp_:bp_ + 64, fo:fo + 512], in_=pc[:],
                        func=AF.Identity,
                        accum_out=sm1[:, t * 16 + c: t * 16 + c + 1])
                # sumsq pass; output recycles the (dead) mA tile
                nc.scalar.activation(
                    out=mA[:].bitcast(F16)[:, 0:4096], in_=y1[:],
                    func=AF.Square, accum_out=sq1[:, t:t + 1])
                nc.sync.dma_start(out=d["y1d"][t], in_=y1[:])

        _bn_allreduce(tc, 0, sm1, sq1, gb1, ab1, d["arin0"], d["arout0"],
                      dup=True, fold_sq=True, fold_sm=False)

        # ---------------- Phase 2: apply BN1+relu, conv2 ----------------
        with tc.tile_pool(name="p2y", bufs=3) as yp, \
             tc.tile_pool(name="p2psum", bufs=6, space="PSUM") as cp:
            for t in range(NT):
                y1 = yp.tile([128, 4096], F16, tag="y1l")
                nc.sync.dma_start(out=y1[:], in_=d["y1d"][t])
                y1r = yp.tile([128, 4096], F16, tag="y1r")
                # bn1+relu on DVE (two f16 4x ops) -- keeps the ACT engine on
                # Identity only, avoiding per-tile function-table reloads
                nc.vector.tensor_scalar(
                    out=y1r[:], in0=y1[:], scalar1=ab1[:, 0:1],
                    scalar2=ab1[:, 1:2], op0=OP.mult, op1=OP.add)
                nc.vector.tensor_scalar_max(y1r[:], y1r[:], 0.0)
                y2 = yp.tile([128, 4096], F16, tag="y2")
                for c in range(16):
                    bp_, fo = _pk(c)
                    pc = cp.tile([C2, 512], F32, tag="pc2")
                    nc.tensor.matmul(
                        out=pc[:], lhsT=w2[bp_:bp_ + 64, :],
                        rhs=y1r[bp_:bp_ + 64, fo:fo + 512],
                        start=True, stop=True)
                    slot = sm2[:, t * 16 + c: t * 16 + c + 1]
                    if c < 11:
                        nc.scalar.activation(
                            out=y2[bp_:bp_ + 64, fo:fo + 512], in_=pc[:],
                            func=AF.Identity, accum_out=slot)
                    else:
                        nc.vector.tensor_scalar(
                            out=y2[bp_:bp_ + 64, fo:fo + 512], in0=pc[:],
                            scalar1=1.0, scalar2=0.0,
                            op0=OP.mult, op1=OP.add, accum_out=slot)
                # sumsq pass; output recycles the y1 tile
                if USE_STT_SUMSQ:
                    nc.vector.scalar_tensor_tensor(
                        out=y1[:], in0=y2[:], scalar=1.0, in1=y2[:],
                        op0=OP.mult, op1=OP.mult,
                        accum_out=sq2[:, t:t + 1])
                else:
                    nc.scalar.activation(
                        out=y1[:], in_=y2[:], func=AF.Square,
                        accum_out=sq2[:, t:t + 1])
                nc.sync.dma_start(out=d["y2d"][t], in_=y2[:])

        _bn_allreduce(tc, 1, sm2, sq2, gb2, ab2, d["arin1"], d["arout1"],
                      dup=True, fold_sq=True, fold_sm=False)

        # ---------------- Phase 3: apply BN2+relu, conv3 ----------------
        with tc.tile_pool(name="p3y", bufs=2) as yp, \
             tc.tile_pool(name="p3y2", bufs=3) as y2p, \
             tc.tile_pool(name="p3psum", bufs=6, space="PSUM") as cp:
            for t in range(NT):
                y2 = y2p.tile([128, 4096], F16, tag="y2l")
                nc.sync.dma_start(out=y2[:], in_=d["y2d"][t])
                y2r = y2p.tile([128, 4096], F16, tag="y2r")
                nc.vector.tensor_scalar(
                    out=y2r[:], in0=y2[:], scalar1=ab2[:, 0:1],
                    scalar2=ab2[:, 1:2], op0=OP.mult, op1=OP.add)
                nc.vector.tensor_scalar_max(y2r[:], y2r[:], 0.0)
                y3 = yp.tile([C3, 8192], F16, tag="y3")
                for c in range(16):
                    bp_, fo = _pk(c)
                    pc = cp.tile([C3, 512], F32, tag="pc3")
                    nc.tensor.matmul(
                        out=pc[:], lhsT=w3[bp_:bp_ + 64, :],
                        rhs=y2r[bp_:bp_ + 64, fo:fo + 512],
                        start=True, stop=True)
                    slot = sm3[:, t * 16 + c: t * 16 + c + 1]
                    if c < 12:
                        nc.scalar.activation(
                            out=y3[:, c * 512:(c + 1) * 512], in_=pc[:],
                            func=AF.Identity, accum_out=slot)
                    else:
                        nc.vector.tensor_scalar(
                            out=y3[:, c * 512:(c + 1) * 512], in0=pc[:],
                            scalar1=1.0, scalar2=0.0,
                            op0=OP.mult, op1=OP.add, accum_out=slot)
                # sumsq halves; outputs recycle y2l / y2r
                if USE_STT_SUMSQ:
                    nc.vector.scalar_tensor_tensor(
                        out=y2[:], in0=y3[:, 0:4096], scalar=1.0,
                        in1=y3[:, 0:4096], op0=OP.mult, op1=OP.mult,
                        accum_out=sq3[:, 2 * t:2 * t + 1])
                    nc.vector.scalar_tensor_tensor(
                        out=y2r[:], in0=y3[:, 4096:8192], scalar=1.0,
                        in1=y3[:, 4096:8192], op0=OP.mult, op1=OP.mult,
                        accum_out=sq3[:, 2 * t + 1:2 * t + 2])
                else:
                    nc.scalar.activation(
                        out=y2[:], in_=y3[:, 0:4096], func=AF.Square,
                        accum_out=sq3[:, 2 * t:2 * t + 1])
                    nc.scalar.activation(
                        out=y2r[:], in_=y3[:, 4096:8192], func=AF.Square,
                        accum_out=sq3[:, 2 * t + 1:2 * t + 2])
                nc.sync.dma_start(out=d["y3d"][t], in_=y3[:])

        _bn_allreduce(tc, 2, sm3, sq3, gb3, ab3, d["arin2"], d["arout2"],
                      dup=False, fold_sq=False, fold_sm=False)

        # ------------- Phase 4: scores, softmax, fusion, output -------------
        with tc.tile_pool(name="p4y", bufs=2) as yp, \
             tc.tile_pool(name="p4yf", bufs=2) as yfp, \
             tc.tile_pool(name="p4work", bufs=2) as wp, \
             tc.tile_pool(name="p4par", bufs=2) as parp, \
             tc.tile_pool(name="p4tree", bufs=3) as trp, \
             tc.tile_pool(name="p4tp", bufs=2, space="PSUM") as tp4, \
             tc.tile_pool(name="p4tps", bufs=1, space="PSUM") as tps, \
             tc.tile_pool(name="p4psum", bufs=2, space="PSUM") as pp4, \
             tc.tile_pool(name="p4opsum", bufs=1, space="PSUM") as opp, \
             tc.tile_pool(name="p4out", bufs=1) as op_:
            outsb = op_.tile([4, QPC], F32)
            for t in range(NT):
                y3 = yp.tile([C3, 8192], F16, tag="y3l")
                nc.sync.dma_start(out=y3[:], in_=d["y3d"][t])
                # bn3 apply WITH relu folded in (relu commutes with the
                # channel-max since it is monotone)
                scA = wp.tile([8, 512], F32, tag="scA")
                scB = wp.tile([8, 512], F32, tag="scB")
                ydt = F16 if PAR_MODE in ("f16q", "mixed") else F32
                y3f = yfp.tile([C3, 8192], ydt, tag="y3f")
                nc.scalar.activation(
                    out=y3f[:], in_=y3[:], func=AF.Relu,
                    scale=ab3[:, 0:1], bias=ab3[:, 1:2])
                if HYBRID_CMAX and (t % 2 == 1):
                    # channel-max via PE transpose (idle Tensor engine) +
                    # DVE free-axis max-reduce straight from PSUM; takes the
                    # GpSimd partition-reduce off every other tile
                    sctT = wp.tile([128, 64], F32, tag="sctT")
                    for bk in range(16):
                        ptp = tp4.tile([128, 512], F16, tag="ptp")
                        for u in range(4):
                            j = bk * 4 + u
                            nc.tensor.transpose(
                                out=ptp[:, u * 128:(u + 1) * 128],
                                in_=y3f[:, j * 128:(j + 1) * 128],
                                identity=ident16[:])
                        nc.vector.tensor_reduce(
                            out=sctT[:, bk * 4:(bk + 1) * 4],
                            in_=ptp[:].rearrange("c (b p) -> c b p", b=4),
                            axis=mybir.AxisListType.X, op=OP.max)
                    # back to chunk-row layout: PE-transpose the small score
                    # tile, then two batched partition-collapse DMAs
                    pts = tps.tile([64, 128], F32, tag="pts")
                    nc.tensor.transpose(
                        out=pts[:], in_=sctT[:], identity=ident[:])
                    scs = wp.tile([64, 128], F32, tag="scs")
                    nc.scalar.activation(
                        out=scs[:], in_=pts[:], func=AF.Identity)
                    nc.sync.dma_start(
                        out=scA[:].rearrange("c (b p) -> c b p", b=4),
                        in_=scs[0:32, :])
                    nc.scalar.dma_start(
                        out=scB[:].rearrange("c (b p) -> c b p", b=4),
                        in_=scs[32:64, :])
                elif PAR_MODE == "mixed":
                    # channel-max split: gpsimd quarters 0-2, DVE f16
                    # max-tree (2x mode) for quarter 3
                    for q in range(3):
                        par = parp.tile([128, 2048], F32, tag="par")
                        nc.gpsimd.partition_all_reduce(
                            out_ap=par[:],
                            in_ap=y3f[:, q * 2048:(q + 1) * 2048],
                            channels=128, reduce_op=bass_isa.ReduceOp.max)
                        dst = scA if q < 2 else scB
                        eng = (nc.sync, nc.scalar, nc.sync)[q]
                        eng.dma_start(
                            out=dst[(q % 2) * 4:(q % 2) * 4 + 4, :],
                            in_=par[0:1, :].rearrange("c (g j) -> c g j", g=4))
                    # SB+SB tensor_tensor requires equal base partitions, so
                    # each tree level shifts the upper half down via DMA on
                    # the idle sync/scalar queues (NOT the busy Pool queue)
                    tmp = trp.tile([64, 2048], F16, tag="tmtree")
                    sh = trp.tile([64, 2048], F16, tag="shtree")
                    nc.sync.dma_start(
                        out=sh[0:64, :], in_=y3f[64:128, 6144:8192])
                    nc.vector.tensor_tensor(
                        out=tmp[:], in0=y3f[0:64, 6144:8192],
                        in1=sh[0:64, :], op=OP.max)
                    tm32 = trp.tile([1, 2048], F32, tag="tm32")
                    lv = 32
                    while lv >= 1:
                        eng = (nc.sync, nc.scalar)[lv % 2]
                        eng.dma_start(
                            out=sh[0:lv, :], in_=tmp[lv:2 * lv, :])
                        if lv == 1:
                            nc.vector.tensor_tensor(
                                out=tm32[:], in0=tmp[0:1, :],
                                in1=sh[0:1, :], op=OP.max)
                        else:
                            nc.vector.tensor_tensor(
                                out=tmp[0:lv, :], in0=tmp[0:lv, :],
                                in1=sh[0:lv, :], op=OP.max)
                        lv //= 2
                    nc.scalar.dma_start(
                        out=scB[4:8, :],
                        in_=tm32[:].rearrange("c (g j) -> c g j", g=4))
                else:
                    for q in range(4):
                        par = parp.tile([128, 2048], F32, tag="par")
                        nc.gpsimd.partition_all_reduce(
                            out_ap=par[:],
                            in_ap=y3f[:, q * 2048:(q + 1) * 2048],
                            channels=128, reduce_op=bass_isa.ReduceOp.max)
                        dst = scA if q < 2 else scB
                        eng = (nc.sync, nc.scalar)[q % 2]
                        eng.dma_start(
                            out=dst[(q % 2) * 4:(q % 2) * 4 + 4, :],
                            in_=par[0:1, :].rearrange("c (g j) -> c g j", g=4))
                # softmax over the 64 neighbors of each query. The max
                # subtraction is skipped: scores are relu'd >= 0 and bounded
                # (BN-normalized channel maxes, << 88), so exp cannot
                # overflow f32. Normalization is deferred to the tiny
                # [128, 16] fusion output (weights stay unnormalized here).
                exA = wp.tile([8, 512], F32, tag="exA")
                exB = wp.tile([8, 512], F32, tag="exB")
                for sct, ext in ((scA, exA), (scB, exB)):
                    nc.scalar.activation(out=ext[:], in_=sct[:], func=AF.Exp)
                esA = wp.tile([8, 16], F32, tag="esA")
                esB = wp.tile([8, 16], F32, tag="esB")
                for ext, est in ((exA, esA), (exB, esB)):
                    nc.vector.tensor_reduce(
                        out=est[:],
                        in_=ext[:].rearrange("c (s p) -> c p s", s=32),
                        axis=mybir.AxisListType.X, op=OP.add)
                nc.vector.tensor_tensor(
                    out=esA[:], in0=esA[:], in1=esB[:], op=OP.add)
                nc.vector.reciprocal(out=esA[:], in_=esA[:])
                # replicate 1/wsum onto band partitions via a selector matmul
                pe = pp4.tile([128, 16], F32, tag="pe")
                nc.tensor.matmul(out=pe[:], lhsT=selw[:], rhs=esA[:],
                                 start=True, stop=True)
                per = wp.tile([128, 16], F32, tag="per")
                nc.scalar.activation(out=per[:], in_=pe[:], func=AF.Identity)
                # fusion: replicate weight rows onto band partitions via a
                # selector matmul, multiply with raw coords, segment-reduce
                g1 = wp.tile([128, 512], F32, tag="g1l")
                g2 = wp.tile([128, 512], F32, tag="g2l")
                nc.sync.dma_start(out=g1[:], in_=d["g1d"][t])
                nc.sync.dma_start(out=g2[:], in_=d["g2d"][t])
                wr1 = wp.tile([128, 512], F32, tag="wr1")
                wr2 = wp.tile([128, 512], F32, tag="wr2")
                for ext, wr in ((exA, wr1), (exB, wr2)):
                    pw = pp4.tile([128, 512], F32, tag="pw")
                    nc.tensor.matmul(
                        out=pw[:], lhsT=selw[:],
                        rhs=ext[:], start=True, stop=True)
                    nc.scalar.activation(out=wr[:], in_=pw[:], func=AF.Identity)
                pr = wp.tile([128, 512], F32, tag="pr")
                nc.vector.tensor_tensor(out=pr[:], in0=g1[:], in1=wr1[:],
                                        op=OP.mult)
                nc.vector.tensor_tensor(out=wr2[:], in0=g2[:], in1=wr2[:],
                                        op=OP.mult)
                nc.vector.tensor_tensor(out=pr[:], in0=pr[:], in1=wr2[:],
                                        op=OP.add)
                fp = wp.tile([128, 16], F32, tag="fp")
                nc.vector.tensor_reduce(
                    out=fp[:], in_=pr[:].rearrange("c (s p) -> c p s", s=32),
                    axis=mybir.AxisListType.X, op=OP.add)
                nc.vector.tensor_tensor(out=fp[:], in0=fp[:], in1=per[:],
                                        op=OP.mult)
                # outsb[c, t*128 + g*16 + p] = fp[16g+c, p] via selector mms
                po = opp.tile([4, 128], F32, tag="po")
                for g in range(8):
                    nc.tensor.matmul(
                        out=po[:, g * 16:(g + 1) * 16],
                        lhsT=eout[:, g * 4:(g + 1) * 4],
                        rhs=fp[:], start=True, stop=True)
                nc.scalar.activation(
                    out=outsb[0:3, t * 128:(t + 1) * 128], in_=po[0:3, :],
                    func=AF.Identity)
            nc.sync.dma_start(out=d["out"][:], in_=outsb[0:3, :])


def _bn_allreduce(tc, li, sm, sq, gbe, ab, arin, arout, dup, fold_sq, fold_sm):
    """Reduce per-chunk/per-tile stat slots, AllReduce across 8 cores, compute
    per-channel scale a = g*rsqrt(var+eps) and bias b = be - a*mean.

    fold_*: the stat tile is [128, S] over PACKED partitions (64 even-chunk
    channels at 0..64, odd at 64..128) -> fold halves with a partition-shift
    DMA + add."""
    nc = tc.nc
    C = gbe.shape[0]
    with tc.tile_pool(name=f"bn{li}", bufs=1) as bp:
        st = bp.tile([C, 2], F32)

        def reduce_into(src, fold, col):
            r = bp.tile([128, 1], F32, tag=f"r{li}{col}")
            nc.vector.tensor_reduce(out=r[0:src.shape[0], :], in_=src[:],
                                    axis=mybir.AxisListType.X, op=OP.add)
            if fold:
                hi = bp.tile([64, 1], F32, tag=f"h{li}{col}")
                nc.sync.dma_start(out=hi[:], in_=r[64:128, :])
                nc.vector.tensor_tensor(out=st[:, col:col + 1], in0=r[0:64, :],
                                        in1=hi[:], op=OP.add)
            else:
                nc.vector.tensor_copy(out=st[:, col:col + 1], in_=r[0:C, :])

        reduce_into(sm, fold_sm, 0)
        reduce_into(sq, fold_sq, 1)
        nc.sync.dma_start(out=arin[:], in_=st[:])
        if getattr(nc, "_single_core_nocoll", False):
            nc.sync.dma_start(out=arout[:], in_=arin[:])
        else:
            nc.gpsimd.collective_compute(
                "AllReduce", OP.add, replica_groups=[list(range(NCORES))],
                ins=[arin.opt()], outs=[arout.opt()])
        ar = bp.tile([C, 2], F32)
        nc.sync.dma_start(out=ar[:], in_=arout[:])
        mean = bp.tile([C, 1], F32)
        var = bp.tile([C, 1], F32)
        nc.vector.tensor_scalar_mul(mean[:], ar[:, 0:1], 1.0 / NTOT)
        nc.vector.tensor_scalar_mul(var[:], ar[:, 1:2], 1.0 / NTOT)
        m2 = bp.tile([C, 1], F32)
        nc.vector.tensor_tensor(out=m2[:], in0=mean[:], in1=mean[:], op=OP.mult)
        nc.vector.tensor_tensor(out=var[:], in0=var[:], in1=m2[:], op=OP.subtract)
        nc.vector.tensor_scalar_add(var[:], var[:], BN_EPS)
        nc.scalar.activation(out=var[:], in_=var[:], func=AF.Sqrt)
        nc.vector.reciprocal(out=var[:], in_=var[:])  # rsqrt(var+eps)
        nc.vector.tensor_tensor(out=ab[0:C, 0:1], in0=var[:], in1=gbe[:, 0:1],
                                op=OP.mult)            # a
        nc.vector.tensor_tensor(out=m2[:], in0=ab[0:C, 0:1], in1=mean[:],
                                op=OP.mult)
        nc.vector.tensor_tensor(out=ab[0:C, 1:2], in0=gbe[:, 1:2], in1=m2[:],
                                op=OP.subtract)        # b = be - a*mean
        if dup:
            nc.vector.tensor_copy(out=ab[C:2 * C, :], in_=ab[0:C, :])


_PROGRAM = None
LAST_RESULT = None


def _get_program():
    global _PROGRAM
    if _PROGRAM is None:
        _PROGRAM = _build_program()
    return _PROGRAM


def _prep_core_inputs(points1, points2, W1, W2, W3, gs, bes, b, h):
    p1 = points1[b]          # [3, N]
    p2 = points2[b]
    q = p1[:, h * QPC:(h + 1) * QPC]            # [3, QPC]
    qf = np.concatenate([2.0 * q, np.ones((1, QPC), np.float32)], axis=0)

    def cand_tab(p):
        sq = (p * p).sum(axis=0, keepdims=True)
        return np.concatenate([p, -sq], axis=0).astype(np.float32)  # [4, N]

    gtab = np.zeros((128, N), np.float32)
    gtab2 = np.zeros((128, N), np.float32)
    for g in range(8):
        gtab[16 * g + 0:16 * g + 3] = p1
        gtab2[16 * g + 0:16 * g + 3] = p2
    nqsqv = (-(q * q).sum(axis=0)).reshape(NT, 128).T.astype(np.float32)

    def dup128(w):      # [64, C] -> [128, C] duplicated
        return np.concatenate([w, w], axis=0).astype(np.float32)

    selw = np.zeros((8, 128), np.float32)
    for g in range(8):
        for c3 in range(3):
            selw[g, 16 * g + c3] = 1.0

    # termt[:, (t*8+g)*64 : +64] = (-W1[:, :3] @ q_block).T   [16, 64]
    termt = np.zeros((16, NT * 8 * C1), np.float32)
    w13 = W1[:, 0:3]                                  # [64, 3]
    for t in range(NT):
        for g in range(8):
            qblk = q[:, t * 128 + g * 16: t * 128 + (g + 1) * 16]  # [3, 16]
            termt[:, (t * 8 + g) * C1:(t * 8 + g + 1) * C1] = \
                -(w13 @ qblk).T
    sel16 = np.tile(np.eye(16, dtype=np.float32), 32)  # [16, 512]

    eoutv = np.zeros((128, 32), np.float32)
    for g in range(8):
        for c3 in range(3):
            eoutv[16 * g + c3, g * 4 + c3] = 1.0

    return {
        "selw": selw,
        "qf": qf.astype(np.float32),
        "t1": cand_tab(p1), "t2": cand_tab(p2), "gt": gtab, "gt2": gtab2,
        "nqsq": np.ascontiguousarray(nqsqv),
        "termt": termt, "sel16": sel16, "eout": eoutv,
        "w1t": np.ascontiguousarray(W1.T).astype(np.float32),
        "w2t": dup128(np.ascontiguousarray(W2.T)).astype(np.float16),
        "w3t": dup128(np.ascontiguousarray(W3.T)).astype(np.float16),
        "gb1": np.stack([gs[0], bes[0]], axis=1).astype(np.float32),
        "gb2": np.stack([gs[1], bes[1]], axis=1).astype(np.float32),
        "gb3": np.stack([gs[2], bes[2]], axis=1).astype(np.float32),
    }


def kernel(points1, points2, k, t, W1, b1, g1, be1, W2, b2, g2, be2,
           W3, b3, g3, be3):
    # b1/b2/b3 cancel inside train-mode BatchNorm; t is unused by the net.
    assert int(np.asarray(k)) == KNN
    points1 = np.asarray(points1, np.float32)
    points2 = np.asarray(points2, np.float32)
    gs = [np.asarray(g1, np.float32), np.asarray(g2, np.float32),
          np.asarray(g3, np.float32)]
    bes = [np.asarray(be1, np.float32), np.asarray(be2, np.float32),
           np.asarray(be3, np.float32)]
    Ws = [np.asarray(W1, np.float32), np.asarray(W2, np.float32),
          np.asarray(W3, np.float32)]

    in_maps = []
    for c in range(NCORES):
        b, h = divmod(c, 2)
        in_maps.append(_prep_core_inputs(points1, points2, *Ws, gs, bes, b, h))

    nc = _get_program()
    bkr = run_bass_kernel_spmd(nc, in_maps, list(range(NCORES)))
    global LAST_RESULT
    LAST_RESULT = bkr
    res = bkr.results

    out = np.zeros((B, 3, N), np.float32)
    for c in range(NCORES):
        b, h = divmod(c, 2)
        out[b, :, h * QPC:(h + 1) * QPC] = res[c]["out"]
    return out


# revision 57
# speedup vs baseline: 1.0188x; 1.0188x over previous
"""PointsFusion Trainium2 kernel (optimized, v2).

Pipeline per batch b (B=4, N=4096, k=32):
  knn1 = 32-NN of p1 in p1, knn2 = 32-NN of p1 in p2 (exact, via DVE 8-max rounds)
  gather neighbor coords, features (resi, dist) -> conv(4->64)->BN->relu
  -> conv(64->64)->BN->relu -> conv(64->128)->BN->relu -> channel-max scores
  -> softmax over 64 neighbors -> weighted sum of neighbor coords.

Sharding: 8 cores = (batch b, half h of the 4096 query points). BatchNorm uses
global batch stats -> 3 tiny AllReduces of per-channel sum/sumsq.

v2 changes vs v1 (3.03ms):
  - phase 1: each (tile, kn) gets its OWN msb distance buffer, distances for
    both knns emitted eagerly, and the two top-k chains of a tile are
    round-interleaved so the DVE never stalls on its own serial chain
  - activation spills y1/y2/y3 + conv2/conv3 weights in fp16 (halves HBM
    traffic; fp16 keeps 0.05% precision so top-k stays exact in f32)
  - BN stats: per-chunk sums ride the PSUM->SBUF copies via accum_out;
    sumsq via one GpSimd scalar_tensor_tensor pass per tile (GpSimd is idle
    in phases 2/3) -- frees ~11us/tile of Scalar time
  - phase 4: channel-max as 2 partition_all_reduce of [128, 4096] instead of
    4 of [128, 2048] (amortizes the ~5us GpSimd handshake)

Layouts (per 128-query tile):
  pixel space: 16 chunks of 512; chunk c = kn*8+g, pixel j = c*512 + s*16 + p
  (g = query group, p = query-in-group, s = neighbor slot, kn = which knn).
  64-channel activations are packed [128, 4096]: chunk c lives at partitions
  64*(c%2)..+64, free 512*(c//2)..+512 (keeps matmul rhs bases in {0, 64}).

Self-contained: hardcodes shapes; no sibling imports.
"""

import sys

import numpy as np

for _p in ("/opt/trn_rl_repo", "/opt/pypackages"):
    if _p not in sys.path:
        sys.path.append(_p)

import concourse.bass as bass  # noqa: E402  (imported for side effects/typing)
import concourse.mybir as mybir  # noqa: E402
import concourse.tile as tile  # noqa: E402
from concourse import bacc, bass_isa  # noqa: E402
from concourse.bass_utils import run_bass_kernel_spmd  # noqa: E402
from concourse.masks import make_identity  # noqa: E402

F32 = mybir.dt.float32
F32R = mybir.dt.float32r
F16 = mybir.dt.float16
U16 = mybir.dt.uint16
I16 = mybir.dt.int16
AF = mybir.ActivationFunctionType
OP = mybir.AluOpType

NCORES = 8
B = 4
N = 4096          # candidate points per batch
KNN = 32          # neighbors per knn
QPC = 2048        # query points per core
NT = 16           # query tiles of 128 per core
C1, C2, C3 = 64, 64, 128
NTOT = float(B * N * 2 * KNN)   # BN stat count (global)
BN_EPS = 1e-3
NEG = -1.0e30

# HW-bisect flags (CoreSim passes all combos; some features hang real HW).
# partition_all_reduce crashes the device for free sizes > 2048 (ucode
# buffer limit) -- only the 2048-wide quarter variants are safe.
USE_TTR_SUMSQ = False    # tensor_tensor_reduce sumsq: CRASHES HW, keep False
USE_STT_SUMSQ = True     # sumsq via vector scalar_tensor_tensor (ph 2/3)
# "mixed" (gpsimd quarters + DVE shift-DMA max-tree) is numerically correct
# in CoreSim but produces wrong results on real HW -- do not use.
PAR_MODE = "f16q"        # f32q | f16q | mixed (gpsimd 3 quarters + DVE tree)
HYBRID_CMAX = True       # odd tiles: PE-transpose + DVE reduce channel-max


def _pk(cc):
    """packed [128, 4096] slice coords for chunk cc."""
    return 64 * (cc % 2), 512 * (cc // 2)


def _build_program(single=False):
    nc = bacc.Bacc(
        "TRN2", target_bir_lowering=False, debug=False,
        num_devices=1 if single else NCORES,
    )
    nc._single_core_nocoll = single

    ap = {}
    def din(name, shape, dt=F32):
        ap[name] = nc.dram_tensor(name, shape, dt, kind="ExternalInput").ap()
    din("qf", [4, QPC])
    din("t1", [4, N])
    din("t2", [4, N])
    din("gt", [128, N])
    din("gt2", [128, N])
    din("nqsq", [128, NT])
    din("w1t", [4, C1], F32R)
    din("w2t", [128, C2], F16)    # duplicated at partition 64
    din("w3t", [128, C3], F16)    # duplicated at partition 64
    din("gb1", [C1, 2])
    din("gb2", [C2, 2])
    din("gb3", [C3, 2])
    din("selw", [8, 128])
    din("termt", [16, NT * 8 * C1], F32R)
    din("sel16", [16, 512], F32R)
    din("eout", [128, 32])

    ap["out"] = nc.dram_tensor("out", [3, QPC], F32, kind="ExternalOutput").ap()

    ap["y1d"] = nc.dram_tensor("y1d", [NT, 128, 4096], F16).ap()
    ap["y2d"] = nc.dram_tensor("y2d", [NT, 128, 4096], F16).ap()
    ap["y3d"] = nc.dram_tensor("y3d", [NT, C3, 8192], F16).ap()
    ap["g1d"] = nc.dram_tensor("g1d", [NT, 128, 512], F32).ap()
    ap["g2d"] = nc.dram_tensor("g2d", [NT, 128, 512], F32).ap()
    for i, c in ((0, C1), (1, C2), (2, C3)):
        ap[f"arin{i}"] = nc.dram_tensor(f"arin{i}", [c * 2], F32).ap()
        ap[f"arout{i}"] = nc.dram_tensor(f"arout{i}", [c * 2], F32).ap()

    with tile.TileContext(nc) as tc:
        _kernel_body(tc, ap)
    nc.compile()
    return nc


def _kernel_body(tc, d):
    nc = tc.nc
    from contextlib import ExitStack

    ctx = ExitStack()
    with ctx:
        cpool = ctx.enter_context(tc.tile_pool(name="consts", bufs=1))
        w2 = cpool.tile([128, C2], F16)
        w3 = cpool.tile([128, C3], F16)
        gb1 = cpool.tile([C1, 2], F32)
        gb2 = cpool.tile([C2, 2], F32)
        gb3 = cpool.tile([C3, 2], F32)
        selw = cpool.tile([8, 128], F32)
        eout = cpool.tile([128, 32], F32)
        ident = cpool.tile([128, 128], F32)
        make_identity(nc, ident[:])
        ident16 = cpool.tile([128, 128], F16)
        nc.vector.tensor_copy(out=ident16[:], in_=ident[:])
        for nm, sb in [("w2t", w2), ("w3t", w3),
                       ("gb1", gb1), ("gb2", gb2), ("gb3", gb3),
                       ("selw", selw), ("eout", eout)]:
            nc.sync.dma_start(out=sb[:], in_=d[nm][:])

        spool = ctx.enter_context(tc.tile_pool(name="stats", bufs=1))
        sm1 = spool.tile([C1, NT * 16], F32)
        sq1 = spool.tile([128, NT], F32)
        sm2 = spool.tile([C2, NT * 16], F32)
        sq2 = spool.tile([128, NT], F32)
        sm3 = spool.tile([C3, NT * 16], F32)
        sq3 = spool.tile([C3, NT * 2], F32)
        ab1 = spool.tile([128, 2], F32)   # col0 = scale a, col1 = bias b (dup at 64)
        ab2 = spool.tile([128, 2], F32)
        ab3 = spool.tile([C3, 2], F32)

        # ---------------- Phase 1: knn + gather + feat + conv1 ----------------
        with tc.tile_pool(name="p1c", bufs=1) as p1c, \
             tc.tile_pool(name="p1m", bufs=2) as mpool, \
             tc.tile_pool(name="p1psum", bufs=3, space="PSUM") as pp, \
             tc.tile_pool(name="p1tp", bufs=1, space="PSUM") as tpp, \
             tc.tile_pool(name="p1cpsum", bufs=3, space="PSUM") as cp, \
             tc.tile_pool(name="p1feat", bufs=1) as fpool, \
             tc.tile_pool(name="p1work", bufs=3) as wp, \
             tc.tile_pool(name="p1tt", bufs=2) as ttp, \
             tc.tile_pool(name="p1y", bufs=2) as yp:
            # phase-1-only constants (pool closes after phase 1, freeing
            # SBUF for the later phases' double buffers)
            tt = p1c.tile([36, N], F32)     # t1 rows 0-3, t2 rows 32-35
            t1 = tt[0:4, :]
            t2 = tt[32:36, :]
            gt = p1c.tile([128, N], F32)
            gt2 = p1c.tile([128, N], F32)
            qfc = p1c.tile([36, QPC], F32)  # qf dup'd at rows 0-3 and 32-35
            nqsq = p1c.tile([128, NT], F32)
            w1 = p1c.tile([4, C1], F32R)
            sel16 = p1c.tile([16, 512], F32R)
            nc.sync.dma_start(out=tt[0:4, :], in_=d["t1"][:])
            nc.sync.dma_start(out=tt[32:36, :], in_=d["t2"][:])
            nc.sync.dma_start(out=qfc[0:4, :], in_=d["qf"][:])
            nc.sync.dma_start(out=qfc[32:36, :], in_=d["qf"][:])
            for nm, sb in [("gt", gt), ("gt2", gt2), ("nqsq", nqsq),
                           ("w1t", w1), ("sel16", sel16)]:
                nc.sync.dma_start(out=sb[:], in_=d[nm][:])
            msbs = {}

            def emit_dist(t, kn, msb):
                # distance matmuls + msb copies for (t, kn)
                tab = (t1, t2)[kn]
                qfk = qfc[32 * kn:32 * kn + 4, :]
                for ch in range(8):
                    pm = pp.tile([128, 512], F32, tag="pm")
                    nc.tensor.matmul(
                        out=pm[:],
                        lhsT=qfk[:, t * 128:(t + 1) * 128],
                        rhs=tab[:, ch * 512:(ch + 1) * 512],
                        start=True, stop=True,
                    )
                    nc.scalar.activation(
                        out=msb[:, ch * 512:(ch + 1) * 512], in_=pm[:],
                        func=AF.Identity, bias=nqsq[:, t:t + 1])

            def start_tile(t):
                for kn in (0, 1):
                    m = mpool.tile([128, N], F32, tag=f"msb{kn}")
                    msbs[(t, kn)] = m
                    emit_dist(t, kn, m)

            start_tile(0)
            for t in range(NT):
                # software pipeline: issue tile t+1's distance stages (both
                # knns) ahead of tile t's topk/conv1 chain
                if t + 1 < NT:
                    start_tile(t + 1)
                mA = msbs.pop((t, 0))
                mB = msbs.pop((t, 1))
                termt = ttp.tile([16, 8 * C1], F32R, tag="termt")
                nc.sync.dma_start(
                    out=termt[:],
                    in_=d["termt"][:, t * 8 * C1:(t + 1) * 8 * C1])
                vals = wp.tile([128, 64], F32, tag="vals")
                idxu = wp.tile([128, 64], U16, tag="idxu")
                idxi = wp.tile([128, 64], I16, tag="idxi")
                # two-level top-32 (exact except when one 128-candidate chunk
                # holds >8 of a query's true top-32: P ~ 3e-5 per query):
                #   L1: top-8 of each of 32 chunks of 128 -> 256 candidates
                #   L2: top-32 of the candidates via max8+match_replace rounds
                #   FIND: global indices via find_index8 on the full row
                # 32 chunks of 128: P(a query's true top-32 has >8 members in
                # one chunk) ~ 3e-5; 16 chunks of 256 pushes rel err over the
                # 2e-2 budget (measured 3.1e-2) -- keep 32.
                NCH = 32
                CW = N // NCH
                l1a = wp.tile([128, NCH * 8], F32, tag="l1v0")
                l1b = wp.tile([128, NCH * 8], F32, tag="l1v1")
                l1 = {0: l1a, 1: l1b}
                for c in range(NCH):
                    for kn, m in ((0, mA), (1, mB)):
                        nc.vector.max(
                            out=l1[kn][:, c * 8:(c + 1) * 8],
                            in_=m[:, c * CW:(c + 1) * CW])
                for r in range(4):
                    for kn in (0, 1):
                        v8 = vals[:, kn * 32 + r * 8: kn * 32 + r * 8 + 8]
                        nc.vector.max(out=v8, in_=l1[kn][:])
                    if r < 3:
                        for kn in (0, 1):
                            v8 = vals[:, kn * 32 + r * 8: kn * 32 + r * 8 + 8]
                            nc.vector.match_replace(
                                out=l1[kn][:], in_to_replace=v8,
                                in_values=l1[kn][:], imm_value=NEG)
                for r in range(4):
                    for kn, m in ((0, mA), (1, mB)):
                        v8 = vals[:, kn * 32 + r * 8: kn * 32 + r * 8 + 8]
                        i8 = idxu[:, kn * 32 + r * 8: kn * 32 + r * 8 + 8]
                        nc.vector.max_index(out=i8, in_max=v8, in_values=m[:])
                nc.vector.tensor_copy(out=idxi[:], in_=idxu[:])

                # gather neighbor coords; both tables carry xyz on band rows
                # 16g+{0..2} (gt = p1 for knn1, gt2 = p2 for knn2); spill raw
                # for the fusion phase
                g1 = wp.tile([128, 512], F32, tag="g1")
                g2 = wp.tile([128, 512], F32, tag="g2")
                nc.gpsimd.ap_gather(
                    out_ap=g1[:], in_ap=gt[:], idxs_ap=idxi[:, 0:32],
                    channels=128, num_elems=N, d=1, num_idxs=512)
                nc.gpsimd.ap_gather(
                    out_ap=g2[:], in_ap=gt2[:], idxs_ap=idxi[:, 32:64],
                    channels=128, num_elems=N, d=1, num_idxs=512)
                nc.sync.dma_start(out=d["g1d"][t], in_=g1[:])
                nc.sync.dma_start(out=d["g2d"][t], in_=g2[:])

                # conv1 rhs must start at partition 0: DMA bands into a flat
                # [4, 8192] tile (raw nn coords; the -q term is folded into
                # the conv1 matmul).  Band copies split across ACT / GpSimd
                # descriptor queues to keep them off the SP sequencer.
                feat = fpool.tile([4, 8192], F32R, tag="feat")
                for g in range(8):
                    nc.scalar.dma_start(
                        out=feat[0:3, g * 512:(g + 1) * 512],
                        in_=g1[16 * g: 16 * g + 3, :].bitcast(F32R))
                    nc.gpsimd.dma_start(
                        out=feat[0:3, (8 + g) * 512:(9 + g) * 512],
                        in_=g2[16 * g: 16 * g + 3, :].bitcast(F32R))

                # dist = sqrt(max(-val, 0)) into feat row 3
                d2 = wp.tile([128, 64], F32, tag="d2")
                nc.vector.tensor_scalar(
                    out=d2[:], in0=vals[:], scalar1=-1.0,
                    scalar2=0.0, op0=OP.mult, op1=OP.max)
                nc.scalar.activation(out=d2[:], in_=d2[:], func=AF.Sqrt)
                # shuffle dist to pixel layout: PE-transpose to [nbr, query],
                # then ONE batched DMA per knn half (dst iterates (s, g, p))
                dtp = tpp.tile([64, 128], F32, tag="dtp")
                nc.tensor.transpose(out=dtp[:], in_=d2[:], identity=ident[:])
                d2t = wp.tile([64, 128], F32, tag="d2t")
                nc.scalar.activation(out=d2t[:], in_=dtp[:], func=AF.Identity)
                for kn in (0, 1):
                    for g in range(8):
                        c = kn * 8 + g
                        eng = (nc.sync, nc.scalar, nc.gpsimd)[c % 3]
                        eng.dma_start(
                            out=feat[3:4, c * 512:(c + 1) * 512]
                                .rearrange("c (s p) -> c s p", s=32),
                            in_=d2t[kn * 32:(kn + 1) * 32,
                                    16 * g:16 * g + 16].bitcast(F32R))

                # conv1: 16 chunks -> y1 packed [128, 4096] fp16; second
                # matmul accumulates the host-precomputed -W1[:, :3] @ q term
                y1 = yp.tile([128, 4096], F16, tag="y1")
                for c in range(16):
                    g = c % 8
                    bp_, fo = _pk(c)
                    pc = cp.tile([C1, 512], F32, tag="pc1")
                    nc.tensor.matmul(
                        out=pc[:],
                        lhsT=w1[:],
                        rhs=feat[:, c * 512:(c + 1) * 512],
                        start=True, stop=False)
                    nc.tensor.matmul(
                        out=pc[:],
                        lhsT=termt[:, g * C1:(g + 1) * C1],
                        rhs=sel16[:],
                        start=False, stop=True)
                    nc.scalar.activation(
                        out=y1[bp_:bp_ + 64, fo:fo + 512], in_=pc[:],
                        func=AF.Identity,
                        accum_out=sm1[:, t * 16 + c: t * 16 + c + 1])
                # sumsq pass; output recycles the (dead) mA tile
                nc.scalar.activation(
                    out=mA[:].bitcast(F16)[:, 0:4096], in_=y1[:],
                    func=AF.Square, accum_out=sq1[:, t:t + 1])
                nc.sync.dma_start(out=d["y1d"][t], in_=y1[:])

        _bn_allreduce(tc, 0, sm1, sq1, gb1, ab1, d["arin0"], d["arout0"],
                      dup=True, fold_sq=True, fold_sm=False)

        # ---------------- Phase 2: apply BN1+relu, conv2 ----------------
        with tc.tile_pool(name="p2y", bufs=3) as yp, \
             tc.tile_pool(name="p2psum", bufs=6, space="PSUM") as cp:
            for t in range(NT):
                y1 = yp.tile([128, 4096], F16, tag="y1l")
                nc.sync.dma_start(out=y1[:], in_=d["y1d"][t])
                y1r = yp.tile([128, 4096], F16, tag="y1r")
                # bn1+relu on DVE (two f16 4x ops) -- keeps the ACT engine on
                # Identity only, avoiding per-tile function-table reloads
                nc.vector.tensor_scalar(
                    out=y1r[:], in0=y1[:], scalar1=ab1[:, 0:1],
                    scalar2=ab1[:, 1:2], op0=OP.mult, op1=OP.add)
                nc.vector.tensor_scalar_max(y1r[:], y1r[:], 0.0)
                y2 = yp.tile([128, 4096], F16, tag="y2")
                for c in range(16):
                    bp_, fo = _pk(c)
                    pc = cp.tile([C2, 512], F32, tag="pc2")
                    nc.tensor.matmul(
                        out=pc[:], lhsT=w2[bp_:bp_ + 64, :],
                        rhs=y1r[bp_:bp_ + 64, fo:fo + 512],
                        start=True, stop=True)
                    slot = sm2[:, t * 16 + c: t * 16 + c + 1]
                    if c < 11:
                        nc.scalar.activation(
                            out=y2[bp_:bp_ + 64, fo:fo + 512], in_=pc[:],
                            func=AF.Identity, accum_out=slot)
                    else:
                        nc.vector.tensor_scalar(
                            out=y2[bp_:bp_ + 64, fo:fo + 512], in0=pc[:],
                            scalar1=1.0, scalar2=0.0,
                            op0=OP.mult, op1=OP.add, accum_out=slot)
                # sumsq pass; output recycles the y1 tile
                if USE_STT_SUMSQ:
                    nc.vector.scalar_tensor_tensor(
                        out=y1[:], in0=y2[:], scalar=1.0, in1=y2[:],
                        op0=OP.mult, op1=OP.mult,
                        accum_out=sq2[:, t:t + 1])
                else:
                    nc.scalar.activation(
                        out=y1[:], in_=y2[:], func=AF.Square,
                        accum_out=sq2[:, t:t + 1])
                nc.sync.dma_start(out=d["y2d"][t], in_=y2[:])

        _bn_allreduce(tc, 1, sm2, sq2, gb2, ab2, d["arin1"], d["arout1"],
                      dup=True, fold_sq=True, fold_sm=False)

        # ---------------- Phase 3: apply BN2+relu, conv3 ----------------
        with tc.tile_pool(name="p3y", bufs=2) as yp, \
             tc.tile_pool(name="p3y2", bufs=3) as y2p, \
             tc.tile_pool(name="p3psum", bufs=6, space="PSUM") as cp:
            for t in range(NT):
                y2 = y2p.tile([128, 4096], F16, tag="y2l")
                nc.sync.dma_start(out=y2[:], in_=d["y2d"][t])
                y2r = y2p.tile([128, 4096], F16, tag="y2r")
                nc.vector.tensor_scalar(
                    out=y2r[:], in0=y2[:], scalar1=ab2[:, 0:1],
                    scalar2=ab2[:, 1:2], op0=OP.mult, op1=OP.add)
                nc.vector.tensor_scalar_max(y2r[:], y2r[:], 0.0)
                y3 = yp.tile([C3, 8192], F16, tag="y3")
                for c in range(16):
                    bp_, fo = _pk(c)
                    pc = cp.tile([C3, 512], F32, tag="pc3")
                    nc.tensor.matmul(
                        out=pc[:], lhsT=w3[bp_:bp_ + 64, :],
                        rhs=y2r[bp_:bp_ + 64, fo:fo + 512],
                        start=True, stop=True)
                    slot = sm3[:, t * 16 + c: t * 16 + c + 1]
                    if c < 12:
                        nc.scalar.activation(
                            out=y3[:, c * 512:(c + 1) * 512], in_=pc[:],
                            func=AF.Identity, accum_out=slot)
                    else:
                        nc.vector.tensor_scalar(
                            out=y3[:, c * 512:(c + 1) * 512], in0=pc[:],
                            scalar1=1.0, scalar2=0.0,
                            op0=OP.mult, op1=OP.add, accum_out=slot)
                # sumsq halves; outputs recycle y2l / y2r
                if USE_STT_SUMSQ:
                    nc.vector.scalar_tensor_tensor(
                        out=y2[:], in0=y3[:, 0:4096], scalar=1.0,
                        in1=y3[:, 0:4096], op0=OP.mult, op1=OP.mult,
                        accum_out=sq3[:, 2 * t:2 * t + 1])
                    nc.vector.scalar_tensor_tensor(
                        out=y2r[:], in0=y3[:, 4096:8192], scalar=1.0,
                        in1=y3[:, 4096:8192], op0=OP.mult, op1=OP.mult,
                        accum_out=sq3[:, 2 * t + 1:2 * t + 2])
                else:
                    nc.scalar.activation(
                        out=y2[:], in_=y3[:, 0:4096], func=AF.Square,
                        accum_out=sq3[:, 2 * t:2 * t + 1])
                    nc.scalar.activation(
                        out=y2r[:], in_=y3[:, 4096:8192], func=AF.Square,
                        accum_out=sq3[:, 2 * t + 1:2 * t + 2])
                nc.sync.dma_start(out=d["y3d"][t], in_=y3[:])

        _bn_allreduce(tc, 2, sm3, sq3, gb3, ab3, d["arin2"], d["arout2"],
                      dup=False, fold_sq=False, fold_sm=False)

        # ------------- Phase 4: scores, softmax, fusion, output -------------
        with tc.tile_pool(name="p4y", bufs=2) as yp, \
             tc.tile_pool(name="p4yf", bufs=2) as yfp, \
             tc.tile_pool(name="p4work", bufs=2) as wp, \
             tc.tile_pool(name="p4par", bufs=2) as parp, \
             tc.tile_pool(name="p4tree", bufs=3) as trp, \
             tc.tile_pool(name="p4tp", bufs=2, space="PSUM") as tp4, \
             tc.tile_pool(name="p4tps", bufs=1, space="PSUM") as tps, \
             tc.tile_pool(name="p4psum", bufs=2, space="PSUM") as pp4, \
             tc.tile_pool(name="p4opsum", bufs=1, space="PSUM") as opp, \
             tc.tile_pool(name="p4out", bufs=1) as op_:
            outsb = op_.tile([4, QPC], F32)
            for t in range(NT):
                y3 = yp.tile([C3, 8192], F16, tag="y3l")
                nc.sync.dma_start(out=y3[:], in_=d["y3d"][t])
                # bn3 apply WITH relu folded in (relu commutes with the
                # channel-max since it is monotone)
                scA = wp.tile([8, 512], F32, tag="scA")
                scB = wp.tile([8, 512], F32, tag="scB")
                ydt = F16 if PAR_MODE in ("f16q", "mixed") else F32
                y3f = yfp.tile([C3, 8192], ydt, tag="y3f")
                # split the apply across ACT and DVE halves to halve the
                # per-tile load->apply->reduce chain latency
                nc.scalar.activation(
                    out=y3f[:, 0:4096], in_=y3[:, 0:4096], func=AF.Relu,
                    scale=ab3[:, 0:1], bias=ab3[:, 1:2])
                nc.vector.tensor_scalar(
                    out=y3f[:, 4096:8192], in0=y3[:, 4096:8192],
                    scalar1=ab3[:, 0:1], scalar2=ab3[:, 1:2],
                    op0=OP.mult, op1=OP.add)
                nc.vector.tensor_scalar_max(
                    y3f[:, 4096:8192], y3f[:, 4096:8192], 0.0)
                if HYBRID_CMAX and (t % 2 == 1):
                    # channel-max via PE transpose (idle Tensor engine) +
                    # DVE free-axis max-reduce straight from PSUM; takes the
                    # GpSimd partition-reduce off every other tile
                    sctT = wp.tile([128, 64], F32, tag="sctT")
                    for bk in range(16):
                        ptp = tp4.tile([128, 512], F16, tag="ptp")
                        for u in range(4):
                            j = bk * 4 + u
                            nc.tensor.transpose(
                                out=ptp[:, u * 128:(u + 1) * 128],
                                in_=y3f[:, j * 128:(j + 1) * 128],
                                identity=ident16[:])
                        nc.vector.tensor_reduce(
                            out=sctT[:, bk * 4:(bk + 1) * 4],
                            in_=ptp[:].rearrange("c (b p) -> c b p", b=4),
                            axis=mybir.AxisListType.X, op=OP.max)
                    # back to chunk-row layout: PE-transpose the small score
                    # tile, then two batched partition-collapse DMAs
                    pts = tps.tile([64, 128], F32, tag="pts")
                    nc.tensor.transpose(
                        out=pts[:], in_=sctT[:], identity=ident[:])
                    scs = wp.tile([64, 128], F32, tag="scs")
                    nc.scalar.activation(
                        out=scs[:], in_=pts[:], func=AF.Identity)
                    nc.sync.dma_start(
                        out=scA[:].rearrange("c (b p) -> c b p", b=4),
                        in_=scs[0:32, :])
                    nc.scalar.dma_start(
                        out=scB[:].rearrange("c (b p) -> c b p", b=4),
                        in_=scs[32:64, :])
                elif PAR_MODE == "mixed":
                    # channel-max split: gpsimd quarters 0-2, DVE f16
                    # max-tree (2x mode) for quarter 3
                    for q in range(3):
                        par = parp.tile([128, 2048], F32, tag="par")
                        nc.gpsimd.partition_all_reduce(
                            out_ap=par[:],
                            in_ap=y3f[:, q * 2048:(q + 1) * 2048],
                            channels=128, reduce_op=bass_isa.ReduceOp.max)
                        dst = scA if q < 2 else scB
                        eng = (nc.sync, nc.scalar, nc.sync)[q]
                        eng.dma_start(
                            out=dst[(q % 2) * 4:(q % 2) * 4 + 4, :],
                            in_=par[0:1, :].rearrange("c (g j) -> c g j", g=4))
                    # SB+SB tensor_tensor requires equal base partitions, so
                    # each tree level shifts the upper half down via DMA on
                    # the idle sync/scalar queues (NOT the busy Pool queue)
                    tmp = trp.tile([64, 2048], F16, tag="tmtree")
                    sh = trp.tile([64, 2048], F16, tag="shtree")
                    nc.sync.dma_start(
                        out=sh[0:64, :], in_=y3f[64:128, 6144:8192])
                    nc.vector.tensor_tensor(
                        out=tmp[:], in0=y3f[0:64, 6144:8192],
                        in1=sh[0:64, :], op=OP.max)
                    tm32 = trp.tile([1, 2048], F32, tag="tm32")
                    lv = 32
                    while lv >= 1:
                        eng = (nc.sync, nc.scalar)[lv % 2]
                        eng.dma_start(
                            out=sh[0:lv, :], in_=tmp[lv:2 * lv, :])
                        if lv == 1:
                            nc.vector.tensor_tensor(
                                out=tm32[:], in0=tmp[0:1, :],
                                in1=sh[0:1, :], op=OP.max)
                        else:
                            nc.vector.tensor_tensor(
                                out=tmp[0:lv, :], in0=tmp[0:lv, :],
                                in1=sh[0:lv, :], op=OP.max)
                        lv //= 2
                    nc.scalar.dma_start(
                        out=scB[4:8, :],
                        in_=tm32[:].rearrange("c (g j) -> c g j", g=4))
                else:
                    for q in range(4):
                        par = parp.tile([128, 2048], F32, tag="par")
                        nc.gpsimd.partition_all_reduce(
                            out_ap=par[:],
                            in_ap=y3f[:, q * 2048:(q + 1) * 2048],
                            channels=128, reduce_op=bass_isa.ReduceOp.max)
                        dst = scA if q < 2 else scB
                        eng = (nc.sync, nc.scalar)[q % 2]
                        eng.dma_start(
                            out=dst[(q % 2) * 4:(q % 2) * 4 + 4, :],
                            in_=par[0:1, :].rearrange("c (g j) -> c g j", g=4))
                # softmax over the 64 neighbors of each query. The max
                # subtraction is skipped: scores are relu'd >= 0 and bounded
                # (BN-normalized channel maxes, << 88), so exp cannot
                # overflow f32. Normalization is deferred to the tiny
                # [128, 16] fusion output (weights stay unnormalized here).
                exA = wp.tile([8, 512], F32, tag="exA")
                exB = wp.tile([8, 512], F32, tag="exB")
                for sct, ext in ((scA, exA), (scB, exB)):
                    nc.scalar.activation(out=ext[:], in_=sct[:], func=AF.Exp)
                esA = wp.tile([8, 16], F32, tag="esA")
                esB = wp.tile([8, 16], F32, tag="esB")
                for ext, est in ((exA, esA), (exB, esB)):
                    nc.vector.tensor_reduce(
                        out=est[:],
                        in_=ext[:].rearrange("c (s p) -> c p s", s=32),
                        axis=mybir.AxisListType.X, op=OP.add)
                nc.vector.tensor_tensor(
                    out=esA[:], in0=esA[:], in1=esB[:], op=OP.add)
                nc.vector.reciprocal(out=esA[:], in_=esA[:])
                # replicate 1/wsum onto band partitions via a selector matmul
                pe = pp4.tile([128, 16], F32, tag="pe")
                nc.tensor.matmul(out=pe[:], lhsT=selw[:], rhs=esA[:],
                                 start=True, stop=True)
                per = wp.tile([128, 16], F32, tag="per")
                nc.scalar.activation(out=per[:], in_=pe[:], func=AF.Identity)
                # fusion: replicate weight rows onto band partitions via a
                # selector matmul, multiply with raw coords, segment-reduce
                g1 = wp.tile([128, 512], F32, tag="g1l")
                g2 = wp.tile([128, 512], F32, tag="g2l")
                nc.sync.dma_start(out=g1[:], in_=d["g1d"][t])
                nc.sync.dma_start(out=g2[:], in_=d["g2d"][t])
                wr1 = wp.tile([128, 512], F32, tag="wr1")
                wr2 = wp.tile([128, 512], F32, tag="wr2")
                for ext, wr in ((exA, wr1), (exB, wr2)):
                    pw = pp4.tile([128, 512], F32, tag="pw")
                    nc.tensor.matmul(
                        out=pw[:], lhsT=selw[:],
                        rhs=ext[:], start=True, stop=True)
                    nc.scalar.activation(out=wr[:], in_=pw[:], func=AF.Identity)
                pr = wp.tile([128, 512], F32, tag="pr")
                nc.vector.tensor_tensor(out=pr[:], in0=g1[:], in1=wr1[:],
                                        op=OP.mult)
                nc.vector.tensor_tensor(out=wr2[:], in0=g2[:], in1=wr2[:],
                                        op=OP.mult)
                nc.vector.tensor_tensor(out=pr[:], in0=pr[:], in1=wr2[:],
                                        op=OP.add)
                fp = wp.tile([128, 16], F32, tag="fp")
                nc.vector.tensor_reduce(
                    out=fp[:], in_=pr[:].rearrange("c (s p) -> c p s", s=32),
                    axis=mybir.AxisListType.X, op=OP.add)
                nc.vector.tensor_tensor(out=fp[:], in0=fp[:], in1=per[:],
                                        op=OP.mult)
                # outsb[c, t*128 + g*16 + p] = fp[16g+c, p] via selector mms
                po = opp.tile([4, 128], F32, tag="po")
                for g in range(8):
                    nc.tensor.matmul(
                        out=po[:, g * 16:(g + 1) * 16],
                        lhsT=eout[:, g * 4:(g + 1) * 4],
                        rhs=fp[:], start=True, stop=True)
                nc.scalar.activation(
                    out=outsb[0:3, t * 128:(t + 1) * 128], in_=po[0:3, :],
                    func=AF.Identity)
            nc.sync.dma_start(out=d["out"][:], in_=outsb[0:3, :])


def _bn_allreduce(tc, li, sm, sq, gbe, ab, arin, arout, dup, fold_sq, fold_sm):
    """Reduce per-chunk/per-tile stat slots, AllReduce across 8 cores, compute
    per-channel scale a = g*rsqrt(var+eps) and bias b = be - a*mean.

    fold_*: the stat tile is [128, S] over PACKED partitions (64 even-chunk
    channels at 0..64, odd at 64..128) -> fold halves with a partition-shift
    DMA + add."""
    nc = tc.nc
    C = gbe.shape[0]
    with tc.tile_pool(name=f"bn{li}", bufs=1) as bp:
        st = bp.tile([C, 2], F32)

        def reduce_into(src, fold, col):
            r = bp.tile([128, 1], F32, tag=f"r{li}{col}")
            nc.vector.tensor_reduce(out=r[0:src.shape[0], :], in_=src[:],
                                    axis=mybir.AxisListType.X, op=OP.add)
            if fold:
                hi = bp.tile([64, 1], F32, tag=f"h{li}{col}")
                nc.sync.dma_start(out=hi[:], in_=r[64:128, :])
                nc.vector.tensor_tensor(out=st[:, col:col + 1], in0=r[0:64, :],
                                        in1=hi[:], op=OP.add)
            else:
                nc.vector.tensor_copy(out=st[:, col:col + 1], in_=r[0:C, :])

        reduce_into(sm, fold_sm, 0)
        reduce_into(sq, fold_sq, 1)
        nc.sync.dma_start(out=arin[:], in_=st[:])
        if getattr(nc, "_single_core_nocoll", False):
            nc.sync.dma_start(out=arout[:], in_=arin[:])
        else:
            nc.gpsimd.collective_compute(
                "AllReduce", OP.add, replica_groups=[list(range(NCORES))],
                ins=[arin.opt()], outs=[arout.opt()])
        ar = bp.tile([C, 2], F32)
        nc.sync.dma_start(out=ar[:], in_=arout[:])
        mean = bp.tile([C, 1], F32)
        var = bp.tile([C, 1], F32)
        nc.vector.tensor_scalar_mul(mean[:], ar[:, 0:1], 1.0 / NTOT)
        nc.vector.tensor_scalar_mul(var[:], ar[:, 1:2], 1.0 / NTOT)
        m2 = bp.tile([C, 1], F32)
        nc.vector.tensor_tensor(out=m2[:], in0=mean[:], in1=mean[:], op=OP.mult)
        nc.vector.tensor_tensor(out=var[:], in0=var[:], in1=m2[:], op=OP.subtract)
        nc.vector.tensor_scalar_add(var[:], var[:], BN_EPS)
        nc.scalar.activation(out=var[:], in_=var[:], func=AF.Sqrt)
        nc.vector.reciprocal(out=var[:], in_=var[:])  # rsqrt(var+eps)
        nc.vector.tensor_tensor(out=ab[0:C, 0:1], in0=var[:], in1=gbe[:, 0:1],
                                op=OP.mult)            # a
        nc.vector.tensor_tensor(out=m2[:], in0=ab[0:C, 0:1], in1=mean[:],
                                op=OP.mult)
        nc.vector.tensor_tensor(out=ab[0:C, 1:2], in0=gbe[:, 1:2], in1=m2[:],
                                op=OP.subtract)        # b = be - a*mean
        if dup:
            nc.vector.tensor_copy(out=ab[C:2 * C, :], in_=ab[0:C, :])


_PROGRAM = None
LAST_RESULT = None


def _get_program():
    global _PROGRAM
    if _PROGRAM is None:
        _PROGRAM = _build_program()
    return _PROGRAM


def _prep_core_inputs(points1, points2, W1, W2, W3, gs, bes, b, h):
    p1 = points1[b]          # [3, N]
    p2 = points2[b]
    q = p1[:, h * QPC:(h + 1) * QPC]            # [3, QPC]
    qf = np.concatenate([2.0 * q, np.ones((1, QPC), np.float32)], axis=0)

    def cand_tab(p):
        sq = (p * p).sum(axis=0, keepdims=True)
        return np.concatenate([p, -sq], axis=0).astype(np.float32)  # [4, N]

    gtab = np.zeros((128, N), np.float32)
    gtab2 = np.zeros((128, N), np.float32)
    for g in range(8):
        gtab[16 * g + 0:16 * g + 3] = p1
        gtab2[16 * g + 0:16 * g + 3] = p2
    nqsqv = (-(q * q).sum(axis=0)).reshape(NT, 128).T.astype(np.float32)

    def dup128(w):      # [64, C] -> [128, C] duplicated
        return np.concatenate([w, w], axis=0).astype(np.float32)

    selw = np.zeros((8, 128), np.float32)
    for g in range(8):
        for c3 in range(3):
            selw[g, 16 * g + c3] = 1.0

    # termt[:, (t*8+g)*64 : +64] = (-W1[:, :3] @ q_block).T   [16, 64]
    termt = np.zeros((16, NT * 8 * C1), np.float32)
    w13 = W1[:, 0:3]                                  # [64, 3]
    for t in range(NT):
        for g in range(8):
            qblk = q[:, t * 128 + g * 16: t * 128 + (g + 1) * 16]  # [3, 16]
            termt[:, (t * 8 + g) * C1:(t * 8 + g + 1) * C1] = \
                -(w13 @ qblk).T
    sel16 = np.tile(np.eye(16, dtype=np.float32), 32)  # [16, 512]

    eoutv = np.zeros((128, 32), np.float32)
    for g in range(8):
        for c3 in range(3):
            eoutv[16 * g + c3, g * 4 + c3] = 1.0

    return {
        "selw": selw,
        "qf": qf.astype(np.float32),
        "t1": cand_tab(p1), "t2": cand_tab(p2), "gt": gtab, "gt2": gtab2,
        "nqsq": np.ascontiguousarray(nqsqv),
        "termt": termt, "sel16": sel16, "eout": eoutv,
        "w1t": np.ascontiguousarray(W1.T).astype(np.float32),
        "w2t": dup128(np.ascontiguousarray(W2.T)).astype(np.float16),
        "w3t": dup128(np.ascontiguousarray(W3.T)).astype(np.float16),
        "gb1": np.stack([gs[0], bes[0]], axis=1).astype(np.float32),
        "gb2": np.stack([gs[1], bes[1]], axis=1).astype(np.float32),
        "gb3": np.stack([gs[2], bes[2]], axis=1).astype(np.float32),
    }


def kernel(points1, points2, k, t, W1, b1, g1, be1, W2, b2, g2, be2,
           W3, b3, g3, be3):
    # b1/b2/b3 cancel inside train-mode BatchNorm; t is unused by the net.
    assert int(np.asarray(k)) == KNN
    points1 = np.asarray(points1, np.float32)
    points2 = np.asarray(points2, np.float32)
    gs = [np.asarray(g1, np.float32), np.asarray(g2, np.float32),
          np.asarray(g3, np.float32)]
    bes = [np.asarray(be1, np.float32), np.asarray(be2, np.float32),
           np.asarray(be3, np.float32)]
    Ws = [np.asarray(W1, np.float32), np.asarray(W2, np.float32),
          np.asarray(W3, np.float32)]

    in_maps = []
    for c in range(NCORES):
        b, h = divmod(c, 2)
        in_maps.append(_prep_core_inputs(points1, points2, *Ws, gs, bes, b, h))

    nc = _get_program()
    bkr = run_bass_kernel_spmd(nc, in_maps, list(range(NCORES)))
    global LAST_RESULT
    LAST_RESULT = bkr
    res = bkr.results

    out = np.zeros((B, 3, N), np.float32)
    for c in range(NCORES):
        b, h = divmod(c, 2)
        out[b, :, h * QPC:(h + 1) * QPC] = res[c]["out"]
    return out


# revision 61
# speedup vs baseline: 1.0252x; 1.0063x over previous
"""PointsFusion Trainium2 kernel (optimized, v2).

Pipeline per batch b (B=4, N=4096, k=32):
  knn1 = 32-NN of p1 in p1, knn2 = 32-NN of p1 in p2 (exact, via DVE 8-max rounds)
  gather neighbor coords, features (resi, dist) -> conv(4->64)->BN->relu
  -> conv(64->64)->BN->relu -> conv(64->128)->BN->relu -> channel-max scores
  -> softmax over 64 neighbors -> weighted sum of neighbor coords.

Sharding: 8 cores = (batch b, half h of the 4096 query points). BatchNorm uses
global batch stats -> 3 tiny AllReduces of per-channel sum/sumsq.

v2 changes vs v1 (3.03ms):
  - phase 1: each (tile, kn) gets its OWN msb distance buffer, distances for
    both knns emitted eagerly, and the two top-k chains of a tile are
    round-interleaved so the DVE never stalls on its own serial chain
  - activation spills y1/y2/y3 + conv2/conv3 weights in fp16 (halves HBM
    traffic; fp16 keeps 0.05% precision so top-k stays exact in f32)
  - BN stats: per-chunk sums ride the PSUM->SBUF copies via accum_out;
    sumsq via one GpSimd scalar_tensor_tensor pass per tile (GpSimd is idle
    in phases 2/3) -- frees ~11us/tile of Scalar time
  - phase 4: channel-max as 2 partition_all_reduce of [128, 4096] instead of
    4 of [128, 2048] (amortizes the ~5us GpSimd handshake)

Layouts (per 128-query tile):
  pixel space: 16 chunks of 512; chunk c = kn*8+g, pixel j = c*512 + s*16 + p
  (g = query group, p = query-in-group, s = neighbor slot, kn = which knn).
  64-channel activations are packed [128, 4096]: chunk c lives at partitions
  64*(c%2)..+64, free 512*(c//2)..+512 (keeps matmul rhs bases in {0, 64}).

Self-contained: hardcodes shapes; no sibling imports.
"""

import sys

import numpy as np

for _p in ("/opt/trn_rl_repo", "/opt/pypackages"):
    if _p not in sys.path:
        sys.path.append(_p)

import concourse.bass as bass  # noqa: E402  (imported for side effects/typing)
import concourse.mybir as mybir  # noqa: E402
import concourse.tile as tile  # noqa: E402
from concourse import bacc, bass_isa  # noqa: E402
from concourse.bass_utils import run_bass_kernel_spmd  # noqa: E402
from concourse.masks import make_identity  # noqa: E402

F32 = mybir.dt.float32
F32R = mybir.dt.float32r
F16 = mybir.dt.float16
U16 = mybir.dt.uint16
I16 = mybir.dt.int16
AF = mybir.ActivationFunctionType
OP = mybir.AluOpType

NCORES = 8
B = 4
N = 4096          # candidate points per batch
KNN = 32          # neighbors per knn
QPC = 2048        # query points per core
NT = 16           # query tiles of 128 per core
C1, C2, C3 = 64, 64, 128
NTOT = float(B * N * 2 * KNN)   # BN stat count (global)
BN_EPS = 1e-3
NEG = -1.0e30

# HW-bisect flags (CoreSim passes all combos; some features hang real HW).
# partition_all_reduce crashes the device for free sizes > 2048 (ucode
# buffer limit) -- only the 2048-wide quarter variants are safe.
USE_TTR_SUMSQ = False    # tensor_tensor_reduce sumsq: CRASHES HW, keep False
USE_STT_SUMSQ = True     # sumsq via vector scalar_tensor_tensor (ph 2/3)
# "mixed" (gpsimd quarters + DVE shift-DMA max-tree) is numerically correct
# in CoreSim but produces wrong results on real HW -- do not use.
PAR_MODE = "f16q"        # f32q | f16q | mixed (gpsimd 3 quarters + DVE tree)
HYBRID_CMAX = True       # odd tiles: PE-transpose + DVE reduce channel-max


def _pk(cc):
    """packed [128, 4096] slice coords for chunk cc."""
    return 64 * (cc % 2), 512 * (cc // 2)


def _build_program(single=False):
    nc = bacc.Bacc(
        "TRN2", target_bir_lowering=False, debug=False,
        num_devices=1 if single else NCORES,
    )
    nc._single_core_nocoll = single

    ap = {}
    def din(name, shape, dt=F32):
        ap[name] = nc.dram_tensor(name, shape, dt, kind="ExternalInput").ap()
    din("qf", [4, QPC])
    din("t1", [4, N])
    din("t2", [4, N])
    din("gt", [128, N])
    din("gt2", [128, N])
    din("nqsq", [128, NT])
    din("w1t", [4, C1], F32R)
    din("w2t", [128, C2], F16)    # duplicated at partition 64
    din("w3t", [128, C3], F16)    # duplicated at partition 64
    din("gb1", [C1, 2])
    din("gb2", [C2, 2])
    din("gb3", [C3, 2])
    din("selw", [8, 128])
    din("termt", [16, NT * 8 * C1], F32R)
    din("sel16", [16, 512], F32R)
    din("eout", [128, 32])

    ap["out"] = nc.dram_tensor("out", [3, QPC], F32, kind="ExternalOutput").ap()

    ap["y1d"] = nc.dram_tensor("y1d", [NT, 128, 4096], F16).ap()
    ap["y2d"] = nc.dram_tensor("y2d", [NT, 128, 4096], F16).ap()
    ap["y3d"] = nc.dram_tensor("y3d", [NT, C3, 8192], F16).ap()
    ap["g1d"] = nc.dram_tensor("g1d", [NT, 128, 512], F32).ap()
    ap["g2d"] = nc.dram_tensor("g2d", [NT, 128, 512], F32).ap()
    for i, c in ((0, C1), (1, C2), (2, C3)):
        ap[f"arin{i}"] = nc.dram_tensor(f"arin{i}", [c * 2], F32).ap()
        ap[f"arout{i}"] = nc.dram_tensor(f"arout{i}", [c * 2], F32).ap()

    with tile.TileContext(nc) as tc:
        _kernel_body(tc, ap)
    nc.compile()
    return nc


def _kernel_body(tc, d):
    nc = tc.nc
    from contextlib import ExitStack

    ctx = ExitStack()
    with ctx:
        cpool = ctx.enter_context(tc.tile_pool(name="consts", bufs=1))
        w2 = cpool.tile([128, C2], F16)
        w3 = cpool.tile([128, C3], F16)
        gb1 = cpool.tile([C1, 2], F32)
        gb2 = cpool.tile([C2, 2], F32)
        gb3 = cpool.tile([C3, 2], F32)
        selw = cpool.tile([8, 128], F32)
        eout = cpool.tile([128, 32], F32)
        ident = cpool.tile([128, 128], F32)
        make_identity(nc, ident[:])
        ident16 = cpool.tile([128, 128], F16)
        nc.vector.tensor_copy(out=ident16[:], in_=ident[:])
        for nm, sb in [("w2t", w2), ("w3t", w3),
                       ("gb1", gb1), ("gb2", gb2), ("gb3", gb3),
                       ("selw", selw), ("eout", eout)]:
            nc.sync.dma_start(out=sb[:], in_=d[nm][:])

        spool = ctx.enter_context(tc.tile_pool(name="stats", bufs=1))
        sm1 = spool.tile([C1, NT * 16], F32)
        sq1 = spool.tile([128, NT], F32)
        sm2 = spool.tile([C2, NT * 16], F32)
        sq2 = spool.tile([128, NT], F32)
        sm3 = spool.tile([C3, NT * 16], F32)
        sq3 = spool.tile([C3, NT * 2], F32)
        ab1 = spool.tile([128, 2], F32)   # col0 = scale a, col1 = bias b (dup at 64)
        ab2 = spool.tile([128, 2], F32)
        ab3 = spool.tile([C3, 2], F32)

        # ---------------- Phase 1: knn + gather + feat + conv1 ----------------
        with tc.tile_pool(name="p1c", bufs=1) as p1c, \
             tc.tile_pool(name="p1m", bufs=2) as mpool, \
             tc.tile_pool(name="p1psum", bufs=3, space="PSUM") as pp, \
             tc.tile_pool(name="p1tp", bufs=1, space="PSUM") as tpp, \
             tc.tile_pool(name="p1cpsum", bufs=3, space="PSUM") as cp, \
             tc.tile_pool(name="p1feat", bufs=1) as fpool, \
             tc.tile_pool(name="p1work", bufs=3) as wp, \
             tc.tile_pool(name="p1tt", bufs=2) as ttp, \
             tc.tile_pool(name="p1y", bufs=2) as yp:
            # phase-1-only constants (pool closes after phase 1, freeing
            # SBUF for the later phases' double buffers)
            tt = p1c.tile([36, N], F32)     # t1 rows 0-3, t2 rows 32-35
            t1 = tt[0:4, :]
            t2 = tt[32:36, :]
            gt = p1c.tile([128, N], F32)
            gt2 = p1c.tile([128, N], F32)
            qfc = p1c.tile([36, QPC], F32)  # qf dup'd at rows 0-3 and 32-35
            nqsq = p1c.tile([128, NT], F32)
            w1 = p1c.tile([4, C1], F32R)
            sel16 = p1c.tile([16, 512], F32R)
            # small consts first: the distance matmuls need only
            # tt/qfc/nqsq -- don't queue them behind the 4MB gt/gt2 loads
            nc.sync.dma_start(out=tt[0:4, :], in_=d["t1"][:])
            nc.sync.dma_start(out=tt[32:36, :], in_=d["t2"][:])
            nc.sync.dma_start(out=qfc[0:4, :], in_=d["qf"][:])
            nc.sync.dma_start(out=qfc[32:36, :], in_=d["qf"][:])
            for nm, sb in [("nqsq", nqsq), ("w1t", w1), ("sel16", sel16),
                           ("gt", gt), ("gt2", gt2)]:
                nc.sync.dma_start(out=sb[:], in_=d[nm][:])
            msbs = {}

            def emit_dist(t, kn, msb):
                # distance matmuls + msb copies for (t, kn)
                tab = (t1, t2)[kn]
                qfk = qfc[32 * kn:32 * kn + 4, :]
                for ch in range(8):
                    pm = pp.tile([128, 512], F32, tag="pm")
                    nc.tensor.matmul(
                        out=pm[:],
                        lhsT=qfk[:, t * 128:(t + 1) * 128],
                        rhs=tab[:, ch * 512:(ch + 1) * 512],
                        start=True, stop=True,
                    )
                    nc.scalar.activation(
                        out=msb[:, ch * 512:(ch + 1) * 512], in_=pm[:],
                        func=AF.Identity, bias=nqsq[:, t:t + 1])

            def start_tile(t):
                for kn in (0, 1):
                    m = mpool.tile([128, N], F32, tag=f"msb{kn}")
                    msbs[(t, kn)] = m
                    emit_dist(t, kn, m)

            start_tile(0)
            for t in range(NT):
                # software pipeline: issue tile t+1's distance stages (both
                # knns) ahead of tile t's topk/conv1 chain
                if t + 1 < NT:
                    start_tile(t + 1)
                mA = msbs.pop((t, 0))
                mB = msbs.pop((t, 1))
                termt = ttp.tile([16, 8 * C1], F32R, tag="termt")
                nc.sync.dma_start(
                    out=termt[:],
                    in_=d["termt"][:, t * 8 * C1:(t + 1) * 8 * C1])
                vals = wp.tile([128, 64], F32, tag="vals")
                idxu = wp.tile([128, 64], U16, tag="idxu")
                idxi = wp.tile([128, 64], I16, tag="idxi")
                # two-level top-32 (exact except when one 128-candidate chunk
                # holds >8 of a query's true top-32: P ~ 3e-5 per query):
                #   L1: top-8 of each of 32 chunks of 128 -> 256 candidates
                #   L2: top-32 of the candidates via max8+match_replace rounds
                #   FIND: global indices via find_index8 on the full row
                # 32 chunks of 128: P(a query's true top-32 has >8 members in
                # one chunk) ~ 3e-5; 16 chunks of 256 pushes rel err over the
                # 2e-2 budget (measured 3.1e-2) -- keep 32.
                NCH = 32
                CW = N // NCH
                l1a = wp.tile([128, NCH * 8], F32, tag="l1v0")
                l1b = wp.tile([128, NCH * 8], F32, tag="l1v1")
                l1 = {0: l1a, 1: l1b}
                for c in range(NCH):
                    for kn, m in ((0, mA), (1, mB)):
                        nc.vector.max(
                            out=l1[kn][:, c * 8:(c + 1) * 8],
                            in_=m[:, c * CW:(c + 1) * CW])
                for r in range(4):
                    for kn in (0, 1):
                        v8 = vals[:, kn * 32 + r * 8: kn * 32 + r * 8 + 8]
                        nc.vector.max(out=v8, in_=l1[kn][:])
                    if r < 3:
                        for kn in (0, 1):
                            v8 = vals[:, kn * 32 + r * 8: kn * 32 + r * 8 + 8]
                            nc.vector.match_replace(
                                out=l1[kn][:], in_to_replace=v8,
                                in_values=l1[kn][:], imm_value=NEG)
                for r in range(4):
                    for kn, m in ((0, mA), (1, mB)):
                        v8 = vals[:, kn * 32 + r * 8: kn * 32 + r * 8 + 8]
                        i8 = idxu[:, kn * 32 + r * 8: kn * 32 + r * 8 + 8]
                        nc.vector.max_index(out=i8, in_max=v8, in_values=m[:])
                nc.vector.tensor_copy(out=idxi[:], in_=idxu[:])

                # gather neighbor coords; both tables carry xyz on band rows
                # 16g+{0..2} (gt = p1 for knn1, gt2 = p2 for knn2); spill raw
                # for the fusion phase
                g1 = wp.tile([128, 512], F32, tag="g1")
                g2 = wp.tile([128, 512], F32, tag="g2")
                nc.gpsimd.ap_gather(
                    out_ap=g1[:], in_ap=gt[:], idxs_ap=idxi[:, 0:32],
                    channels=128, num_elems=N, d=1, num_idxs=512)
                nc.gpsimd.ap_gather(
                    out_ap=g2[:], in_ap=gt2[:], idxs_ap=idxi[:, 32:64],
                    channels=128, num_elems=N, d=1, num_idxs=512)
                nc.sync.dma_start(out=d["g1d"][t], in_=g1[:])
                nc.sync.dma_start(out=d["g2d"][t], in_=g2[:])

                # conv1 rhs must start at partition 0: DMA bands into a flat
                # [4, 8192] tile (raw nn coords; the -q term is folded into
                # the conv1 matmul).  Band copies split across ACT / GpSimd
                # descriptor queues to keep them off the SP sequencer.
                feat = fpool.tile([4, 8192], F32R, tag="feat")
                for g in range(8):
                    nc.scalar.dma_start(
                        out=feat[0:3, g * 512:(g + 1) * 512],
                        in_=g1[16 * g: 16 * g + 3, :].bitcast(F32R))
                    nc.gpsimd.dma_start(
                        out=feat[0:3, (8 + g) * 512:(9 + g) * 512],
                        in_=g2[16 * g: 16 * g + 3, :].bitcast(F32R))

                # dist = sqrt(max(-val, 0)) into feat row 3
                d2 = wp.tile([128, 64], F32, tag="d2")
                nc.vector.tensor_scalar(
                    out=d2[:], in0=vals[:], scalar1=-1.0,
                    scalar2=0.0, op0=OP.mult, op1=OP.max)
                nc.scalar.activation(out=d2[:], in_=d2[:], func=AF.Sqrt)
                # shuffle dist to pixel layout: PE-transpose to [nbr, query],
                # then ONE batched DMA per knn half (dst iterates (s, g, p))
                dtp = tpp.tile([64, 128], F32, tag="dtp")
                nc.tensor.transpose(out=dtp[:], in_=d2[:], identity=ident[:])
                d2t = wp.tile([64, 128], F32, tag="d2t")
                nc.scalar.activation(out=d2t[:], in_=dtp[:], func=AF.Identity)
                for kn in (0, 1):
                    for g in range(8):
                        c = kn * 8 + g
                        eng = (nc.sync, nc.scalar, nc.gpsimd)[c % 3]
                        eng.dma_start(
                            out=feat[3:4, c * 512:(c + 1) * 512]
                                .rearrange("c (s p) -> c s p", s=32),
                            in_=d2t[kn * 32:(kn + 1) * 32,
                                    16 * g:16 * g + 16].bitcast(F32R))

                # conv1: 16 chunks -> y1 packed [128, 4096] fp16; second
                # matmul accumulates the host-precomputed -W1[:, :3] @ q term
                y1 = yp.tile([128, 4096], F16, tag="y1")
                for c in range(16):
                    g = c % 8
                    bp_, fo = _pk(c)
                    pc = cp.tile([C1, 512], F32, tag="pc1")
                    nc.tensor.matmul(
                        out=pc[:],
                        lhsT=w1[:],
                        rhs=feat[:, c * 512:(c + 1) * 512],
                        start=True, stop=False)
                    nc.tensor.matmul(
                        out=pc[:],
                        lhsT=termt[:, g * C1:(g + 1) * C1],
                        rhs=sel16[:],
                        start=False, stop=True)
                    nc.scalar.activation(
                        out=y1[bp_:bp_ + 64, fo:fo + 512], in_=pc[:],
                        func=AF.Identity,
                        accum_out=sm1[:, t * 16 + c: t * 16 + c + 1])
                # sumsq pass; output recycles the (dead) mA tile
                nc.scalar.activation(
                    out=mA[:].bitcast(F16)[:, 0:4096], in_=y1[:],
                    func=AF.Square, accum_out=sq1[:, t:t + 1])
                nc.sync.dma_start(out=d["y1d"][t], in_=y1[:])

        _bn_allreduce(tc, 0, sm1, sq1, gb1, ab1, d["arin0"], d["arout0"],
                      dup=True, fold_sq=True, fold_sm=False)

        # ---------------- Phase 2: apply BN1+relu, conv2 ----------------
        with tc.tile_pool(name="p2y", bufs=3) as yp, \
             tc.tile_pool(name="p2psum", bufs=6, space="PSUM") as cp:
            for t in range(NT):
                y1 = yp.tile([128, 4096], F16, tag="y1l")
                nc.sync.dma_start(out=y1[:], in_=d["y1d"][t])
                y1r = yp.tile([128, 4096], F16, tag="y1r")
                # bn1+relu on DVE (f16 4x ops), split into column halves so
                # the first 8 conv matmuls start after only half the apply
                for hb in range(2):
                    cols = slice(hb * 2048, (hb + 1) * 2048)
                    nc.vector.tensor_scalar(
                        out=y1r[:, cols], in0=y1[:, cols],
                        scalar1=ab1[:, 0:1], scalar2=ab1[:, 1:2],
                        op0=OP.mult, op1=OP.add)
                    nc.vector.tensor_scalar_max(
                        y1r[:, cols], y1r[:, cols], 0.0)
                y2 = yp.tile([128, 4096], F16, tag="y2")
                for c in range(16):
                    bp_, fo = _pk(c)
                    pc = cp.tile([C2, 512], F32, tag="pc2")
                    nc.tensor.matmul(
                        out=pc[:], lhsT=w2[bp_:bp_ + 64, :],
                        rhs=y1r[bp_:bp_ + 64, fo:fo + 512],
                        start=True, stop=True)
                    slot = sm2[:, t * 16 + c: t * 16 + c + 1]
                    if c < 11:
                        nc.scalar.activation(
                            out=y2[bp_:bp_ + 64, fo:fo + 512], in_=pc[:],
                            func=AF.Identity, accum_out=slot)
                    else:
                        nc.vector.tensor_scalar(
                            out=y2[bp_:bp_ + 64, fo:fo + 512], in0=pc[:],
                            scalar1=1.0, scalar2=0.0,
                            op0=OP.mult, op1=OP.add, accum_out=slot)
                # sumsq pass; output recycles the y1 tile
                if USE_STT_SUMSQ:
                    nc.vector.scalar_tensor_tensor(
                        out=y1[:], in0=y2[:], scalar=1.0, in1=y2[:],
                        op0=OP.mult, op1=OP.mult,
                        accum_out=sq2[:, t:t + 1])
                else:
                    nc.scalar.activation(
                        out=y1[:], in_=y2[:], func=AF.Square,
                        accum_out=sq2[:, t:t + 1])
                nc.sync.dma_start(out=d["y2d"][t], in_=y2[:])

        _bn_allreduce(tc, 1, sm2, sq2, gb2, ab2, d["arin1"], d["arout1"],
                      dup=True, fold_sq=True, fold_sm=False)

        # ---------------- Phase 3: apply BN2+relu, conv3 ----------------
        with tc.tile_pool(name="p3y", bufs=3) as yp, \
             tc.tile_pool(name="p3y2", bufs=3) as y2p, \
             tc.tile_pool(name="p3psum", bufs=6, space="PSUM") as cp:
            for t in range(NT):
                y2 = y2p.tile([128, 4096], F16, tag="y2l")
                nc.sync.dma_start(out=y2[:], in_=d["y2d"][t])
                y2r = y2p.tile([128, 4096], F16, tag="y2r")
                for hb in range(2):
                    cols = slice(hb * 2048, (hb + 1) * 2048)
                    nc.vector.tensor_scalar(
                        out=y2r[:, cols], in0=y2[:, cols],
                        scalar1=ab2[:, 0:1], scalar2=ab2[:, 1:2],
                        op0=OP.mult, op1=OP.add)
                    nc.vector.tensor_scalar_max(
                        y2r[:, cols], y2r[:, cols], 0.0)
                y3 = yp.tile([C3, 8192], F16, tag="y3")
                for c in range(16):
                    bp_, fo = _pk(c)
                    pc = cp.tile([C3, 512], F32, tag="pc3")
                    nc.tensor.matmul(
                        out=pc[:], lhsT=w3[bp_:bp_ + 64, :],
                        rhs=y2r[bp_:bp_ + 64, fo:fo + 512],
                        start=True, stop=True)
                    slot = sm3[:, t * 16 + c: t * 16 + c + 1]
                    if c < 12:
                        nc.scalar.activation(
                            out=y3[:, c * 512:(c + 1) * 512], in_=pc[:],
                            func=AF.Identity, accum_out=slot)
                    else:
                        nc.vector.tensor_scalar(
                            out=y3[:, c * 512:(c + 1) * 512], in0=pc[:],
                            scalar1=1.0, scalar2=0.0,
                            op0=OP.mult, op1=OP.add, accum_out=slot)
                # sumsq halves; outputs recycle y2l / y2r
                if USE_STT_SUMSQ:
                    nc.vector.scalar_tensor_tensor(
                        out=y2[:], in0=y3[:, 0:4096], scalar=1.0,
                        in1=y3[:, 0:4096], op0=OP.mult, op1=OP.mult,
                        accum_out=sq3[:, 2 * t:2 * t + 1])
                    nc.vector.scalar_tensor_tensor(
                        out=y2r[:], in0=y3[:, 4096:8192], scalar=1.0,
                        in1=y3[:, 4096:8192], op0=OP.mult, op1=OP.mult,
                        accum_out=sq3[:, 2 * t + 1:2 * t + 2])
                else:
                    nc.scalar.activation(
                        out=y2[:], in_=y3[:, 0:4096], func=AF.Square,
                        accum_out=sq3[:, 2 * t:2 * t + 1])
                    nc.scalar.activation(
                        out=y2r[:], in_=y3[:, 4096:8192], func=AF.Square,
                        accum_out=sq3[:, 2 * t + 1:2 * t + 2])
                nc.sync.dma_start(out=d["y3d"][t], in_=y3[:])

        _bn_allreduce(tc, 2, sm3, sq3, gb3, ab3, d["arin2"], d["arout2"],
                      dup=False, fold_sq=False, fold_sm=False)

        # ------------- Phase 4: scores, softmax, fusion, output -------------
        with tc.tile_pool(name="p4y", bufs=2) as yp, \
             tc.tile_pool(name="p4yf", bufs=2) as yfp, \
             tc.tile_pool(name="p4work", bufs=2) as wp, \
             tc.tile_pool(name="p4par", bufs=2) as parp, \
             tc.tile_pool(name="p4tree", bufs=3) as trp, \
             tc.tile_pool(name="p4tp", bufs=2, space="PSUM") as tp4, \
             tc.tile_pool(name="p4tps", bufs=1, space="PSUM") as tps, \
             tc.tile_pool(name="p4psum", bufs=2, space="PSUM") as pp4, \
             tc.tile_pool(name="p4opsum", bufs=1, space="PSUM") as opp, \
             tc.tile_pool(name="p4out", bufs=1) as op_:
            outsb = op_.tile([4, QPC], F32)
            for t in range(NT):
                y3 = yp.tile([C3, 8192], F16, tag="y3l")
                nc.sync.dma_start(out=y3[:], in_=d["y3d"][t])
                # bn3 apply WITH relu folded in (relu commutes with the
                # channel-max since it is monotone)
                scA = wp.tile([8, 512], F32, tag="scA")
                scB = wp.tile([8, 512], F32, tag="scB")
                ydt = F16 if PAR_MODE in ("f16q", "mixed") else F32
                y3f = yfp.tile([C3, 8192], ydt, tag="y3f")
                # split the apply across ACT and DVE halves to halve the
                # per-tile load->apply->reduce chain latency
                nc.scalar.activation(
                    out=y3f[:, 0:4096], in_=y3[:, 0:4096], func=AF.Relu,
                    scale=ab3[:, 0:1], bias=ab3[:, 1:2])
                nc.vector.tensor_scalar(
                    out=y3f[:, 4096:8192], in0=y3[:, 4096:8192],
                    scalar1=ab3[:, 0:1], scalar2=ab3[:, 1:2],
                    op0=OP.mult, op1=OP.add)
                nc.vector.tensor_scalar_max(
                    y3f[:, 4096:8192], y3f[:, 4096:8192], 0.0)
                if HYBRID_CMAX and (t % 2 == 1):
                    # channel-max via PE transpose (idle Tensor engine) +
                    # DVE free-axis max-reduce straight from PSUM; takes the
                    # GpSimd partition-reduce off every other tile
                    sctT = wp.tile([128, 64], F32, tag="sctT")
                    for bk in range(16):
                        ptp = tp4.tile([128, 512], F16, tag="ptp")
                        for u in range(4):
                            j = bk * 4 + u
                            nc.tensor.transpose(
                                out=ptp[:, u * 128:(u + 1) * 128],
                                in_=y3f[:, j * 128:(j + 1) * 128],
                                identity=ident16[:])
                        nc.vector.tensor_reduce(
                            out=sctT[:, bk * 4:(bk + 1) * 4],
                            in_=ptp[:].rearrange("c (b p) -> c b p", b=4),
                            axis=mybir.AxisListType.X, op=OP.max)
                    # back to chunk-row layout: PE-transpose the small score
                    # tile, then two batched partition-collapse DMAs
                    pts = tps.tile([64, 128], F32, tag="pts")
                    nc.tensor.transpose(
                        out=pts[:], in_=sctT[:], identity=ident[:])
                    scs = wp.tile([64, 128], F32, tag="scs")
                    nc.scalar.activation(
                        out=scs[:], in_=pts[:], func=AF.Identity)
                    nc.sync.dma_start(
                        out=scA[:].rearrange("c (b p) -> c b p", b=4),
                        in_=scs[0:32, :])
                    nc.scalar.dma_start(
                        out=scB[:].rearrange("c (b p) -> c b p", b=4),
                        in_=scs[32:64, :])
                elif PAR_MODE == "mixed":
                    # channel-max split: gpsimd quarters 0-2, DVE f16
                    # max-tree (2x mode) for quarter 3
                    for q in range(3):
                        par = parp.tile([128, 2048], F32, tag="par")
                        nc.gpsimd.partition_all_reduce(
                            out_ap=par[:],
                            in_ap=y3f[:, q * 2048:(q + 1) * 2048],
                            channels=128, reduce_op=bass_isa.ReduceOp.max)
                        dst = scA if q < 2 else scB
                        eng = (nc.sync, nc.scalar, nc.sync)[q]
                        eng.dma_start(
                            out=dst[(q % 2) * 4:(q % 2) * 4 + 4, :],
                            in_=par[0:1, :].rearrange("c (g j) -> c g j", g=4))
                    # SB+SB tensor_tensor requires equal base partitions, so
                    # each tree level shifts the upper half down via DMA on
                    # the idle sync/scalar queues (NOT the busy Pool queue)
                    tmp = trp.tile([64, 2048], F16, tag="tmtree")
                    sh = trp.tile([64, 2048], F16, tag="shtree")
                    nc.sync.dma_start(
                        out=sh[0:64, :], in_=y3f[64:128, 6144:8192])
                    nc.vector.tensor_tensor(
                        out=tmp[:], in0=y3f[0:64, 6144:8192],
                        in1=sh[0:64, :], op=OP.max)
                    tm32 = trp.tile([1, 2048], F32, tag="tm32")
                    lv = 32
                    while lv >= 1:
                        eng = (nc.sync, nc.scalar)[lv % 2]
                        eng.dma_start(
                            out=sh[0:lv, :], in_=tmp[lv:2 * lv, :])
                        if lv == 1:
                            nc.vector.tensor_tensor(
                                out=tm32[:], in0=tmp[0:1, :],
                                in1=sh[0:1, :], op=OP.max)
                        else:
                            nc.vector.tensor_tensor(
                                out=tmp[0:lv, :], in0=tmp[0:lv, :],
                                in1=sh[0:lv, :], op=OP.max)
                        lv //= 2
                    nc.scalar.dma_start(
                        out=scB[4:8, :],
                        in_=tm32[:].rearrange("c (g j) -> c g j", g=4))
                else:
                    for q in range(4):
                        par = parp.tile([128, 2048], F32, tag="par")
                        nc.gpsimd.partition_all_reduce(
                            out_ap=par[:],
                            in_ap=y3f[:, q * 2048:(q + 1) * 2048],
                            channels=128, reduce_op=bass_isa.ReduceOp.max)
                        dst = scA if q < 2 else scB
                        eng = (nc.sync, nc.scalar)[q % 2]
                        eng.dma_start(
                            out=dst[(q % 2) * 4:(q % 2) * 4 + 4, :],
                            in_=par[0:1, :].rearrange("c (g j) -> c g j", g=4))
                # softmax over the 64 neighbors of each query. The max
                # subtraction is skipped: scores are relu'd >= 0 and bounded
                # (BN-normalized channel maxes, << 88), so exp cannot
                # overflow f32. Normalization is deferred to the tiny
                # [128, 16] fusion output (weights stay unnormalized here).
                exA = wp.tile([8, 512], F32, tag="exA")
                exB = wp.tile([8, 512], F32, tag="exB")
                for sct, ext in ((scA, exA), (scB, exB)):
                    nc.scalar.activation(out=ext[:], in_=sct[:], func=AF.Exp)
                esA = wp.tile([8, 16], F32, tag="esA")
                esB = wp.tile([8, 16], F32, tag="esB")
                for ext, est in ((exA, esA), (exB, esB)):
                    nc.vector.tensor_reduce(
                        out=est[:],
                        in_=ext[:].rearrange("c (s p) -> c p s", s=32),
                        axis=mybir.AxisListType.X, op=OP.add)
                nc.vector.tensor_tensor(
                    out=esA[:], in0=esA[:], in1=esB[:], op=OP.add)
                nc.vector.reciprocal(out=esA[:], in_=esA[:])
                # replicate 1/wsum onto band partitions via a selector matmul
                pe = pp4.tile([128, 16], F32, tag="pe")
                nc.tensor.matmul(out=pe[:], lhsT=selw[:], rhs=esA[:],
                                 start=True, stop=True)
                per = wp.tile([128, 16], F32, tag="per")
                nc.scalar.activation(out=per[:], in_=pe[:], func=AF.Identity)
                # fusion: replicate weight rows onto band partitions via a
                # selector matmul, multiply with raw coords, segment-reduce
                g1 = wp.tile([128, 512], F32, tag="g1l")
                g2 = wp.tile([128, 512], F32, tag="g2l")
                nc.sync.dma_start(out=g1[:], in_=d["g1d"][t])
                nc.sync.dma_start(out=g2[:], in_=d["g2d"][t])
                wr1 = wp.tile([128, 512], F32, tag="wr1")
                wr2 = wp.tile([128, 512], F32, tag="wr2")
                for ext, wr in ((exA, wr1), (exB, wr2)):
                    pw = pp4.tile([128, 512], F32, tag="pw")
                    nc.tensor.matmul(
                        out=pw[:], lhsT=selw[:],
                        rhs=ext[:], start=True, stop=True)
                    nc.scalar.activation(out=wr[:], in_=pw[:], func=AF.Identity)
                pr = wp.tile([128, 512], F32, tag="pr")
                nc.vector.tensor_tensor(out=pr[:], in0=g1[:], in1=wr1[:],
                                        op=OP.mult)
                nc.vector.tensor_tensor(out=wr2[:], in0=g2[:], in1=wr2[:],
                                        op=OP.mult)
                nc.vector.tensor_tensor(out=pr[:], in0=pr[:], in1=wr2[:],
                                        op=OP.add)
                fp = wp.tile([128, 16], F32, tag="fp")
                nc.vector.tensor_reduce(
                    out=fp[:], in_=pr[:].rearrange("c (s p) -> c p s", s=32),
                    axis=mybir.AxisListType.X, op=OP.add)
                nc.vector.tensor_tensor(out=fp[:], in0=fp[:], in1=per[:],
                                        op=OP.mult)
                # outsb[c, t*128 + g*16 + p] = fp[16g+c, p] via selector mms
                po = opp.tile([4, 128], F32, tag="po")
                for g in range(8):
                    nc.tensor.matmul(
                        out=po[:, g * 16:(g + 1) * 16],
                        lhsT=eout[:, g * 4:(g + 1) * 4],
                        rhs=fp[:], start=True, stop=True)
                nc.scalar.activation(
                    out=outsb[0:3, t * 128:(t + 1) * 128], in_=po[0:3, :],
                    func=AF.Identity)
            nc.sync.dma_start(out=d["out"][:], in_=outsb[0:3, :])


def _bn_allreduce(tc, li, sm, sq, gbe, ab, arin, arout, dup, fold_sq, fold_sm):
    """Reduce per-chunk/per-tile stat slots, AllReduce across 8 cores, compute
    per-channel scale a = g*rsqrt(var+eps) and bias b = be - a*mean.

    fold_*: the stat tile is [128, S] over PACKED partitions (64 even-chunk
    channels at 0..64, odd at 64..128) -> fold halves with a partition-shift
    DMA + add."""
    nc = tc.nc
    C = gbe.shape[0]
    with tc.tile_pool(name=f"bn{li}", bufs=1) as bp:
        st = bp.tile([C, 2], F32)

        def reduce_into(src, fold, col):
            r = bp.tile([128, 1], F32, tag=f"r{li}{col}")
            nc.vector.tensor_reduce(out=r[0:src.shape[0], :], in_=src[:],
                                    axis=mybir.AxisListType.X, op=OP.add)
            if fold:
                hi = bp.tile([64, 1], F32, tag=f"h{li}{col}")
                nc.sync.dma_start(out=hi[:], in_=r[64:128, :])
                nc.vector.tensor_tensor(out=st[:, col:col + 1], in0=r[0:64, :],
                                        in1=hi[:], op=OP.add)
            else:
                nc.vector.tensor_copy(out=st[:, col:col + 1], in_=r[0:C, :])

        reduce_into(sm, fold_sm, 0)
        reduce_into(sq, fold_sq, 1)
        nc.sync.dma_start(out=arin[:], in_=st[:])
        if getattr(nc, "_single_core_nocoll", False):
            nc.sync.dma_start(out=arout[:], in_=arin[:])
        else:
            nc.gpsimd.collective_compute(
                "AllReduce", OP.add, replica_groups=[list(range(NCORES))],
                ins=[arin.opt()], outs=[arout.opt()])
        ar = bp.tile([C, 2], F32)
        nc.sync.dma_start(out=ar[:], in_=arout[:])
        mean = bp.tile([C, 1], F32)
        var = bp.tile([C, 1], F32)
        nc.vector.tensor_scalar_mul(mean[:], ar[:, 0:1], 1.0 / NTOT)
        nc.vector.tensor_scalar_mul(var[:], ar[:, 1:2], 1.0 / NTOT)
        m2 = bp.tile([C, 1], F32)
        nc.vector.tensor_tensor(out=m2[:], in0=mean[:], in1=mean[:], op=OP.mult)
        nc.vector.tensor_tensor(out=var[:], in0=var[:], in1=m2[:], op=OP.subtract)
        nc.vector.tensor_scalar_add(var[:], var[:], BN_EPS)
        nc.scalar.activation(out=var[:], in_=var[:], func=AF.Sqrt)
        nc.vector.reciprocal(out=var[:], in_=var[:])  # rsqrt(var+eps)
        nc.vector.tensor_tensor(out=ab[0:C, 0:1], in0=var[:], in1=gbe[:, 0:1],
                                op=OP.mult)            # a
        nc.vector.tensor_tensor(out=m2[:], in0=ab[0:C, 0:1], in1=mean[:],
                                op=OP.mult)
        nc.vector.tensor_tensor(out=ab[0:C, 1:2], in0=gbe[:, 1:2], in1=m2[:],
                                op=OP.subtract)        # b = be - a*mean
        if dup:
            nc.vector.tensor_copy(out=ab[C:2 * C, :], in_=ab[0:C, :])


_PROGRAM = None
LAST_RESULT = None


def _get_program():
    global _PROGRAM
    if _PROGRAM is None:
        _PROGRAM = _build_program()
    return _PROGRAM


def _prep_core_inputs(points1, points2, W1, W2, W3, gs, bes, b, h):
    p1 = points1[b]          # [3, N]
    p2 = points2[b]
    q = p1[:, h * QPC:(h + 1) * QPC]            # [3, QPC]
    qf = np.concatenate([2.0 * q, np.ones((1, QPC), np.float32)], axis=0)

    def cand_tab(p):
        sq = (p * p).sum(axis=0, keepdims=True)
        return np.concatenate([p, -sq], axis=0).astype(np.float32)  # [4, N]

    gtab = np.zeros((128, N), np.float32)
    gtab2 = np.zeros((128, N), np.float32)
    for g in range(8):
        gtab[16 * g + 0:16 * g + 3] = p1
        gtab2[16 * g + 0:16 * g + 3] = p2
    nqsqv = (-(q * q).sum(axis=0)).reshape(NT, 128).T.astype(np.float32)

    def dup128(w):      # [64, C] -> [128, C] duplicated
        return np.concatenate([w, w], axis=0).astype(np.float32)

    selw = np.zeros((8, 128), np.float32)
    for g in range(8):
        for c3 in range(3):
            selw[g, 16 * g + c3] = 1.0

    # termt[:, (t*8+g)*64 : +64] = (-W1[:, :3] @ q_block).T   [16, 64]
    termt = np.zeros((16, NT * 8 * C1), np.float32)
    w13 = W1[:, 0:3]                                  # [64, 3]
    for t in range(NT):
        for g in range(8):
            qblk = q[:, t * 128 + g * 16: t * 128 + (g + 1) * 16]  # [3, 16]
            termt[:, (t * 8 + g) * C1:(t * 8 + g + 1) * C1] = \
                -(w13 @ qblk).T
    sel16 = np.tile(np.eye(16, dtype=np.float32), 32)  # [16, 512]

    eoutv = np.zeros((128, 32), np.float32)
    for g in range(8):
        for c3 in range(3):
            eoutv[16 * g + c3, g * 4 + c3] = 1.0

    return {
        "selw": selw,
        "qf": qf.astype(np.float32),
        "t1": cand_tab(p1), "t2": cand_tab(p2), "gt": gtab, "gt2": gtab2,
        "nqsq": np.ascontiguousarray(nqsqv),
        "termt": termt, "sel16": sel16, "eout": eoutv,
        "w1t": np.ascontiguousarray(W1.T).astype(np.float32),
        "w2t": dup128(np.ascontiguousarray(W2.T)).astype(np.float16),
        "w3t": dup128(np.ascontiguousarray(W3.T)).astype(np.float16),
        "gb1": np.stack([gs[0], bes[0]], axis=1).astype(np.float32),
        "gb2": np.stack([gs[1], bes[1]], axis=1).astype(np.float32),
        "gb3": np.stack([gs[2], bes[2]], axis=1).astype(np.float32),
    }


def kernel(points1, points2, k, t, W1, b1, g1, be1, W2, b2, g2, be2,
           W3, b3, g3, be3):
    # b1/b2/b3 cancel inside train-mode BatchNorm; t is unused by the net.
    assert int(np.asarray(k)) == KNN
    points1 = np.asarray(points1, np.float32)
    points2 = np.asarray(points2, np.float32)
    gs = [np.asarray(g1, np.float32), np.asarray(g2, np.float32),
          np.asarray(g3, np.float32)]
    bes = [np.asarray(be1, np.float32), np.asarray(be2, np.float32),
           np.asarray(be3, np.float32)]
    Ws = [np.asarray(W1, np.float32), np.asarray(W2, np.float32),
          np.asarray(W3, np.float32)]

    in_maps = []
    for c in range(NCORES):
        b, h = divmod(c, 2)
        in_maps.append(_prep_core_inputs(points1, points2, *Ws, gs, bes, b, h))

    nc = _get_program()
    bkr = run_bass_kernel_spmd(nc, in_maps, list(range(NCORES)))
    global LAST_RESULT
    LAST_RESULT = bkr
    res = bkr.results

    out = np.zeros((B, 3, N), np.float32)
    for c in range(NCORES):
        b, h = divmod(c, 2)
        out[b, :, h * QPC:(h + 1) * QPC] = res[c]["out"]
    return out


# revision 62
# speedup vs baseline: 1.0495x; 1.0237x over previous
"""PointsFusion Trainium2 kernel (optimized, v2).

Pipeline per batch b (B=4, N=4096, k=32):
  knn1 = 32-NN of p1 in p1, knn2 = 32-NN of p1 in p2 (exact, via DVE 8-max rounds)
  gather neighbor coords, features (resi, dist) -> conv(4->64)->BN->relu
  -> conv(64->64)->BN->relu -> conv(64->128)->BN->relu -> channel-max scores
  -> softmax over 64 neighbors -> weighted sum of neighbor coords.

Sharding: 8 cores = (batch b, half h of the 4096 query points). BatchNorm uses
global batch stats -> 3 tiny AllReduces of per-channel sum/sumsq.

v2 changes vs v1 (3.03ms):
  - phase 1: each (tile, kn) gets its OWN msb distance buffer, distances for
    both knns emitted eagerly, and the two top-k chains of a tile are
    round-interleaved so the DVE never stalls on its own serial chain
  - activation spills y1/y2/y3 + conv2/conv3 weights in fp16 (halves HBM
    traffic; fp16 keeps 0.05% precision so top-k stays exact in f32)
  - BN stats: per-chunk sums ride the PSUM->SBUF copies via accum_out;
    sumsq via one GpSimd scalar_tensor_tensor pass per tile (GpSimd is idle
    in phases 2/3) -- frees ~11us/tile of Scalar time
  - phase 4: channel-max as 2 partition_all_reduce of [128, 4096] instead of
    4 of [128, 2048] (amortizes the ~5us GpSimd handshake)

Layouts (per 128-query tile):
  pixel space: 16 chunks of 512; chunk c = kn*8+g, pixel j = c*512 + s*16 + p
  (g = query group, p = query-in-group, s = neighbor slot, kn = which knn).
  64-channel activations are packed [128, 4096]: chunk c lives at partitions
  64*(c%2)..+64, free 512*(c//2)..+512 (keeps matmul rhs bases in {0, 64}).

Self-contained: hardcodes shapes; no sibling imports.
"""

import sys

import numpy as np

for _p in ("/opt/trn_rl_repo", "/opt/pypackages"):
    if _p not in sys.path:
        sys.path.append(_p)

import concourse.bass as bass  # noqa: E402  (imported for side effects/typing)
import concourse.mybir as mybir  # noqa: E402
import concourse.tile as tile  # noqa: E402
from concourse import bacc, bass_isa  # noqa: E402
from concourse.bass_utils import run_bass_kernel_spmd  # noqa: E402
from concourse.masks import make_identity  # noqa: E402

F32 = mybir.dt.float32
F32R = mybir.dt.float32r
F16 = mybir.dt.float16
U16 = mybir.dt.uint16
I16 = mybir.dt.int16
AF = mybir.ActivationFunctionType
OP = mybir.AluOpType

NCORES = 8
B = 4
N = 4096          # candidate points per batch
KNN = 32          # neighbors per knn
QPC = 2048        # query points per core
NT = 16           # query tiles of 128 per core
C1, C2, C3 = 64, 64, 128
NTOT = float(B * N * 2 * KNN)   # BN stat count (global)
BN_EPS = 1e-3
NEG = -1.0e30

# HW-bisect flags (CoreSim passes all combos; some features hang real HW).
# partition_all_reduce crashes the device for free sizes > 2048 (ucode
# buffer limit) -- only the 2048-wide quarter variants are safe.
USE_TTR_SUMSQ = False    # tensor_tensor_reduce sumsq: CRASHES HW, keep False
USE_STT_SUMSQ = True     # sumsq via vector scalar_tensor_tensor (ph 2/3)
# "mixed" (gpsimd quarters + DVE shift-DMA max-tree) is numerically correct
# in CoreSim but produces wrong results on real HW -- do not use.
PAR_MODE = "f16q"        # f32q | f16q | mixed (gpsimd 3 quarters + DVE tree)
HYBRID_CMAX = True       # odd tiles: PE-transpose + DVE reduce channel-max


def _pk(cc):
    """packed [128, 4096] slice coords for chunk cc."""
    return 64 * (cc % 2), 512 * (cc // 2)


def _build_program(single=False):
    nc = bacc.Bacc(
        "TRN2", target_bir_lowering=False, debug=False,
        num_devices=1 if single else NCORES,
    )
    nc._single_core_nocoll = single

    ap = {}
    def din(name, shape, dt=F32):
        ap[name] = nc.dram_tensor(name, shape, dt, kind="ExternalInput").ap()
    din("qf", [4, QPC])
    din("t1", [4, N])
    din("t2", [4, N])
    din("gt", [128, N])
    din("gt2", [128, N])
    din("nqsq", [128, NT])
    din("w1t", [4, C1], F32R)
    din("w2t", [128, C2], F16)    # duplicated at partition 64
    din("w3t", [128, C3], F16)    # duplicated at partition 64
    din("gb1", [C1, 2])
    din("gb2", [C2, 2])
    din("gb3", [C3, 2])
    din("selw", [8, 128])
    din("termt", [16, NT * 8 * C1], F32R)
    din("sel16", [16, 512], F32R)
    din("eout", [128, 32])

    ap["out"] = nc.dram_tensor("out", [3, QPC], F32, kind="ExternalOutput").ap()

    ap["y1d"] = nc.dram_tensor("y1d", [NT, 128, 4096], F16).ap()
    ap["y2d"] = nc.dram_tensor("y2d", [NT, 128, 4096], F16).ap()
    ap["y3d"] = nc.dram_tensor("y3d", [NT, C3, 8192], F16).ap()
    ap["g1d"] = nc.dram_tensor("g1d", [NT, 128, 512], F32).ap()
    ap["g2d"] = nc.dram_tensor("g2d", [NT, 128, 512], F32).ap()
    for i, c in ((0, C1), (1, C2), (2, C3)):
        ap[f"arin{i}"] = nc.dram_tensor(f"arin{i}", [c * 2], F32).ap()
        ap[f"arout{i}"] = nc.dram_tensor(f"arout{i}", [c * 2], F32).ap()

    with tile.TileContext(nc) as tc:
        _kernel_body(tc, ap)
    nc.compile()
    return nc


def _kernel_body(tc, d):
    nc = tc.nc
    from contextlib import ExitStack

    ctx = ExitStack()
    with ctx:
        cpool = ctx.enter_context(tc.tile_pool(name="consts", bufs=1))
        w2 = cpool.tile([128, C2], F16)
        w3 = cpool.tile([128, C3], F16)
        gb1 = cpool.tile([C1, 2], F32)
        gb2 = cpool.tile([C2, 2], F32)
        gb3 = cpool.tile([C3, 2], F32)
        selw = cpool.tile([8, 128], F32)
        eout = cpool.tile([128, 32], F32)
        ident = cpool.tile([128, 128], F32)
        make_identity(nc, ident[:])
        ident16 = cpool.tile([128, 128], F16)
        nc.vector.tensor_copy(out=ident16[:], in_=ident[:])
        for nm, sb in [("w2t", w2), ("w3t", w3),
                       ("gb1", gb1), ("gb2", gb2), ("gb3", gb3),
                       ("selw", selw), ("eout", eout)]:
            nc.sync.dma_start(out=sb[:], in_=d[nm][:])

        spool = ctx.enter_context(tc.tile_pool(name="stats", bufs=1))
        sm1 = spool.tile([C1, NT * 16], F32)
        sq1 = spool.tile([128, NT], F32)
        sm2 = spool.tile([C2, NT * 16], F32)
        sq2 = spool.tile([128, NT], F32)
        sm3 = spool.tile([C3, NT * 16], F32)
        sq3 = spool.tile([C3, NT * 2], F32)
        ab1 = spool.tile([128, 2], F32)   # col0 = scale a, col1 = bias b (dup at 64)
        ab2 = spool.tile([128, 2], F32)
        ab3 = spool.tile([C3, 2], F32)

        # ---------------- Phase 1: knn + gather + feat + conv1 ----------------
        with tc.tile_pool(name="p1c", bufs=1) as p1c, \
             tc.tile_pool(name="p1m", bufs=2) as mpool, \
             tc.tile_pool(name="p1psum", bufs=3, space="PSUM") as pp, \
             tc.tile_pool(name="p1tp", bufs=1, space="PSUM") as tpp, \
             tc.tile_pool(name="p1cpsum", bufs=3, space="PSUM") as cp, \
             tc.tile_pool(name="p1feat", bufs=1) as fpool, \
             tc.tile_pool(name="p1work", bufs=3) as wp, \
             tc.tile_pool(name="p1tt", bufs=2) as ttp, \
             tc.tile_pool(name="p1y", bufs=2) as yp:
            # phase-1-only constants (pool closes after phase 1, freeing
            # SBUF for the later phases' double buffers)
            tt = p1c.tile([36, N], F32)     # t1 rows 0-3, t2 rows 32-35
            t1 = tt[0:4, :]
            t2 = tt[32:36, :]
            gt = p1c.tile([128, N], F32)
            gt2 = p1c.tile([128, N], F32)
            qfc = p1c.tile([36, QPC], F32)  # qf dup'd at rows 0-3 and 32-35
            nqsq = p1c.tile([128, NT], F32)
            w1 = p1c.tile([4, C1], F32R)
            sel16 = p1c.tile([16, 512], F32R)
            # small consts first: the distance matmuls need only
            # tt/qfc/nqsq -- don't queue them behind the 4MB gt/gt2 loads
            nc.sync.dma_start(out=tt[0:4, :], in_=d["t1"][:])
            nc.sync.dma_start(out=tt[32:36, :], in_=d["t2"][:])
            nc.sync.dma_start(out=qfc[0:4, :], in_=d["qf"][:])
            nc.sync.dma_start(out=qfc[32:36, :], in_=d["qf"][:])
            for nm, sb in [("nqsq", nqsq), ("w1t", w1), ("sel16", sel16),
                           ("gt", gt), ("gt2", gt2)]:
                nc.sync.dma_start(out=sb[:], in_=d[nm][:])
            msbs = {}

            def emit_dist(t, kn, msb):
                # distance matmuls + msb copies for (t, kn)
                tab = (t1, t2)[kn]
                qfk = qfc[32 * kn:32 * kn + 4, :]
                for ch in range(8):
                    pm = pp.tile([128, 512], F32, tag="pm")
                    nc.tensor.matmul(
                        out=pm[:],
                        lhsT=qfk[:, t * 128:(t + 1) * 128],
                        rhs=tab[:, ch * 512:(ch + 1) * 512],
                        start=True, stop=True,
                    )
                    nc.scalar.activation(
                        out=msb[:, ch * 512:(ch + 1) * 512], in_=pm[:],
                        func=AF.Identity, bias=nqsq[:, t:t + 1])

            def start_tile(t):
                for kn in (0, 1):
                    m = mpool.tile([128, N], F32, tag=f"msb{kn}")
                    msbs[(t, kn)] = m
                    emit_dist(t, kn, m)

            start_tile(0)
            for t in range(NT):
                # software pipeline: issue tile t+1's distance stages (both
                # knns) ahead of tile t's topk/conv1 chain
                if t + 1 < NT:
                    start_tile(t + 1)
                mA = msbs.pop((t, 0))
                mB = msbs.pop((t, 1))
                termt = ttp.tile([16, 8 * C1], F32R, tag="termt")
                nc.sync.dma_start(
                    out=termt[:],
                    in_=d["termt"][:, t * 8 * C1:(t + 1) * 8 * C1])
                vals = wp.tile([128, 64], F32, tag="vals")
                idxu = wp.tile([128, 64], U16, tag="idxu")
                idxi = wp.tile([128, 64], I16, tag="idxi")
                # two-level top-32 (exact except when one 128-candidate chunk
                # holds >8 of a query's true top-32: P ~ 3e-5 per query):
                #   L1: top-8 of each of 32 chunks of 128 -> 256 candidates
                #   L2: top-32 of the candidates via max8+match_replace rounds
                #   FIND: global indices via find_index8 on the full row
                # 32 chunks of 128: P(a query's true top-32 has >8 members in
                # one chunk) ~ 3e-5; 16 chunks of 256 pushes rel err over the
                # 2e-2 budget (measured 3.1e-2) -- keep 32.
                NCH = 32
                CW = N // NCH
                l1a = wp.tile([128, NCH * 8], F32, tag="l1v0")
                l1b = wp.tile([128, NCH * 8], F32, tag="l1v1")
                l1 = {0: l1a, 1: l1b}
                for c in range(NCH):
                    for kn, m in ((0, mA), (1, mB)):
                        nc.vector.max(
                            out=l1[kn][:, c * 8:(c + 1) * 8],
                            in_=m[:, c * CW:(c + 1) * CW])
                for r in range(4):
                    for kn in (0, 1):
                        v8 = vals[:, kn * 32 + r * 8: kn * 32 + r * 8 + 8]
                        nc.vector.max(out=v8, in_=l1[kn][:])
                    if r < 3:
                        for kn in (0, 1):
                            v8 = vals[:, kn * 32 + r * 8: kn * 32 + r * 8 + 8]
                            nc.vector.match_replace(
                                out=l1[kn][:], in_to_replace=v8,
                                in_values=l1[kn][:], imm_value=NEG)
                for r in range(4):
                    for kn, m in ((0, mA), (1, mB)):
                        v8 = vals[:, kn * 32 + r * 8: kn * 32 + r * 8 + 8]
                        i8 = idxu[:, kn * 32 + r * 8: kn * 32 + r * 8 + 8]
                        nc.vector.max_index(out=i8, in_max=v8, in_values=m[:])
                nc.vector.tensor_copy(out=idxi[:], in_=idxu[:])

                # gather neighbor coords; both tables carry xyz on band rows
                # 16g+{0..2} (gt = p1 for knn1, gt2 = p2 for knn2); spill raw
                # for the fusion phase
                g1 = wp.tile([128, 512], F32, tag="g1")
                g2 = wp.tile([128, 512], F32, tag="g2")
                nc.gpsimd.ap_gather(
                    out_ap=g1[:], in_ap=gt[:], idxs_ap=idxi[:, 0:32],
                    channels=128, num_elems=N, d=1, num_idxs=512)
                nc.gpsimd.ap_gather(
                    out_ap=g2[:], in_ap=gt2[:], idxs_ap=idxi[:, 32:64],
                    channels=128, num_elems=N, d=1, num_idxs=512)
                nc.sync.dma_start(out=d["g1d"][t], in_=g1[:])
                nc.sync.dma_start(out=d["g2d"][t], in_=g2[:])

                # conv1 rhs must start at partition 0: DMA bands into a flat
                # [4, 8192] tile (raw nn coords; the -q term is folded into
                # the conv1 matmul).  Band copies split across ACT / GpSimd
                # descriptor queues to keep them off the SP sequencer.
                feat = fpool.tile([4, 8192], F32R, tag="feat")
                for g in range(8):
                    nc.scalar.dma_start(
                        out=feat[0:3, g * 512:(g + 1) * 512],
                        in_=g1[16 * g: 16 * g + 3, :].bitcast(F32R))
                    nc.gpsimd.dma_start(
                        out=feat[0:3, (8 + g) * 512:(9 + g) * 512],
                        in_=g2[16 * g: 16 * g + 3, :].bitcast(F32R))

                # dist = sqrt(max(-val, 0)) into feat row 3
                d2 = wp.tile([128, 64], F32, tag="d2")
                nc.vector.tensor_scalar(
                    out=d2[:], in0=vals[:], scalar1=-1.0,
                    scalar2=0.0, op0=OP.mult, op1=OP.max)
                nc.scalar.activation(out=d2[:], in_=d2[:], func=AF.Sqrt)
                # shuffle dist to pixel layout: PE-transpose to [nbr, query],
                # then ONE batched DMA per knn half (dst iterates (s, g, p))
                dtp = tpp.tile([64, 128], F32, tag="dtp")
                nc.tensor.transpose(out=dtp[:], in_=d2[:], identity=ident[:])
                d2t = wp.tile([64, 128], F32, tag="d2t")
                nc.scalar.activation(out=d2t[:], in_=dtp[:], func=AF.Identity)
                for kn in (0, 1):
                    for g in range(8):
                        c = kn * 8 + g
                        eng = (nc.sync, nc.scalar, nc.gpsimd)[c % 3]
                        eng.dma_start(
                            out=feat[3:4, c * 512:(c + 1) * 512]
                                .rearrange("c (s p) -> c s p", s=32),
                            in_=d2t[kn * 32:(kn + 1) * 32,
                                    16 * g:16 * g + 16].bitcast(F32R))

                # conv1: 16 chunks -> y1 packed [128, 4096] fp16; second
                # matmul accumulates the host-precomputed -W1[:, :3] @ q term
                y1 = yp.tile([128, 4096], F16, tag="y1")
                for c in range(16):
                    g = c % 8
                    bp_, fo = _pk(c)
                    pc = cp.tile([C1, 512], F32, tag="pc1")
                    nc.tensor.matmul(
                        out=pc[:],
                        lhsT=w1[:],
                        rhs=feat[:, c * 512:(c + 1) * 512],
                        start=True, stop=False)
                    nc.tensor.matmul(
                        out=pc[:],
                        lhsT=termt[:, g * C1:(g + 1) * C1],
                        rhs=sel16[:],
                        start=False, stop=True)
                    nc.scalar.activation(
                        out=y1[bp_:bp_ + 64, fo:fo + 512], in_=pc[:],
                        func=AF.Identity,
                        accum_out=sm1[:, t * 16 + c: t * 16 + c + 1])
                # sumsq pass; output recycles the (dead) mA tile
                nc.scalar.activation(
                    out=mA[:].bitcast(F16)[:, 0:4096], in_=y1[:],
                    func=AF.Square, accum_out=sq1[:, t:t + 1])
                nc.sync.dma_start(out=d["y1d"][t], in_=y1[:])

        _bn_allreduce(tc, 0, sm1, sq1, gb1, ab1, d["arin0"], d["arout0"],
                      dup=True, fold_sq=True, fold_sm=False)

        # ---------------- Phase 2: apply BN1+relu, conv2 ----------------
        with tc.tile_pool(name="p2y", bufs=3) as yp, \
             tc.tile_pool(name="p2psum", bufs=6, space="PSUM") as cp:
            for t in range(NT):
                y1 = yp.tile([128, 4096], F16, tag="y1l")
                nc.sync.dma_start(out=y1[:], in_=d["y1d"][t])
                y1r = yp.tile([128, 4096], F16, tag="y1r")
                # bn1+relu on DVE (f16 4x ops), split into column halves so
                # the first 8 conv matmuls start after only half the apply
                for hb in range(2):
                    cols = slice(hb * 2048, (hb + 1) * 2048)
                    nc.vector.tensor_scalar(
                        out=y1r[:, cols], in0=y1[:, cols],
                        scalar1=ab1[:, 0:1], scalar2=ab1[:, 1:2],
                        op0=OP.mult, op1=OP.add)
                    nc.vector.tensor_scalar_max(
                        y1r[:, cols], y1r[:, cols], 0.0)
                y2 = yp.tile([128, 4096], F16, tag="y2")
                for c in range(16):
                    bp_, fo = _pk(c)
                    pc = cp.tile([C2, 512], F32, tag="pc2")
                    nc.tensor.matmul(
                        out=pc[:], lhsT=w2[bp_:bp_ + 64, :],
                        rhs=y1r[bp_:bp_ + 64, fo:fo + 512],
                        start=True, stop=True)
                    slot = sm2[:, t * 16 + c: t * 16 + c + 1]
                    if c < 11:
                        nc.scalar.activation(
                            out=y2[bp_:bp_ + 64, fo:fo + 512], in_=pc[:],
                            func=AF.Identity, accum_out=slot)
                    else:
                        nc.vector.tensor_scalar(
                            out=y2[bp_:bp_ + 64, fo:fo + 512], in0=pc[:],
                            scalar1=1.0, scalar2=0.0,
                            op0=OP.mult, op1=OP.add, accum_out=slot)
                # sumsq pass; output recycles the y1 tile
                if USE_STT_SUMSQ:
                    nc.vector.scalar_tensor_tensor(
                        out=y1[:], in0=y2[:], scalar=1.0, in1=y2[:],
                        op0=OP.mult, op1=OP.mult,
                        accum_out=sq2[:, t:t + 1])
                else:
                    nc.scalar.activation(
                        out=y1[:], in_=y2[:], func=AF.Square,
                        accum_out=sq2[:, t:t + 1])
                nc.sync.dma_start(out=d["y2d"][t], in_=y2[:])

        _bn_allreduce(tc, 1, sm2, sq2, gb2, ab2, d["arin1"], d["arout1"],
                      dup=True, fold_sq=True, fold_sm=False)

        # ---------------- Phase 3: apply BN2+relu, conv3 ----------------
        with tc.tile_pool(name="p3y", bufs=3) as yp, \
             tc.tile_pool(name="p3y2", bufs=3) as y2p, \
             tc.tile_pool(name="p3psum", bufs=6, space="PSUM") as cp:
            for t in range(NT):
                y2 = y2p.tile([128, 4096], F16, tag="y2l")
                nc.sync.dma_start(out=y2[:], in_=d["y2d"][t])
                y2r = y2p.tile([128, 4096], F16, tag="y2r")
                for hb in range(2):
                    cols = slice(hb * 2048, (hb + 1) * 2048)
                    nc.vector.tensor_scalar(
                        out=y2r[:, cols], in0=y2[:, cols],
                        scalar1=ab2[:, 0:1], scalar2=ab2[:, 1:2],
                        op0=OP.mult, op1=OP.add)
                    nc.vector.tensor_scalar_max(
                        y2r[:, cols], y2r[:, cols], 0.0)
                y3 = yp.tile([C3, 8192], F16, tag="y3")
                for c in range(16):
                    bp_, fo = _pk(c)
                    pc = cp.tile([C3, 512], F32, tag="pc3")
                    nc.tensor.matmul(
                        out=pc[:], lhsT=w3[bp_:bp_ + 64, :],
                        rhs=y2r[bp_:bp_ + 64, fo:fo + 512],
                        start=True, stop=True)
                    slot = sm3[:, t * 16 + c: t * 16 + c + 1]
                    if c < 12:
                        nc.scalar.activation(
                            out=y3[:, c * 512:(c + 1) * 512], in_=pc[:],
                            func=AF.Identity, accum_out=slot)
                    else:
                        nc.vector.tensor_scalar(
                            out=y3[:, c * 512:(c + 1) * 512], in0=pc[:],
                            scalar1=1.0, scalar2=0.0,
                            op0=OP.mult, op1=OP.add, accum_out=slot)
                # sumsq halves; outputs recycle y2l / y2r
                if USE_STT_SUMSQ:
                    nc.vector.scalar_tensor_tensor(
                        out=y2[:], in0=y3[:, 0:4096], scalar=1.0,
                        in1=y3[:, 0:4096], op0=OP.mult, op1=OP.mult,
                        accum_out=sq3[:, 2 * t:2 * t + 1])
                    nc.vector.scalar_tensor_tensor(
                        out=y2r[:], in0=y3[:, 4096:8192], scalar=1.0,
                        in1=y3[:, 4096:8192], op0=OP.mult, op1=OP.mult,
                        accum_out=sq3[:, 2 * t + 1:2 * t + 2])
                else:
                    nc.scalar.activation(
                        out=y2[:], in_=y3[:, 0:4096], func=AF.Square,
                        accum_out=sq3[:, 2 * t:2 * t + 1])
                    nc.scalar.activation(
                        out=y2r[:], in_=y3[:, 4096:8192], func=AF.Square,
                        accum_out=sq3[:, 2 * t + 1:2 * t + 2])
                nc.sync.dma_start(out=d["y3d"][t], in_=y3[:])

        _bn_allreduce(tc, 2, sm3, sq3, gb3, ab3, d["arin2"], d["arout2"],
                      dup=False, fold_sq=False, fold_sm=False)

        # ------------- Phase 4: scores, softmax, fusion, output -------------
        with tc.tile_pool(name="p4y", bufs=2) as yp, \
             tc.tile_pool(name="p4yf", bufs=2) as yfp, \
             tc.tile_pool(name="p4work", bufs=2) as wp, \
             tc.tile_pool(name="p4par", bufs=2) as parp, \
             tc.tile_pool(name="p4tree", bufs=3) as trp, \
             tc.tile_pool(name="p4tp", bufs=2, space="PSUM") as tp4, \
             tc.tile_pool(name="p4tps", bufs=1, space="PSUM") as tps, \
             tc.tile_pool(name="p4psum", bufs=2, space="PSUM") as pp4, \
             tc.tile_pool(name="p4opsum", bufs=1, space="PSUM") as opp, \
             tc.tile_pool(name="p4out", bufs=1) as op_:
            outsb = op_.tile([4, QPC], F32)
            for t in range(NT):
                y3 = yp.tile([C3, 8192], F16, tag="y3l")
                nc.sync.dma_start(out=y3[:], in_=d["y3d"][t])
                # bn3 apply WITH relu folded in (relu commutes with the
                # channel-max since it is monotone)
                scA = wp.tile([8, 512], F32, tag="scA")
                scB = wp.tile([8, 512], F32, tag="scB")
                ydt = F16 if PAR_MODE in ("f16q", "mixed") else F32
                y3f = yfp.tile([C3, 8192], ydt, tag="y3f")
                # split the apply across ACT and DVE halves to halve the
                # per-tile load->apply->reduce chain latency
                nc.scalar.activation(
                    out=y3f[:, 0:4096], in_=y3[:, 0:4096], func=AF.Relu,
                    scale=ab3[:, 0:1], bias=ab3[:, 1:2])
                nc.vector.tensor_scalar(
                    out=y3f[:, 4096:8192], in0=y3[:, 4096:8192],
                    scalar1=ab3[:, 0:1], scalar2=ab3[:, 1:2],
                    op0=OP.mult, op1=OP.add)
                nc.vector.tensor_scalar_max(
                    y3f[:, 4096:8192], y3f[:, 4096:8192], 0.0)
                if HYBRID_CMAX:
                    # per-tile split channel-max: kn0 half via two gpsimd
                    # partition-reduces (serial chain halved vs four), kn1
                    # half via PE transposes (idle Tensor) + DVE max-reduces
                    # straight from PSUM
                    for q in range(2):
                        par = parp.tile([128, 2048], F32, tag="par")
                        nc.gpsimd.partition_all_reduce(
                            out_ap=par[:],
                            in_ap=y3f[:, q * 2048:(q + 1) * 2048],
                            channels=128, reduce_op=bass_isa.ReduceOp.max)
                        nc.sync.dma_start(
                            out=scA[(q % 2) * 4:(q % 2) * 4 + 4, :],
                            in_=par[0:1, :].rearrange("c (g j) -> c g j", g=4))
                    sctT = wp.tile([128, 32], F32, tag="sctT")
                    for bk in range(8):
                        ptp = tp4.tile([128, 512], F16, tag="ptp")
                        for u in range(4):
                            j = 32 + bk * 4 + u
                            nc.tensor.transpose(
                                out=ptp[:, u * 128:(u + 1) * 128],
                                in_=y3f[:, j * 128:(j + 1) * 128],
                                identity=ident16[:])
                        nc.vector.tensor_reduce(
                            out=sctT[:, bk * 4:(bk + 1) * 4],
                            in_=ptp[:].rearrange("c (b p) -> c b p", b=4),
                            axis=mybir.AxisListType.X, op=OP.max)
                    # back to chunk-row layout: PE-transpose the small score
                    # tile, then one batched partition-collapse DMA
                    pts = tps.tile([32, 128], F32, tag="pts")
                    nc.tensor.transpose(
                        out=pts[:], in_=sctT[:], identity=ident[:])
                    scs = wp.tile([32, 128], F32, tag="scs")
                    nc.scalar.activation(
                        out=scs[:], in_=pts[:], func=AF.Identity)
                    nc.scalar.dma_start(
                        out=scB[:].rearrange("c (b p) -> c b p", b=4),
                        in_=scs[0:32, :])
                elif PAR_MODE == "mixed":
                    # channel-max split: gpsimd quarters 0-2, DVE f16
                    # max-tree (2x mode) for quarter 3
                    for q in range(3):
                        par = parp.tile([128, 2048], F32, tag="par")
                        nc.gpsimd.partition_all_reduce(
                            out_ap=par[:],
                            in_ap=y3f[:, q * 2048:(q + 1) * 2048],
                            channels=128, reduce_op=bass_isa.ReduceOp.max)
                        dst = scA if q < 2 else scB
                        eng = (nc.sync, nc.scalar, nc.sync)[q]
                        eng.dma_start(
                            out=dst[(q % 2) * 4:(q % 2) * 4 + 4, :],
                            in_=par[0:1, :].rearrange("c (g j) -> c g j", g=4))
                    # SB+SB tensor_tensor requires equal base partitions, so
                    # each tree level shifts the upper half down via DMA on
                    # the idle sync/scalar queues (NOT the busy Pool queue)
                    tmp = trp.tile([64, 2048], F16, tag="tmtree")
                    sh = trp.tile([64, 2048], F16, tag="shtree")
                    nc.sync.dma_start(
                        out=sh[0:64, :], in_=y3f[64:128, 6144:8192])
                    nc.vector.tensor_tensor(
                        out=tmp[:], in0=y3f[0:64, 6144:8192],
                        in1=sh[0:64, :], op=OP.max)
                    tm32 = trp.tile([1, 2048], F32, tag="tm32")
                    lv = 32
                    while lv >= 1:
                        eng = (nc.sync, nc.scalar)[lv % 2]
                        eng.dma_start(
                            out=sh[0:lv, :], in_=tmp[lv:2 * lv, :])
                        if lv == 1:
                            nc.vector.tensor_tensor(
                                out=tm32[:], in0=tmp[0:1, :],
                                in1=sh[0:1, :], op=OP.max)
                        else:
                            nc.vector.tensor_tensor(
                                out=tmp[0:lv, :], in0=tmp[0:lv, :],
                                in1=sh[0:lv, :], op=OP.max)
                        lv //= 2
                    nc.scalar.dma_start(
                        out=scB[4:8, :],
                        in_=tm32[:].rearrange("c (g j) -> c g j", g=4))
                else:
                    for q in range(4):
                        par = parp.tile([128, 2048], F32, tag="par")
                        nc.gpsimd.partition_all_reduce(
                            out_ap=par[:],
                            in_ap=y3f[:, q * 2048:(q + 1) * 2048],
                            channels=128, reduce_op=bass_isa.ReduceOp.max)
                        dst = scA if q < 2 else scB
                        eng = (nc.sync, nc.scalar)[q % 2]
                        eng.dma_start(
                            out=dst[(q % 2) * 4:(q % 2) * 4 + 4, :],
                            in_=par[0:1, :].rearrange("c (g j) -> c g j", g=4))
                # softmax over the 64 neighbors of each query. The max
                # subtraction is skipped: scores are relu'd >= 0 and bounded
                # (BN-normalized channel maxes, << 88), so exp cannot
                # overflow f32. Normalization is deferred to the tiny
                # [128, 16] fusion output (weights stay unnormalized here).
                exA = wp.tile([8, 512], F32, tag="exA")
                exB = wp.tile([8, 512], F32, tag="exB")
                for sct, ext in ((scA, exA), (scB, exB)):
                    nc.scalar.activation(out=ext[:], in_=sct[:], func=AF.Exp)
                esA = wp.tile([8, 16], F32, tag="esA")
                esB = wp.tile([8, 16], F32, tag="esB")
                for ext, est in ((exA, esA), (exB, esB)):
                    nc.vector.tensor_reduce(
                        out=est[:],
                        in_=ext[:].rearrange("c (s p) -> c p s", s=32),
                        axis=mybir.AxisListType.X, op=OP.add)
                nc.vector.tensor_tensor(
                    out=esA[:], in0=esA[:], in1=esB[:], op=OP.add)
                nc.vector.reciprocal(out=esA[:], in_=esA[:])
                # replicate 1/wsum onto band partitions via a selector matmul
                pe = pp4.tile([128, 16], F32, tag="pe")
                nc.tensor.matmul(out=pe[:], lhsT=selw[:], rhs=esA[:],
                                 start=True, stop=True)
                per = wp.tile([128, 16], F32, tag="per")
                nc.scalar.activation(out=per[:], in_=pe[:], func=AF.Identity)
                # fusion: replicate weight rows onto band partitions via a
                # selector matmul, multiply with raw coords, segment-reduce
                g1 = wp.tile([128, 512], F32, tag="g1l")
                g2 = wp.tile([128, 512], F32, tag="g2l")
                nc.sync.dma_start(out=g1[:], in_=d["g1d"][t])
                nc.sync.dma_start(out=g2[:], in_=d["g2d"][t])
                wr1 = wp.tile([128, 512], F32, tag="wr1")
                wr2 = wp.tile([128, 512], F32, tag="wr2")
                for ext, wr in ((exA, wr1), (exB, wr2)):
                    pw = pp4.tile([128, 512], F32, tag="pw")
                    nc.tensor.matmul(
                        out=pw[:], lhsT=selw[:],
                        rhs=ext[:], start=True, stop=True)
                    nc.scalar.activation(out=wr[:], in_=pw[:], func=AF.Identity)
                pr = wp.tile([128, 512], F32, tag="pr")
                nc.vector.tensor_tensor(out=pr[:], in0=g1[:], in1=wr1[:],
                                        op=OP.mult)
                nc.vector.tensor_tensor(out=wr2[:], in0=g2[:], in1=wr2[:],
                                        op=OP.mult)
                nc.vector.tensor_tensor(out=pr[:], in0=pr[:], in1=wr2[:],
                                        op=OP.add)
                fp = wp.tile([128, 16], F32, tag="fp")
                nc.vector.tensor_reduce(
                    out=fp[:], in_=pr[:].rearrange("c (s p) -> c p s", s=32),
                    axis=mybir.AxisListType.X, op=OP.add)
                nc.vector.tensor_tensor(out=fp[:], in0=fp[:], in1=per[:],
                                        op=OP.mult)
                # outsb[c, t*128 + g*16 + p] = fp[16g+c, p] via selector mms
                po = opp.tile([4, 128], F32, tag="po")
                for g in range(8):
                    nc.tensor.matmul(
                        out=po[:, g * 16:(g + 1) * 16],
                        lhsT=eout[:, g * 4:(g + 1) * 4],
                        rhs=fp[:], start=True, stop=True)
                nc.scalar.activation(
                    out=outsb[0:3, t * 128:(t + 1) * 128], in_=po[0:3, :],
                    func=AF.Identity)
            nc.sync.dma_start(out=d["out"][:], in_=outsb[0:3, :])


def _bn_allreduce(tc, li, sm, sq, gbe, ab, arin, arout, dup, fold_sq, fold_sm):
    """Reduce per-chunk/per-tile stat slots, AllReduce across 8 cores, compute
    per-channel scale a = g*rsqrt(var+eps) and bias b = be - a*mean.

    fold_*: the stat tile is [128, S] over PACKED partitions (64 even-chunk
    channels at 0..64, odd at 64..128) -> fold halves with a partition-shift
    DMA + add."""
    nc = tc.nc
    C = gbe.shape[0]
    with tc.tile_pool(name=f"bn{li}", bufs=1) as bp:
        st = bp.tile([C, 2], F32)

        def reduce_into(src, fold, col):
            r = bp.tile([128, 1], F32, tag=f"r{li}{col}")
            nc.vector.tensor_reduce(out=r[0:src.shape[0], :], in_=src[:],
                                    axis=mybir.AxisListType.X, op=OP.add)
            if fold:
                hi = bp.tile([64, 1], F32, tag=f"h{li}{col}")
                nc.sync.dma_start(out=hi[:], in_=r[64:128, :])
                nc.vector.tensor_tensor(out=st[:, col:col + 1], in0=r[0:64, :],
                                        in1=hi[:], op=OP.add)
            else:
                nc.vector.tensor_copy(out=st[:, col:col + 1], in_=r[0:C, :])

        reduce_into(sm, fold_sm, 0)
        reduce_into(sq, fold_sq, 1)
        nc.sync.dma_start(out=arin[:], in_=st[:])
        if getattr(nc, "_single_core_nocoll", False):
            nc.sync.dma_start(out=arout[:], in_=arin[:])
        else:
            nc.gpsimd.collective_compute(
                "AllReduce", OP.add, replica_groups=[list(range(NCORES))],
                ins=[arin.opt()], outs=[arout.opt()])
        ar = bp.tile([C, 2], F32)
        nc.sync.dma_start(out=ar[:], in_=arout[:])
        mean = bp.tile([C, 1], F32)
        var = bp.tile([C, 1], F32)
        nc.vector.tensor_scalar_mul(mean[:], ar[:, 0:1], 1.0 / NTOT)
        nc.vector.tensor_scalar_mul(var[:], ar[:, 1:2], 1.0 / NTOT)
        m2 = bp.tile([C, 1], F32)
        nc.vector.tensor_tensor(out=m2[:], in0=mean[:], in1=mean[:], op=OP.mult)
        nc.vector.tensor_tensor(out=var[:], in0=var[:], in1=m2[:], op=OP.subtract)
        nc.vector.tensor_scalar_add(var[:], var[:], BN_EPS)
        nc.scalar.activation(out=var[:], in_=var[:], func=AF.Sqrt)
        nc.vector.reciprocal(out=var[:], in_=var[:])  # rsqrt(var+eps)
        nc.vector.tensor_tensor(out=ab[0:C, 0:1], in0=var[:], in1=gbe[:, 0:1],
                                op=OP.mult)            # a
        nc.vector.tensor_tensor(out=m2[:], in0=ab[0:C, 0:1], in1=mean[:],
                                op=OP.mult)
        nc.vector.tensor_tensor(out=ab[0:C, 1:2], in0=gbe[:, 1:2], in1=m2[:],
                                op=OP.subtract)        # b = be - a*mean
        if dup:
            nc.vector.tensor_copy(out=ab[C:2 * C, :], in_=ab[0:C, :])


_PROGRAM = None
LAST_RESULT = None


def _get_program():
    global _PROGRAM
    if _PROGRAM is None:
        _PROGRAM = _build_program()
    return _PROGRAM


def _prep_core_inputs(points1, points2, W1, W2, W3, gs, bes, b, h):
    p1 = points1[b]          # [3, N]
    p2 = points2[b]
    q = p1[:, h * QPC:(h + 1) * QPC]            # [3, QPC]
    qf = np.concatenate([2.0 * q, np.ones((1, QPC), np.float32)], axis=0)

    def cand_tab(p):
        sq = (p * p).sum(axis=0, keepdims=True)
        return np.concatenate([p, -sq], axis=0).astype(np.float32)  # [4, N]

    gtab = np.zeros((128, N), np.float32)
    gtab2 = np.zeros((128, N), np.float32)
    for g in range(8):
        gtab[16 * g + 0:16 * g + 3] = p1
        gtab2[16 * g + 0:16 * g + 3] = p2
    nqsqv = (-(q * q).sum(axis=0)).reshape(NT, 128).T.astype(np.float32)

    def dup128(w):      # [64, C] -> [128, C] duplicated
        return np.concatenate([w, w], axis=0).astype(np.float32)

    selw = np.zeros((8, 128), np.float32)
    for g in range(8):
        for c3 in range(3):
            selw[g, 16 * g + c3] = 1.0

    # termt[:, (t*8+g)*64 : +64] = (-W1[:, :3] @ q_block).T   [16, 64]
    termt = np.zeros((16, NT * 8 * C1), np.float32)
    w13 = W1[:, 0:3]                                  # [64, 3]
    for t in range(NT):
        for g in range(8):
            qblk = q[:, t * 128 + g * 16: t * 128 + (g + 1) * 16]  # [3, 16]
            termt[:, (t * 8 + g) * C1:(t * 8 + g + 1) * C1] = \
                -(w13 @ qblk).T
    sel16 = np.tile(np.eye(16, dtype=np.float32), 32)  # [16, 512]

    eoutv = np.zeros((128, 32), np.float32)
    for g in range(8):
        for c3 in range(3):
            eoutv[16 * g + c3, g * 4 + c3] = 1.0

    return {
        "selw": selw,
        "qf": qf.astype(np.float32),
        "t1": cand_tab(p1), "t2": cand_tab(p2), "gt": gtab, "gt2": gtab2,
        "nqsq": np.ascontiguousarray(nqsqv),
        "termt": termt, "sel16": sel16, "eout": eoutv,
        "w1t": np.ascontiguousarray(W1.T).astype(np.float32),
        "w2t": dup128(np.ascontiguousarray(W2.T)).astype(np.float16),
        "w3t": dup128(np.ascontiguousarray(W3.T)).astype(np.float16),
        "gb1": np.stack([gs[0], bes[0]], axis=1).astype(np.float32),
        "gb2": np.stack([gs[1], bes[1]], axis=1).astype(np.float32),
        "gb3": np.stack([gs[2], bes[2]], axis=1).astype(np.float32),
    }


def kernel(points1, points2, k, t, W1, b1, g1, be1, W2, b2, g2, be2,
           W3, b3, g3, be3):
    # b1/b2/b3 cancel inside train-mode BatchNorm; t is unused by the net.
    assert int(np.asarray(k)) == KNN
    points1 = np.asarray(points1, np.float32)
    points2 = np.asarray(points2, np.float32)
    gs = [np.asarray(g1, np.float32), np.asarray(g2, np.float32),
          np.asarray(g3, np.float32)]
    bes = [np.asarray(be1, np.float32), np.asarray(be2, np.float32),
           np.asarray(be3, np.float32)]
    Ws = [np.asarray(W1, np.float32), np.asarray(W2, np.float32),
          np.asarray(W3, np.float32)]

    in_maps = []
    for c in range(NCORES):
        b, h = divmod(c, 2)
        in_maps.append(_prep_core_inputs(points1, points2, *Ws, gs, bes, b, h))

    nc = _get_program()
    bkr = run_bass_kernel_spmd(nc, in_maps, list(range(NCORES)))
    global LAST_RESULT
    LAST_RESULT = bkr
    res = bkr.results

    out = np.zeros((B, 3, N), np.float32)
    for c in range(NCORES):
        b, h = divmod(c, 2)
        out[b, :, h * QPC:(h + 1) * QPC] = res[c]["out"]
    return out


# revision 66
# speedup vs baseline: 1.0500x; 1.0005x over previous
"""PointsFusion Trainium2 kernel (optimized, v2).

Pipeline per batch b (B=4, N=4096, k=32):
  knn1 = 32-NN of p1 in p1, knn2 = 32-NN of p1 in p2 (exact, via DVE 8-max rounds)
  gather neighbor coords, features (resi, dist) -> conv(4->64)->BN->relu
  -> conv(64->64)->BN->relu -> conv(64->128)->BN->relu -> channel-max scores
  -> softmax over 64 neighbors -> weighted sum of neighbor coords.

Sharding: 8 cores = (batch b, half h of the 4096 query points). BatchNorm uses
global batch stats -> 3 tiny AllReduces of per-channel sum/sumsq.

v2 changes vs v1 (3.03ms):
  - phase 1: each (tile, kn) gets its OWN msb distance buffer, distances for
    both knns emitted eagerly, and the two top-k chains of a tile are
    round-interleaved so the DVE never stalls on its own serial chain
  - activation spills y1/y2/y3 + conv2/conv3 weights in fp16 (halves HBM
    traffic; fp16 keeps 0.05% precision so top-k stays exact in f32)
  - BN stats: per-chunk sums ride the PSUM->SBUF copies via accum_out;
    sumsq via one GpSimd scalar_tensor_tensor pass per tile (GpSimd is idle
    in phases 2/3) -- frees ~11us/tile of Scalar time
  - phase 4: channel-max as 2 partition_all_reduce of [128, 4096] instead of
    4 of [128, 2048] (amortizes the ~5us GpSimd handshake)

Layouts (per 128-query tile):
  pixel space: 16 chunks of 512; chunk c = kn*8+g, pixel j = c*512 + s*16 + p
  (g = query group, p = query-in-group, s = neighbor slot, kn = which knn).
  64-channel activations are packed [128, 4096]: chunk c lives at partitions
  64*(c%2)..+64, free 512*(c//2)..+512 (keeps matmul rhs bases in {0, 64}).

Self-contained: hardcodes shapes; no sibling imports.
"""

import sys

import numpy as np

for _p in ("/opt/trn_rl_repo", "/opt/pypackages"):
    if _p not in sys.path:
        sys.path.append(_p)

import concourse.bass as bass  # noqa: E402  (imported for side effects/typing)
import concourse.mybir as mybir  # noqa: E402
import concourse.tile as tile  # noqa: E402
from concourse import bacc, bass_isa  # noqa: E402
from concourse.bass_utils import run_bass_kernel_spmd  # noqa: E402
from concourse.masks import make_identity  # noqa: E402

F32 = mybir.dt.float32
F32R = mybir.dt.float32r
F16 = mybir.dt.float16
U16 = mybir.dt.uint16
I16 = mybir.dt.int16
AF = mybir.ActivationFunctionType
OP = mybir.AluOpType

NCORES = 8
B = 4
N = 4096          # candidate points per batch
KNN = 32          # neighbors per knn
QPC = 2048        # query points per core
NT = 16           # query tiles of 128 per core
C1, C2, C3 = 64, 64, 128
NTOT = float(B * N * 2 * KNN)   # BN stat count (global)
BN_EPS = 1e-3
NEG = -1.0e30

# HW-bisect flags (CoreSim passes all combos; some features hang real HW).
# partition_all_reduce crashes the device for free sizes > 2048 (ucode
# buffer limit) -- only the 2048-wide quarter variants are safe.
USE_TTR_SUMSQ = False    # tensor_tensor_reduce sumsq: CRASHES HW, keep False
USE_STT_SUMSQ = True     # sumsq via vector scalar_tensor_tensor (ph 2/3)
# "mixed" (gpsimd quarters + DVE shift-DMA max-tree) is numerically correct
# in CoreSim but produces wrong results on real HW -- do not use.
PAR_MODE = "f16q"        # f32q | f16q | mixed (gpsimd 3 quarters + DVE tree)
HYBRID_CMAX = True       # odd tiles: PE-transpose + DVE reduce channel-max


def _pk(cc):
    """packed [128, 4096] slice coords for chunk cc."""
    return 64 * (cc % 2), 512 * (cc // 2)


def _build_program(single=False):
    nc = bacc.Bacc(
        "TRN2", target_bir_lowering=False, debug=False,
        num_devices=1 if single else NCORES,
    )
    nc._single_core_nocoll = single

    ap = {}
    def din(name, shape, dt=F32):
        ap[name] = nc.dram_tensor(name, shape, dt, kind="ExternalInput").ap()
    din("qf", [4, QPC])
    din("t1", [4, N])
    din("t2", [4, N])
    din("gt", [128, N])
    din("gt2", [128, N])
    din("nqsq", [128, NT])
    din("w1t", [4, C1], F32R)
    din("w2t", [128, C2], F16)    # duplicated at partition 64
    din("w3t", [128, C3], F16)    # duplicated at partition 64
    din("gb1", [C1, 2])
    din("gb2", [C2, 2])
    din("gb3", [C3, 2])
    din("selw", [8, 128])
    din("termt", [16, NT * 8 * C1], F32R)
    din("sel16", [16, 512], F32R)
    din("eout", [128, 32])

    ap["out"] = nc.dram_tensor("out", [3, QPC], F32, kind="ExternalOutput").ap()

    ap["y1d"] = nc.dram_tensor("y1d", [NT, 128, 4096], F16).ap()
    ap["y2d"] = nc.dram_tensor("y2d", [NT, 128, 4096], F16).ap()
    ap["y3d"] = nc.dram_tensor("y3d", [NT, C3, 8192], F16).ap()
    ap["g1d"] = nc.dram_tensor("g1d", [NT, 128, 512], F32).ap()
    ap["g2d"] = nc.dram_tensor("g2d", [NT, 128, 512], F32).ap()
    for i, c in ((0, C1), (1, C2), (2, C3)):
        ap[f"arin{i}"] = nc.dram_tensor(f"arin{i}", [c * 2], F32).ap()
        ap[f"arout{i}"] = nc.dram_tensor(f"arout{i}", [c * 2], F32).ap()

    with tile.TileContext(nc) as tc:
        _kernel_body(tc, ap)
    nc.compile()
    return nc


def _kernel_body(tc, d):
    nc = tc.nc
    from contextlib import ExitStack

    ctx = ExitStack()
    with ctx:
        cpool = ctx.enter_context(tc.tile_pool(name="consts", bufs=1))
        w2 = cpool.tile([128, C2], F16)
        w3 = cpool.tile([128, C3], F16)
        gb1 = cpool.tile([C1, 2], F32)
        gb2 = cpool.tile([C2, 2], F32)
        gb3 = cpool.tile([C3, 2], F32)
        selw = cpool.tile([8, 128], F32)
        eout = cpool.tile([128, 32], F32)
        ident = cpool.tile([128, 128], F32)
        make_identity(nc, ident[:])
        ident16 = cpool.tile([128, 128], F16)
        nc.vector.tensor_copy(out=ident16[:], in_=ident[:])
        for nm, sb in [("w2t", w2), ("w3t", w3),
                       ("gb1", gb1), ("gb2", gb2), ("gb3", gb3),
                       ("selw", selw), ("eout", eout)]:
            nc.sync.dma_start(out=sb[:], in_=d[nm][:])

        spool = ctx.enter_context(tc.tile_pool(name="stats", bufs=1))
        sm1 = spool.tile([C1, NT * 16], F32)
        sq1 = spool.tile([128, NT], F32)
        sm2 = spool.tile([C2, NT * 16], F32)
        sq2 = spool.tile([128, NT], F32)
        sm3 = spool.tile([C3, NT * 16], F32)
        sq3 = spool.tile([C3, NT * 2], F32)
        ab1 = spool.tile([128, 2], F32)   # col0 = scale a, col1 = bias b (dup at 64)
        ab2 = spool.tile([128, 2], F32)
        ab3 = spool.tile([C3, 2], F32)

        # ---------------- Phase 1: knn + gather + feat + conv1 ----------------
        with tc.tile_pool(name="p1c", bufs=1) as p1c, \
             tc.tile_pool(name="p1m", bufs=2) as mpool, \
             tc.tile_pool(name="p1psum", bufs=3, space="PSUM") as pp, \
             tc.tile_pool(name="p1tp", bufs=1, space="PSUM") as tpp, \
             tc.tile_pool(name="p1cpsum", bufs=3, space="PSUM") as cp, \
             tc.tile_pool(name="p1feat", bufs=1) as fpool, \
             tc.tile_pool(name="p1work", bufs=3) as wp, \
             tc.tile_pool(name="p1tt", bufs=2) as ttp, \
             tc.tile_pool(name="p1y", bufs=2) as yp:
            # phase-1-only constants (pool closes after phase 1, freeing
            # SBUF for the later phases' double buffers)
            tt = p1c.tile([36, N], F32)     # t1 rows 0-3, t2 rows 32-35
            t1 = tt[0:4, :]
            t2 = tt[32:36, :]
            gt = p1c.tile([128, N], F32)
            gt2 = p1c.tile([128, N], F32)
            qfc = p1c.tile([36, QPC], F32)  # qf dup'd at rows 0-3 and 32-35
            nqsq = p1c.tile([128, NT], F32)
            w1 = p1c.tile([4, C1], F32R)
            sel16 = p1c.tile([16, 512], F32R)
            # small consts first: the distance matmuls need only
            # tt/qfc/nqsq -- don't queue them behind the 4MB gt/gt2 loads
            nc.sync.dma_start(out=tt[0:4, :], in_=d["t1"][:])
            nc.sync.dma_start(out=tt[32:36, :], in_=d["t2"][:])
            nc.sync.dma_start(out=qfc[0:4, :], in_=d["qf"][:])
            nc.sync.dma_start(out=qfc[32:36, :], in_=d["qf"][:])
            for nm, sb in [("nqsq", nqsq), ("w1t", w1), ("sel16", sel16),
                           ("gt", gt), ("gt2", gt2)]:
                nc.sync.dma_start(out=sb[:], in_=d[nm][:])
            msbs = {}

            def emit_dist(t, kn, msb):
                # distance matmuls + msb copies for (t, kn)
                tab = (t1, t2)[kn]
                qfk = qfc[32 * kn:32 * kn + 4, :]
                for ch in range(8):
                    pm = pp.tile([128, 512], F32, tag="pm")
                    nc.tensor.matmul(
                        out=pm[:],
                        lhsT=qfk[:, t * 128:(t + 1) * 128],
                        rhs=tab[:, ch * 512:(ch + 1) * 512],
                        start=True, stop=True,
                    )
                    nc.scalar.activation(
                        out=msb[:, ch * 512:(ch + 1) * 512], in_=pm[:],
                        func=AF.Identity, bias=nqsq[:, t:t + 1])

            def start_tile(t):
                for kn in (0, 1):
                    m = mpool.tile([128, N], F32, tag=f"msb{kn}")
                    msbs[(t, kn)] = m
                    emit_dist(t, kn, m)

            start_tile(0)
            for t in range(NT):
                # software pipeline: issue tile t+1's distance stages (both
                # knns) ahead of tile t's topk/conv1 chain
                if t + 1 < NT:
                    start_tile(t + 1)
                mA = msbs.pop((t, 0))
                mB = msbs.pop((t, 1))
                termt = ttp.tile([16, 8 * C1], F32R, tag="termt")
                nc.sync.dma_start(
                    out=termt[:],
                    in_=d["termt"][:, t * 8 * C1:(t + 1) * 8 * C1])
                vals = wp.tile([128, 64], F32, tag="vals")
                idxu = wp.tile([128, 64], U16, tag="idxu")
                idxi = wp.tile([128, 64], I16, tag="idxi")
                # two-level top-32 (exact except when one 128-candidate chunk
                # holds >8 of a query's true top-32: P ~ 3e-5 per query):
                #   L1: top-8 of each of 32 chunks of 128 -> 256 candidates
                #   L2: top-32 of the candidates via max8+match_replace rounds
                #   FIND: global indices via find_index8 on the full row
                # 32 chunks of 128: P(a query's true top-32 has >8 members in
                # one chunk) ~ 3e-5; 16 chunks of 256 pushes rel err over the
                # 2e-2 budget (measured 3.1e-2) -- keep 32.
                NCH = 32
                CW = N // NCH
                l1a = wp.tile([128, NCH * 8], F32, tag="l1v0")
                l1b = wp.tile([128, NCH * 8], F32, tag="l1v1")
                l1 = {0: l1a, 1: l1b}
                for c in range(NCH):
                    for kn, m in ((0, mA), (1, mB)):
                        nc.vector.max(
                            out=l1[kn][:, c * 8:(c + 1) * 8],
                            in_=m[:, c * CW:(c + 1) * CW])
                for r in range(4):
                    for kn in (0, 1):
                        v8 = vals[:, kn * 32 + r * 8: kn * 32 + r * 8 + 8]
                        nc.vector.max(out=v8, in_=l1[kn][:])
                    if r < 3:
                        for kn in (0, 1):
                            v8 = vals[:, kn * 32 + r * 8: kn * 32 + r * 8 + 8]
                            nc.vector.match_replace(
                                out=l1[kn][:], in_to_replace=v8,
                                in_values=l1[kn][:], imm_value=NEG)
                for r in range(4):
                    for kn, m in ((0, mA), (1, mB)):
                        v8 = vals[:, kn * 32 + r * 8: kn * 32 + r * 8 + 8]
                        i8 = idxu[:, kn * 32 + r * 8: kn * 32 + r * 8 + 8]
                        nc.vector.max_index(out=i8, in_max=v8, in_values=m[:])
                nc.vector.tensor_copy(out=idxi[:], in_=idxu[:])

                # gather neighbor coords; both tables carry xyz on band rows
                # 16g+{0..2} (gt = p1 for knn1, gt2 = p2 for knn2); spill raw
                # for the fusion phase
                g1 = wp.tile([128, 512], F32, tag="g1")
                g2 = wp.tile([128, 512], F32, tag="g2")
                nc.gpsimd.ap_gather(
                    out_ap=g1[:], in_ap=gt[:], idxs_ap=idxi[:, 0:32],
                    channels=128, num_elems=N, d=1, num_idxs=512)
                nc.gpsimd.ap_gather(
                    out_ap=g2[:], in_ap=gt2[:], idxs_ap=idxi[:, 32:64],
                    channels=128, num_elems=N, d=1, num_idxs=512)
                nc.sync.dma_start(out=d["g1d"][t], in_=g1[:])
                nc.sync.dma_start(out=d["g2d"][t], in_=g2[:])

                # conv1 rhs must start at partition 0: DMA bands into a flat
                # [4, 8192] tile (raw nn coords; the -q term is folded into
                # the conv1 matmul).  Band copies split across ACT / GpSimd
                # descriptor queues to keep them off the SP sequencer.
                feat = fpool.tile([4, 8192], F32R, tag="feat")
                for g in range(8):
                    nc.scalar.dma_start(
                        out=feat[0:3, g * 512:(g + 1) * 512],
                        in_=g1[16 * g: 16 * g + 3, :].bitcast(F32R))
                    nc.gpsimd.dma_start(
                        out=feat[0:3, (8 + g) * 512:(9 + g) * 512],
                        in_=g2[16 * g: 16 * g + 3, :].bitcast(F32R))

                # dist = sqrt(max(-val, 0)) into feat row 3
                d2 = wp.tile([128, 64], F32, tag="d2")
                nc.vector.tensor_scalar(
                    out=d2[:], in0=vals[:], scalar1=-1.0,
                    scalar2=0.0, op0=OP.mult, op1=OP.max)
                nc.scalar.activation(out=d2[:], in_=d2[:], func=AF.Sqrt)
                # shuffle dist to pixel layout: PE-transpose to [nbr, query],
                # then ONE batched DMA per knn half (dst iterates (s, g, p))
                dtp = tpp.tile([64, 128], F32, tag="dtp")
                nc.tensor.transpose(out=dtp[:], in_=d2[:], identity=ident[:])
                d2t = wp.tile([64, 128], F32, tag="d2t")
                nc.scalar.activation(out=d2t[:], in_=dtp[:], func=AF.Identity)
                for kn in (0, 1):
                    for g in range(8):
                        c = kn * 8 + g
                        eng = (nc.sync, nc.scalar, nc.gpsimd)[c % 3]
                        eng.dma_start(
                            out=feat[3:4, c * 512:(c + 1) * 512]
                                .rearrange("c (s p) -> c s p", s=32),
                            in_=d2t[kn * 32:(kn + 1) * 32,
                                    16 * g:16 * g + 16].bitcast(F32R))

                # conv1: 16 chunks -> y1 packed [128, 4096] fp16; second
                # matmul accumulates the host-precomputed -W1[:, :3] @ q term
                y1 = yp.tile([128, 4096], F16, tag="y1")
                for c in range(16):
                    g = c % 8
                    bp_, fo = _pk(c)
                    pc = cp.tile([C1, 512], F32, tag="pc1")
                    nc.tensor.matmul(
                        out=pc[:],
                        lhsT=w1[:],
                        rhs=feat[:, c * 512:(c + 1) * 512],
                        start=True, stop=False)
                    nc.tensor.matmul(
                        out=pc[:],
                        lhsT=termt[:, g * C1:(g + 1) * C1],
                        rhs=sel16[:],
                        start=False, stop=True)
                    nc.scalar.activation(
                        out=y1[bp_:bp_ + 64, fo:fo + 512], in_=pc[:],
                        func=AF.Identity,
                        accum_out=sm1[:, t * 16 + c: t * 16 + c + 1])
                # sumsq pass; output recycles the (dead) mA tile
                nc.scalar.activation(
                    out=mA[:].bitcast(F16)[:, 0:4096], in_=y1[:],
                    func=AF.Square, accum_out=sq1[:, t:t + 1])
                nc.sync.dma_start(out=d["y1d"][t], in_=y1[:])

        _bn_allreduce(tc, 0, sm1, sq1, gb1, ab1, d["arin0"], d["arout0"],
                      dup=True, fold_sq=True, fold_sm=False)

        # ---------------- Phase 2: apply BN1+relu, conv2 ----------------
        with tc.tile_pool(name="p2y", bufs=3) as yp, \
             tc.tile_pool(name="p2psum", bufs=6, space="PSUM") as cp:
            for t in range(NT):
                y1 = yp.tile([128, 4096], F16, tag="y1l")
                nc.sync.dma_start(out=y1[:], in_=d["y1d"][t])
                y1r = yp.tile([128, 4096], F16, tag="y1r")
                # bn1+relu on ACT, split into column halves so the first 8
                # conv matmuls start after only half the apply
                for hb in range(2):
                    cols = slice(hb * 2048, (hb + 1) * 2048)
                    nc.scalar.activation(
                        out=y1r[:, cols], in_=y1[:, cols], func=AF.Relu,
                        scale=ab1[:, 0:1], bias=ab1[:, 1:2])
                y2 = yp.tile([128, 4096], F16, tag="y2")
                for c in range(16):
                    bp_, fo = _pk(c)
                    pc = cp.tile([C2, 512], F32, tag="pc2")
                    nc.tensor.matmul(
                        out=pc[:], lhsT=w2[bp_:bp_ + 64, :],
                        rhs=y1r[bp_:bp_ + 64, fo:fo + 512],
                        start=True, stop=True)
                    slot = sm2[:, t * 16 + c: t * 16 + c + 1]
                    if c < 7:
                        nc.scalar.activation(
                            out=y2[bp_:bp_ + 64, fo:fo + 512], in_=pc[:],
                            func=AF.Identity, accum_out=slot)
                    else:
                        nc.vector.tensor_scalar(
                            out=y2[bp_:bp_ + 64, fo:fo + 512], in0=pc[:],
                            scalar1=1.0, scalar2=0.0,
                            op0=OP.mult, op1=OP.add, accum_out=slot)
                # sumsq pass; output recycles the y1 tile
                if USE_STT_SUMSQ:
                    nc.vector.scalar_tensor_tensor(
                        out=y1[:], in0=y2[:], scalar=1.0, in1=y2[:],
                        op0=OP.mult, op1=OP.mult,
                        accum_out=sq2[:, t:t + 1])
                else:
                    nc.scalar.activation(
                        out=y1[:], in_=y2[:], func=AF.Square,
                        accum_out=sq2[:, t:t + 1])
                nc.sync.dma_start(out=d["y2d"][t], in_=y2[:])

        _bn_allreduce(tc, 1, sm2, sq2, gb2, ab2, d["arin1"], d["arout1"],
                      dup=True, fold_sq=True, fold_sm=False)

        # ---------------- Phase 3: apply BN2+relu, conv3 ----------------
        with tc.tile_pool(name="p3y", bufs=3) as yp, \
             tc.tile_pool(name="p3y2", bufs=3) as y2p, \
             tc.tile_pool(name="p3psum", bufs=6, space="PSUM") as cp:
            for t in range(NT):
                y2 = y2p.tile([128, 4096], F16, tag="y2l")
                nc.sync.dma_start(out=y2[:], in_=d["y2d"][t])
                y2r = y2p.tile([128, 4096], F16, tag="y2r")
                for hb in range(2):
                    cols = slice(hb * 2048, (hb + 1) * 2048)
                    nc.scalar.activation(
                        out=y2r[:, cols], in_=y2[:, cols], func=AF.Relu,
                        scale=ab2[:, 0:1], bias=ab2[:, 1:2])
                y3 = yp.tile([C3, 8192], F16, tag="y3")
                for c in range(16):
                    bp_, fo = _pk(c)
                    pc = cp.tile([C3, 512], F32, tag="pc3")
                    nc.tensor.matmul(
                        out=pc[:], lhsT=w3[bp_:bp_ + 64, :],
                        rhs=y2r[bp_:bp_ + 64, fo:fo + 512],
                        start=True, stop=True)
                    slot = sm3[:, t * 16 + c: t * 16 + c + 1]
                    if c < 8:
                        nc.scalar.activation(
                            out=y3[:, c * 512:(c + 1) * 512], in_=pc[:],
                            func=AF.Identity, accum_out=slot)
                    else:
                        nc.vector.tensor_scalar(
                            out=y3[:, c * 512:(c + 1) * 512], in0=pc[:],
                            scalar1=1.0, scalar2=0.0,
                            op0=OP.mult, op1=OP.add, accum_out=slot)
                # sumsq halves; outputs recycle y2l / y2r
                if USE_STT_SUMSQ:
                    nc.vector.scalar_tensor_tensor(
                        out=y2[:], in0=y3[:, 0:4096], scalar=1.0,
                        in1=y3[:, 0:4096], op0=OP.mult, op1=OP.mult,
                        accum_out=sq3[:, 2 * t:2 * t + 1])
                    nc.vector.scalar_tensor_tensor(
                        out=y2r[:], in0=y3[:, 4096:8192], scalar=1.0,
                        in1=y3[:, 4096:8192], op0=OP.mult, op1=OP.mult,
                        accum_out=sq3[:, 2 * t + 1:2 * t + 2])
                else:
                    nc.scalar.activation(
                        out=y2[:], in_=y3[:, 0:4096], func=AF.Square,
                        accum_out=sq3[:, 2 * t:2 * t + 1])
                    nc.scalar.activation(
                        out=y2r[:], in_=y3[:, 4096:8192], func=AF.Square,
                        accum_out=sq3[:, 2 * t + 1:2 * t + 2])
                nc.sync.dma_start(out=d["y3d"][t], in_=y3[:])

        _bn_allreduce(tc, 2, sm3, sq3, gb3, ab3, d["arin2"], d["arout2"],
                      dup=False, fold_sq=False, fold_sm=False)

        # ------------- Phase 4: scores, softmax, fusion, output -------------
        with tc.tile_pool(name="p4y", bufs=2) as yp, \
             tc.tile_pool(name="p4yf", bufs=2) as yfp, \
             tc.tile_pool(name="p4work", bufs=2) as wp, \
             tc.tile_pool(name="p4par", bufs=2) as parp, \
             tc.tile_pool(name="p4tree", bufs=3) as trp, \
             tc.tile_pool(name="p4tp", bufs=2, space="PSUM") as tp4, \
             tc.tile_pool(name="p4tps", bufs=1, space="PSUM") as tps, \
             tc.tile_pool(name="p4psum", bufs=2, space="PSUM") as pp4, \
             tc.tile_pool(name="p4opsum", bufs=1, space="PSUM") as opp, \
             tc.tile_pool(name="p4out", bufs=1) as op_:
            outsb = op_.tile([4, QPC], F32)
            for t in range(NT):
                y3 = yp.tile([C3, 8192], F16, tag="y3l")
                nc.sync.dma_start(out=y3[:], in_=d["y3d"][t])
                # bn3 apply WITH relu folded in (relu commutes with the
                # channel-max since it is monotone)
                scA = wp.tile([8, 512], F32, tag="scA")
                scB = wp.tile([8, 512], F32, tag="scB")
                ydt = F16 if PAR_MODE in ("f16q", "mixed") else F32
                y3f = yfp.tile([C3, 8192], ydt, tag="y3f")
                # split the apply across ACT and DVE halves to halve the
                # per-tile load->apply->reduce chain latency
                nc.scalar.activation(
                    out=y3f[:, 0:4096], in_=y3[:, 0:4096], func=AF.Relu,
                    scale=ab3[:, 0:1], bias=ab3[:, 1:2])
                nc.vector.tensor_scalar(
                    out=y3f[:, 4096:8192], in0=y3[:, 4096:8192],
                    scalar1=ab3[:, 0:1], scalar2=ab3[:, 1:2],
                    op0=OP.mult, op1=OP.add)
                nc.vector.tensor_scalar_max(
                    y3f[:, 4096:8192], y3f[:, 4096:8192], 0.0)
                if HYBRID_CMAX:
                    # per-tile split channel-max: kn0 half via two gpsimd
                    # partition-reduces (serial chain halved vs four), kn1
                    # half via PE transposes (idle Tensor) + DVE max-reduces
                    # straight from PSUM
                    for q in range(2):
                        par = parp.tile([128, 2048], F32, tag="par")
                        nc.gpsimd.partition_all_reduce(
                            out_ap=par[:],
                            in_ap=y3f[:, q * 2048:(q + 1) * 2048],
                            channels=128, reduce_op=bass_isa.ReduceOp.max)
                        nc.sync.dma_start(
                            out=scA[(q % 2) * 4:(q % 2) * 4 + 4, :],
                            in_=par[0:1, :].rearrange("c (g j) -> c g j", g=4))
                    sctT = wp.tile([128, 32], F32, tag="sctT")
                    for bk in range(8):
                        ptp = tp4.tile([128, 512], F16, tag="ptp")
                        for u in range(4):
                            j = 32 + bk * 4 + u
                            nc.tensor.transpose(
                                out=ptp[:, u * 128:(u + 1) * 128],
                                in_=y3f[:, j * 128:(j + 1) * 128],
                                identity=ident16[:])
                        nc.vector.tensor_reduce(
                            out=sctT[:, bk * 4:(bk + 1) * 4],
                            in_=ptp[:].rearrange("c (b p) -> c b p", b=4),
                            axis=mybir.AxisListType.X, op=OP.max)
                    # back to chunk-row layout: PE-transpose the small score
                    # tile, then one batched partition-collapse DMA
                    pts = tps.tile([32, 128], F32, tag="pts")
                    nc.tensor.transpose(
                        out=pts[:], in_=sctT[:], identity=ident[:])
                    scs = wp.tile([32, 128], F32, tag="scs")
                    nc.scalar.activation(
                        out=scs[:], in_=pts[:], func=AF.Identity)
                    nc.scalar.dma_start(
                        out=scB[:].rearrange("c (b p) -> c b p", b=4),
                        in_=scs[0:32, :])
                elif PAR_MODE == "mixed":
                    # channel-max split: gpsimd quarters 0-2, DVE f16
                    # max-tree (2x mode) for quarter 3
                    for q in range(3):
                        par = parp.tile([128, 2048], F32, tag="par")
                        nc.gpsimd.partition_all_reduce(
                            out_ap=par[:],
                            in_ap=y3f[:, q * 2048:(q + 1) * 2048],
                            channels=128, reduce_op=bass_isa.ReduceOp.max)
                        dst = scA if q < 2 else scB
                        eng = (nc.sync, nc.scalar, nc.sync)[q]
                        eng.dma_start(
                            out=dst[(q % 2) * 4:(q % 2) * 4 + 4, :],
                            in_=par[0:1, :].rearrange("c (g j) -> c g j", g=4))
                    # SB+SB tensor_tensor requires equal base partitions, so
                    # each tree level shifts the upper half down via DMA on
                    # the idle sync/scalar queues (NOT the busy Pool queue)
                    tmp = trp.tile([64, 2048], F16, tag="tmtree")
                    sh = trp.tile([64, 2048], F16, tag="shtree")
                    nc.sync.dma_start(
                        out=sh[0:64, :], in_=y3f[64:128, 6144:8192])
                    nc.vector.tensor_tensor(
                        out=tmp[:], in0=y3f[0:64, 6144:8192],
                        in1=sh[0:64, :], op=OP.max)
                    tm32 = trp.tile([1, 2048], F32, tag="tm32")
                    lv = 32
                    while lv >= 1:
                        eng = (nc.sync, nc.scalar)[lv % 2]
                        eng.dma_start(
                            out=sh[0:lv, :], in_=tmp[lv:2 * lv, :])
                        if lv == 1:
                            nc.vector.tensor_tensor(
                                out=tm32[:], in0=tmp[0:1, :],
                                in1=sh[0:1, :], op=OP.max)
                        else:
                            nc.vector.tensor_tensor(
                                out=tmp[0:lv, :], in0=tmp[0:lv, :],
                                in1=sh[0:lv, :], op=OP.max)
                        lv //= 2
                    nc.scalar.dma_start(
                        out=scB[4:8, :],
                        in_=tm32[:].rearrange("c (g j) -> c g j", g=4))
                else:
                    for q in range(4):
                        par = parp.tile([128, 2048], F32, tag="par")
                        nc.gpsimd.partition_all_reduce(
                            out_ap=par[:],
                            in_ap=y3f[:, q * 2048:(q + 1) * 2048],
                            channels=128, reduce_op=bass_isa.ReduceOp.max)
                        dst = scA if q < 2 else scB
                        eng = (nc.sync, nc.scalar)[q % 2]
                        eng.dma_start(
                            out=dst[(q % 2) * 4:(q % 2) * 4 + 4, :],
                            in_=par[0:1, :].rearrange("c (g j) -> c g j", g=4))
                # softmax over the 64 neighbors of each query. The max
                # subtraction is skipped: scores are relu'd >= 0 and bounded
                # (BN-normalized channel maxes, << 88), so exp cannot
                # overflow f32. Normalization is deferred to the tiny
                # [128, 16] fusion output (weights stay unnormalized here).
                exA = wp.tile([8, 512], F32, tag="exA")
                exB = wp.tile([8, 512], F32, tag="exB")
                for sct, ext in ((scA, exA), (scB, exB)):
                    nc.scalar.activation(out=ext[:], in_=sct[:], func=AF.Exp)
                esA = wp.tile([8, 16], F32, tag="esA")
                esB = wp.tile([8, 16], F32, tag="esB")
                for ext, est in ((exA, esA), (exB, esB)):
                    nc.vector.tensor_reduce(
                        out=est[:],
                        in_=ext[:].rearrange("c (s p) -> c p s", s=32),
                        axis=mybir.AxisListType.X, op=OP.add)
                nc.vector.tensor_tensor(
                    out=esA[:], in0=esA[:], in1=esB[:], op=OP.add)
                nc.vector.reciprocal(out=esA[:], in_=esA[:])
                # replicate 1/wsum onto band partitions via a selector matmul
                pe = pp4.tile([128, 16], F32, tag="pe")
                nc.tensor.matmul(out=pe[:], lhsT=selw[:], rhs=esA[:],
                                 start=True, stop=True)
                per = wp.tile([128, 16], F32, tag="per")
                nc.scalar.activation(out=per[:], in_=pe[:], func=AF.Identity)
                # fusion: replicate weight rows onto band partitions via a
                # selector matmul, multiply with raw coords, segment-reduce
                g1 = wp.tile([128, 512], F32, tag="g1l")
                g2 = wp.tile([128, 512], F32, tag="g2l")
                nc.sync.dma_start(out=g1[:], in_=d["g1d"][t])
                nc.sync.dma_start(out=g2[:], in_=d["g2d"][t])
                wr1 = wp.tile([128, 512], F32, tag="wr1")
                wr2 = wp.tile([128, 512], F32, tag="wr2")
                for ext, wr in ((exA, wr1), (exB, wr2)):
                    pw = pp4.tile([128, 512], F32, tag="pw")
                    nc.tensor.matmul(
                        out=pw[:], lhsT=selw[:],
                        rhs=ext[:], start=True, stop=True)
                    nc.scalar.activation(out=wr[:], in_=pw[:], func=AF.Identity)
                pr = wp.tile([128, 512], F32, tag="pr")
                nc.vector.tensor_tensor(out=pr[:], in0=g1[:], in1=wr1[:],
                                        op=OP.mult)
                nc.vector.tensor_tensor(out=wr2[:], in0=g2[:], in1=wr2[:],
                                        op=OP.mult)
                nc.vector.tensor_tensor(out=pr[:], in0=pr[:], in1=wr2[:],
                                        op=OP.add)
                fp = wp.tile([128, 16], F32, tag="fp")
                nc.vector.tensor_reduce(
                    out=fp[:], in_=pr[:].rearrange("c (s p) -> c p s", s=32),
                    axis=mybir.AxisListType.X, op=OP.add)
                nc.vector.tensor_tensor(out=fp[:], in0=fp[:], in1=per[:],
                                        op=OP.mult)
                # outsb[c, t*128 + g*16 + p] = fp[16g+c, p] via selector mms
                po = opp.tile([4, 128], F32, tag="po")
                for g in range(8):
                    nc.tensor.matmul(
                        out=po[:, g * 16:(g + 1) * 16],
                        lhsT=eout[:, g * 4:(g + 1) * 4],
                        rhs=fp[:], start=True, stop=True)
                nc.scalar.activation(
                    out=outsb[0:3, t * 128:(t + 1) * 128], in_=po[0:3, :],
                    func=AF.Identity)
            nc.sync.dma_start(out=d["out"][:], in_=outsb[0:3, :])


def _bn_allreduce(tc, li, sm, sq, gbe, ab, arin, arout, dup, fold_sq, fold_sm):
    """Reduce per-chunk/per-tile stat slots, AllReduce across 8 cores, compute
    per-channel scale a = g*rsqrt(var+eps) and bias b = be - a*mean.

    fold_*: the stat tile is [128, S] over PACKED partitions (64 even-chunk
    channels at 0..64, odd at 64..128) -> fold halves with a partition-shift
    DMA + add."""
    nc = tc.nc
    C = gbe.shape[0]
    with tc.tile_pool(name=f"bn{li}", bufs=1) as bp:
        st = bp.tile([C, 2], F32)

        def reduce_into(src, fold, col):
            r = bp.tile([128, 1], F32, tag=f"r{li}{col}")
            nc.vector.tensor_reduce(out=r[0:src.shape[0], :], in_=src[:],
                                    axis=mybir.AxisListType.X, op=OP.add)
            if fold:
                hi = bp.tile([64, 1], F32, tag=f"h{li}{col}")
                nc.sync.dma_start(out=hi[:], in_=r[64:128, :])
                nc.vector.tensor_tensor(out=st[:, col:col + 1], in0=r[0:64, :],
                                        in1=hi[:], op=OP.add)
            else:
                nc.vector.tensor_copy(out=st[:, col:col + 1], in_=r[0:C, :])

        reduce_into(sm, fold_sm, 0)
        reduce_into(sq, fold_sq, 1)
        nc.sync.dma_start(out=arin[:], in_=st[:])
        if getattr(nc, "_single_core_nocoll", False):
            nc.sync.dma_start(out=arout[:], in_=arin[:])
        else:
            nc.gpsimd.collective_compute(
                "AllReduce", OP.add, replica_groups=[list(range(NCORES))],
                ins=[arin.opt()], outs=[arout.opt()])
        ar = bp.tile([C, 2], F32)
        nc.sync.dma_start(out=ar[:], in_=arout[:])
        mean = bp.tile([C, 1], F32)
        var = bp.tile([C, 1], F32)
        nc.vector.tensor_scalar_mul(mean[:], ar[:, 0:1], 1.0 / NTOT)
        nc.vector.tensor_scalar_mul(var[:], ar[:, 1:2], 1.0 / NTOT)
        m2 = bp.tile([C, 1], F32)
        nc.vector.tensor_tensor(out=m2[:], in0=mean[:], in1=mean[:], op=OP.mult)
        nc.vector.tensor_tensor(out=var[:], in0=var[:], in1=m2[:], op=OP.subtract)
        nc.vector.tensor_scalar_add(var[:], var[:], BN_EPS)
        nc.scalar.activation(out=var[:], in_=var[:], func=AF.Sqrt)
        nc.vector.reciprocal(out=var[:], in_=var[:])  # rsqrt(var+eps)
        nc.vector.tensor_tensor(out=ab[0:C, 0:1], in0=var[:], in1=gbe[:, 0:1],
                                op=OP.mult)            # a
        nc.vector.tensor_tensor(out=m2[:], in0=ab[0:C, 0:1], in1=mean[:],
                                op=OP.mult)
        nc.vector.tensor_tensor(out=ab[0:C, 1:2], in0=gbe[:, 1:2], in1=m2[:],
                                op=OP.subtract)        # b = be - a*mean
        if dup:
            nc.vector.tensor_copy(out=ab[C:2 * C, :], in_=ab[0:C, :])


_PROGRAM = None
LAST_RESULT = None


def _get_program():
    global _PROGRAM
    if _PROGRAM is None:
        _PROGRAM = _build_program()
    return _PROGRAM


def _prep_core_inputs(points1, points2, W1, W2, W3, gs, bes, b, h):
    p1 = points1[b]          # [3, N]
    p2 = points2[b]
    q = p1[:, h * QPC:(h + 1) * QPC]            # [3, QPC]
    qf = np.concatenate([2.0 * q, np.ones((1, QPC), np.float32)], axis=0)

    def cand_tab(p):
        sq = (p * p).sum(axis=0, keepdims=True)
        return np.concatenate([p, -sq], axis=0).astype(np.float32)  # [4, N]

    gtab = np.zeros((128, N), np.float32)
    gtab2 = np.zeros((128, N), np.float32)
    for g in range(8):
        gtab[16 * g + 0:16 * g + 3] = p1
        gtab2[16 * g + 0:16 * g + 3] = p2
    nqsqv = (-(q * q).sum(axis=0)).reshape(NT, 128).T.astype(np.float32)

    def dup128(w):      # [64, C] -> [128, C] duplicated
        return np.concatenate([w, w], axis=0).astype(np.float32)

    selw = np.zeros((8, 128), np.float32)
    for g in range(8):
        for c3 in range(3):
            selw[g, 16 * g + c3] = 1.0

    # termt[:, (t*8+g)*64 : +64] = (-W1[:, :3] @ q_block).T   [16, 64]
    termt = np.zeros((16, NT * 8 * C1), np.float32)
    w13 = W1[:, 0:3]                                  # [64, 3]
    for t in range(NT):
        for g in range(8):
            qblk = q[:, t * 128 + g * 16: t * 128 + (g + 1) * 16]  # [3, 16]
            termt[:, (t * 8 + g) * C1:(t * 8 + g + 1) * C1] = \
                -(w13 @ qblk).T
    sel16 = np.tile(np.eye(16, dtype=np.float32), 32)  # [16, 512]

    eoutv = np.zeros((128, 32), np.float32)
    for g in range(8):
        for c3 in range(3):
            eoutv[16 * g + c3, g * 4 + c3] = 1.0

    return {
        "selw": selw,
        "qf": qf.astype(np.float32),
        "t1": cand_tab(p1), "t2": cand_tab(p2), "gt": gtab, "gt2": gtab2,
        "nqsq": np.ascontiguousarray(nqsqv),
        "termt": termt, "sel16": sel16, "eout": eoutv,
        "w1t": np.ascontiguousarray(W1.T).astype(np.float32),
        "w2t": dup128(np.ascontiguousarray(W2.T)).astype(np.float16),
        "w3t": dup128(np.ascontiguousarray(W3.T)).astype(np.float16),
        "gb1": np.stack([gs[0], bes[0]], axis=1).astype(np.float32),
        "gb2": np.stack([gs[1], bes[1]], axis=1).astype(np.float32),
        "gb3": np.stack([gs[2], bes[2]], axis=1).astype(np.float32),
    }


def kernel(points1, points2, k, t, W1, b1, g1, be1, W2, b2, g2, be2,
           W3, b3, g3, be3):
    # b1/b2/b3 cancel inside train-mode BatchNorm; t is unused by the net.
    assert int(np.asarray(k)) == KNN
    points1 = np.asarray(points1, np.float32)
    points2 = np.asarray(points2, np.float32)
    gs = [np.asarray(g1, np.float32), np.asarray(g2, np.float32),
          np.asarray(g3, np.float32)]
    bes = [np.asarray(be1, np.float32), np.asarray(be2, np.float32),
           np.asarray(be3, np.float32)]
    Ws = [np.asarray(W1, np.float32), np.asarray(W2, np.float32),
          np.asarray(W3, np.float32)]

    in_maps = []
    for c in range(NCORES):
        b, h = divmod(c, 2)
        in_maps.append(_prep_core_inputs(points1, points2, *Ws, gs, bes, b, h))

    nc = _get_program()
    bkr = run_bass_kernel_spmd(nc, in_maps, list(range(NCORES)))
    global LAST_RESULT
    LAST_RESULT = bkr
    res = bkr.results

    out = np.zeros((B, 3, N), np.float32)
    for c in range(NCORES):
        b, h = divmod(c, 2)
        out[b, :, h * QPC:(h + 1) * QPC] = res[c]["out"]
    return out
